# revision 6
# baseline (speedup 1.0000x reference)
"""Clover-Wilson Dirac operator on Trainium2 (8 NeuronCores, T-sharded).

Math summary (derived + numerically verified against the reference):
- The reference's 4-leaf "clover" Q for plane (mu,nu) factorizes as
      Q(x) = W(x) + W(x+d1)^+ + W(x+d2)^+ + W(x+d3)^+
  with W(x) = [U_mu(x) U_nu(x+mu)] [U_nu(x) U_mu(x+nu)]^+,
  d1 = nu-mu, d2 = -2mu-2nu, d3 = -2nu (unit lattice vectors).
- With G = W - W^+ (anti-Hermitian), Ftil := Q - Q^+ = G(x) - G(x+d1) - G(x+d2) - G(x+d3).
- C psi + (4+m) psi = (5+m) psi + (csw/32) * sum_p (sigma_p (x) (-i Ftil_p)) psi,
  where sigma_p is block-diagonal (2x2 chiral blocks) in this basis.
- Wilson hop uses the standard spin-projection trick (2 half-spinors per direction).

Distribution: T=32 sharded 4 slices per core; U needs halo t0-2..t0+4 (7 slices),
psi needs t0-1..t0+4. All jnp.roll shifts are pushed into host-precomputed
pre-rolled planar fp16 arrays; on-device shifted reads of the intermediate G
use DRAM->DRAM affine shuffle DMAs.
"""
import numpy as np

T, Z, Y, X = 32, 24, 24, 24
NCOL, NS = 3, 4
MASS, CSW = 0.1, 1.0
PAIRS = [(0, 1), (0, 2), (0, 3), (1, 2), (1, 3), (2, 3)]
NCORES = 8
TLOC = T // NCORES          # 4 output slices per core
NSITE = Z * Y * X           # 13824
P = 128
F = NSITE // P              # 108
NWIN = 7                    # U window slices: t0-2 .. t0+4
DIAG = 5.0 + MASS           # (4+m) + clover identity
CCLOV = CSW / 32.0          # |coefficient| of sigma (x) Ftil; overall factor -i


# ----------------------------------------------------------------- tables

def _gammas():
    i = 1j
    g0 = np.array([[0, 0, 1, 0], [0, 0, 0, 1], [1, 0, 0, 0], [0, 1, 0, 0]], np.complex128)
    g1 = np.array([[0, 0, 0, i], [0, 0, i, 0], [0, -i, 0, 0], [-i, 0, 0, 0]], np.complex128)
    g2 = np.array([[0, 0, 0, -1], [0, 0, 1, 0], [0, 1, 0, 0], [-1, 0, 0, 0]], np.complex128)
    g3 = np.array([[0, 0, i, 0], [0, 0, 0, -i], [-i, 0, 0, 0], [0, i, 0, 0]], np.complex128)
    return [g0, g1, g2, g3]


def _sigma_blocks():
    """Chiral 2x2 blocks of sigma_{mu nu} = i g_mu g_nu for each plane."""
    G = _gammas()
    ups, dns = [], []
    for mu, nu in PAIRS:
        s = 1j * (G[mu] @ G[nu])
        assert np.abs(s[:2, 2:]).max() < 1e-12 and np.abs(s[2:, :2]).max() < 1e-12
        ups.append(s[:2, :2].copy())
        dns.append(s[2:, 2:].copy())
    return ups, dns


SIG_UP, SIG_DN = _sigma_blocks()

# per-plane shift deltas (t, z, y, x) for the W-factorization
def _deltas():
    out = []
    for mu, nu in PAIRS:
        e_mu = np.zeros(4, np.int64); e_mu[mu] = 1
        e_nu = np.zeros(4, np.int64); e_nu[nu] = 1
        out.append([tuple(e_nu - e_mu), tuple(-2 * e_mu - 2 * e_nu), tuple(-2 * e_nu)])
    return out


DELTAS = _deltas()

# debug toggles (affect both simulate_core and the device program)
ENABLE_CLOVER = True
ENABLE_HOP = True
DEBUG_DUMP = False

# hop projection tables: psi_h[c] = psi[c] + coef * psi[b[c]]; lower rows:
# row_{2+c} = rc[c] * h[m[c]]  (forward, i.e. (1-gamma)); backward negates
# coef and rc. Verified against gammas in _check_hop_tables().
HOP = {
    0: dict(b=(2, 3), coef=(-1, -1), m=(0, 1), rc=(-1, -1)),
    1: dict(b=(3, 2), coef=(-1j, -1j), m=(1, 0), rc=(1j, 1j)),
    2: dict(b=(3, 2), coef=(1, -1), m=(1, 0), rc=(-1, 1)),
    3: dict(b=(2, 3), coef=(-1j, 1j), m=(0, 1), rc=(1j, -1j)),
}


def _check_hop_tables():
    G = _gammas()
    for mu, t in HOP.items():
        for sgn in (+1, -1):  # +1: (1-g) fwd ; -1: (1+g) bwd
            M = np.eye(4) - sgn * G[mu]
            # build from table
            B = np.zeros((4, 4), np.complex128)
            for c in range(2):
                B[c, c] += 1
                B[c, t['b'][c]] += sgn * t['coef'][c]
            for c in range(2):
                rc = sgn * t['rc'][c]
                B[2 + c, t['m'][c]] += rc
                B[2 + c, t['b'][t['m'][c]]] += rc * sgn * t['coef'][t['m'][c]]
            assert np.abs(B - M).max() < 1e-12, (mu, sgn, B, M)


_check_hop_tables()


# ------------------------------------------------- planar layout helpers

def _to_planar_links(U):
    """U: (T,Z,Y,X,4,3,3) complex64 -> dict of fp16 planar arrays.

    Returns variants[key] = array [T, 18, NSITE] fp16 with comp c=(i*3+j)*2+r.
    Keys: ('c', d) centered; ('f', d, e) = U_d(x+e_hat) spatial e;
          ('b', d) = U_d(x - d_hat) spatial d.
    """
    Uf32 = np.ascontiguousarray(U)  # complex64
    planar = np.empty((4, T, 18, NSITE), np.float16)
    Um = Uf32.reshape(T, NSITE, 4, 9)
    for d in range(4):
        re = Um[..., d, :].real.astype(np.float16)  # (T, NSITE, 9)
        im = Um[..., d, :].imag.astype(np.float16)
        planar[d, :, 0::2, :] = re.transpose(0, 2, 1)
        planar[d, :, 1::2, :] = im.transpose(0, 2, 1)

    def roll_sites(arr, delta):  # arr [..., NSITE]; value at x+delta
        dz, dy, dx = delta
        a = arr.reshape(*arr.shape[:-1], Z, Y, X)
        if dz: a = np.roll(a, -dz, axis=-3)
        if dy: a = np.roll(a, -dy, axis=-2)
        if dx: a = np.roll(a, -dx, axis=-1)
        return a.reshape(*arr.shape[:-1], NSITE)

    variants = {}
    for d in range(4):
        variants[('c', d)] = planar[d]
    needed_f = {(0, 1), (0, 2), (0, 3), (2, 1), (3, 1), (3, 2), (1, 2), (1, 3), (2, 3)}
    for (d, e) in needed_f:
        delta = [0, 0, 0]; delta[e - 1] = 1
        variants[('f', d, e)] = roll_sites(planar[d], delta)
    for d in (1, 2, 3):
        delta = [0, 0, 0]; delta[d - 1] = -1
        variants[('b', d)] = roll_sites(planar[d], delta)
    return variants


def _to_planar_psi(psi):
    """psi: (T,Z,Y,X,4,3) complex64 -> dict: ('c',) and ('s', e, sgn) ->
    [T, 24, NSITE] fp16, comp c=(s*3+cl)*2+r."""
    pm = psi.reshape(T, NSITE, 12)
    planar = np.empty((T, 24, NSITE), np.float16)
    planar[:, 0::2, :] = pm.real.astype(np.float16).transpose(0, 2, 1)
    planar[:, 1::2, :] = pm.imag.astype(np.float16).transpose(0, 2, 1)

    def roll_sites(arr, delta):
        dz, dy, dx = delta
        a = arr.reshape(*arr.shape[:-1], Z, Y, X)
        if dz: a = np.roll(a, -dz, axis=-3)
        if dy: a = np.roll(a, -dy, axis=-2)
        if dx: a = np.roll(a, -dx, axis=-1)
        return a.reshape(*arr.shape[:-1], NSITE)

    out = {('c',): planar}
    for e in (1, 2, 3):
        for sgn in (1, -1):
            delta = [0, 0, 0]; delta[e - 1] = sgn
            out[('s', e, sgn)] = roll_sites(planar, delta)
    return out


# ------------------------------------------------------ numpy simulator
# Step-wise fp16 mirror of the device dataflow (for validation).

def _cmm16(A, B, dag_b=False):
    """A,B: [18, N] fp16 planar 3x3 complex; returns C = A @ B(^+) fp16."""
    C = np.zeros_like(A)
    for i in range(3):
        for k in range(3):
            cre = np.zeros(A.shape[-1], np.float16)
            cim = np.zeros(A.shape[-1], np.float16)
            for j in range(3):
                ar = A[(i * 3 + j) * 2]; ai = A[(i * 3 + j) * 2 + 1]
                if dag_b:
                    br = B[(k * 3 + j) * 2]; bi = -B[(k * 3 + j) * 2 + 1].astype(np.float16)
                else:
                    br = B[(j * 3 + k) * 2]; bi = B[(j * 3 + k) * 2 + 1]
                cre = (cre + (ar * br - ai * bi)).astype(np.float16)
                cim = (cim + (ar * bi + ai * br)).astype(np.float16)
            C[(i * 3 + k) * 2] = cre
            C[(i * 3 + k) * 2 + 1] = cim
    return C


def _antiherm9(Wm):
    """W planar 18 -> G = W - W^+ in 9-comp layout:
    q*2 / q*2+1 = re/im of G[i,j] for (i,j) in [(0,1),(0,2),(1,2)]; 6+d = im G[d,d]."""
    G = np.empty((9, Wm.shape[-1]), np.float16)
    offd = [(0, 1), (0, 2), (1, 2)]
    for q, (i, j) in enumerate(offd):
        G[q * 2] = (Wm[(i * 3 + j) * 2] - Wm[(j * 3 + i) * 2]).astype(np.float16)
        G[q * 2 + 1] = (Wm[(i * 3 + j) * 2 + 1] + Wm[(j * 3 + i) * 2 + 1]).astype(np.float16)
    for d in range(3):
        G[6 + d] = (Wm[(d * 3 + d) * 2 + 1] * np.float16(2.0)).astype(np.float16)
    return G


def _f9_entry(F9, i, j):
    """(re, im) pair (arrays or (None, arr)) of Ftil[i,j] from 9-comp planar."""
    offd = {(0, 1): 0, (0, 2): 1, (1, 2): 2}
    if i == j:
        return None, F9[6 + i]
    if (i, j) in offd:
        q = offd[(i, j)]
        return F9[q * 2], F9[q * 2 + 1]
    q = offd[(j, i)]
    return -F9[q * 2], F9[q * 2 + 1]  # G[i>j] = -conj(G[j,i]) -> (-re, +im)


def _roll_sites_np(a, delta):
    dz, dy, dx = delta
    a = a.reshape(*a.shape[:-1], Z, Y, X)
    if dz: a = np.roll(a, -dz, axis=-3)
    if dy: a = np.roll(a, -dy, axis=-2)
    if dx: a = np.roll(a, -dx, axis=-1)
    return a.reshape(*a.shape[:-2], -1) if False else a.reshape(*a.shape[:-4], a.shape[-4] if a.ndim > 3 else -1, NSITE) if False else a.reshape(-1, NSITE) if a.ndim == 4 else a.reshape(NSITE)


def simulate_core(link_vars, psi_vars, t0):
    """Numpy fp16 mirror. link_vars/psi_vars: full-T variant dicts.
    Returns planar out [TLOC, 24, NSITE] float32."""
    tw = [(t0 - 2 + w) % T for w in range(NWIN)]

    def LV(key, w):
        return link_vars[key][tw[w]]

    def PV(key, w):
        return psi_vars[key][tw[w]]

    # ---- phase 1: G per plane per window slice
    Gs = {}
    for p, (mu, nu) in enumerate(PAIRS):
        ws = range(0, 6) if mu == 0 else range(2, 6)
        for w in ws:
            if mu == 0:
                M1, M2 = LV(('c', 0), w), LV(('c', nu), w + 1)
                M3, M4 = LV(('c', nu), w), LV(('f', 0, nu), w)
            else:
                M1, M2 = LV(('c', mu), w), LV(('f', nu, mu), w)
                M3, M4 = LV(('c', nu), w), LV(('f', mu, nu), w)
            A = _cmm16(M1, M2)
            B = _cmm16(M3, M4)
            Wm = _cmm16(A, B, dag_b=True)
            Gs[(p, w)] = _antiherm9(Wm)

    out = np.zeros((TLOC, 24, NSITE), np.float32)
    for o in range(TLOC):
        w = o + 2
        # ---- Ftil per plane
        F9s = []
        for p in range(6):
            acc = Gs[(p, w)].copy()
            for (dt, dz, dy, dx) in DELTAS[p]:
                g = Gs[(p, w + dt)]
                gsh = g.reshape(9, Z, Y, X)
                if dz: gsh = np.roll(gsh, -dz, axis=1)
                if dy: gsh = np.roll(gsh, -dy, axis=2)
                if dx: gsh = np.roll(gsh, -dx, axis=3)
                acc = (acc - gsh.reshape(9, NSITE)).astype(np.float16)
            F9s.append(acc)

        if not ENABLE_CLOVER:
            F9s = [np.zeros((9, NSITE), np.float16) for _ in range(6)]
        # ---- B blocks (full 6x6 complex per chirality block), fp16
        Bblk = [np.zeros((6, 6, 2, NSITE), np.float16) for _ in range(2)]
        for blk, sigs in enumerate((SIG_UP, SIG_DN)):
            for p in range(6):
                sig = sigs[p]
                for a in range(2):
                    for b in range(2):
                        s = sig[a, b]
                        if abs(s) < 1e-12:
                            continue
                        cf = -1j * CCLOV * s  # complex coefficient
                        for i in range(3):
                            for j in range(3):
                                fre, fim = _f9_entry(F9s[p], i, j)
                                A_, B_ = a * 3 + i, b * 3 + j
                                # coeff*(fre + i fim): accumulate re and im
                                cr, ci = cf.real, cf.imag
                                tgt = Bblk[blk][A_, B_]
                                if fre is not None:
                                    if cr: tgt[0] = (tgt[0] + np.float16(cr) * fre).astype(np.float16)
                                    if ci: tgt[1] = (tgt[1] + np.float16(ci) * fre).astype(np.float16)
                                if cr: tgt[1] = (tgt[1] + np.float16(cr) * fim).astype(np.float16)
                                if ci: tgt[0] = (tgt[0] - np.float16(ci) * fim).astype(np.float16)
            for A_ in range(6):
                Bblk[blk][A_, A_, 0] = (Bblk[blk][A_, A_, 0] + np.float16(DIAG)).astype(np.float16)

        # ---- apply B to psi
        psi_c = PV(('c',), w)
        for blk in range(2):
            for A_ in range(6):
                s_out = (blk * 2 + A_ // 3) * 3 + (A_ % 3)  # spinor comp index s*3+cl
                accr = np.zeros(NSITE, np.float16)
                acci = np.zeros(NSITE, np.float16)
                for B_ in range(6):
                    s_in = (blk * 2 + B_ // 3) * 3 + (B_ % 3)
                    pr = psi_c[s_in * 2]; pi = psi_c[s_in * 2 + 1]
                    br = Bblk[blk][A_, B_, 0]; bi = Bblk[blk][A_, B_, 1]
                    accr = (accr + br * pr - bi * pi).astype(np.float16)
                    acci = (acci + br * pi + bi * pr).astype(np.float16)
                out[o, s_out * 2] += accr.astype(np.float32)
                out[o, s_out * 2 + 1] += acci.astype(np.float32)

        # ---- hop terms
        for mu in (range(4) if ENABLE_HOP else ()):
            tbl = HOP[mu]
            for sgn, wpsi_key, woff, ukey, udag in (
                (+1, 'f', +1, ('c', mu), False),
                (-1, 'b', -1, ('b', mu) if mu else ('c', 0), True),
            ):
                if mu == 0:
                    psv = PV(('c',), w + woff)
                else:
                    psv = PV(('s', mu, +1 if sgn > 0 else -1), w)
                uar = LV(ukey, w) if mu else LV(ukey, w + (0 if sgn > 0 else -1))
                # project: h[c] = psi[c] + sgn*coef[c]*psi[b[c]] (2 spins x 3 col)
                h = np.zeros((2, 3, 2, NSITE), np.float16)
                for c in range(2):
                    cf = sgn * tbl['coef'][c]
                    for cl in range(3):
                        pr = psv[(c * 3 + cl) * 2]; pi = psv[(c * 3 + cl) * 2 + 1]
                        qr = psv[(tbl['b'][c] * 3 + cl) * 2]; qi = psv[(tbl['b'][c] * 3 + cl) * 2 + 1]
                        if cf == 1:
                            h[c, cl, 0] = (pr + qr).astype(np.float16); h[c, cl, 1] = (pi + qi).astype(np.float16)
                        elif cf == -1:
                            h[c, cl, 0] = (pr - qr).astype(np.float16); h[c, cl, 1] = (pi - qi).astype(np.float16)
                        elif cf == 1j:
                            h[c, cl, 0] = (pr - qi).astype(np.float16); h[c, cl, 1] = (pi + qr).astype(np.float16)
                        else:  # -1j
                            h[c, cl, 0] = (pr + qi).astype(np.float16); h[c, cl, 1] = (pi - qr).astype(np.float16)
                # color mult: uh[c, i] = sum_j U[i,j] h[c, j] (or U^+ )
                uh = np.zeros((2, 3, 2, NSITE), np.float16)
                for c in range(2):
                    for i in range(3):
                        ar = np.zeros(NSITE, np.float16); ai = np.zeros(NSITE, np.float16)
                        for j in range(3):
                            if udag:
                                ur = uar[(j * 3 + i) * 2]; ui = -uar[(j * 3 + i) * 2 + 1].astype(np.float16)
                            else:
                                ur = uar[(i * 3 + j) * 2]; ui = uar[(i * 3 + j) * 2 + 1]
                            ar = (ar + ur * h[c, j, 0] - ui * h[c, j, 1]).astype(np.float16)
                            ai = (ai + ur * h[c, j, 1] + ui * h[c, j, 0]).astype(np.float16)
                        uh[c, i, 0] = ar; uh[c, i, 1] = ai
                # accumulate: rows 0,1: -1/2*uh[c]; rows 2+c': -1/2*sgn... rc
                for c in range(2):
                    for cl in range(3):
                        out[o, (c * 3 + cl) * 2] -= 0.5 * uh[c, cl, 0].astype(np.float32)
                        out[o, (c * 3 + cl) * 2 + 1] -= 0.5 * uh[c, cl, 1].astype(np.float32)
                for cp in range(2):
                    rc = sgn * tbl['rc'][cp]
                    mm = tbl['m'][cp]
                    for cl in range(3):
                        tr = uh[mm, cl, 0].astype(np.float32); ti = uh[mm, cl, 1].astype(np.float32)
                        if rc == 1:
                            out[o, ((2 + cp) * 3 + cl) * 2] -= 0.5 * tr
                            out[o, ((2 + cp) * 3 + cl) * 2 + 1] -= 0.5 * ti
                        elif rc == -1:
                            out[o, ((2 + cp) * 3 + cl) * 2] += 0.5 * tr
                            out[o, ((2 + cp) * 3 + cl) * 2 + 1] += 0.5 * ti
                        elif rc == 1j:
                            out[o, ((2 + cp) * 3 + cl) * 2] += 0.5 * ti
                            out[o, ((2 + cp) * 3 + cl) * 2 + 1] -= 0.5 * tr
                        else:  # -1j
                            out[o, ((2 + cp) * 3 + cl) * 2] -= 0.5 * ti
                            out[o, ((2 + cp) * 3 + cl) * 2 + 1] += 0.5 * tr
    return out


def simulate(psi, U):
    """Full-lattice numpy fp16 simulation -> complex64 (T,Z,Y,X,4,3)."""
    link_vars = _to_planar_links(U)
    psi_vars = _to_planar_psi(psi)
    out = np.zeros((T, 24, NSITE), np.float32)
    for core in range(NCORES):
        out[core * TLOC:(core + 1) * TLOC] = simulate_core(link_vars, psi_vars, core * TLOC)
    res = (out[:, 0::2, :] + 1j * out[:, 1::2, :]).astype(np.complex64)
    return res.transpose(0, 2, 1).reshape(T, Z, Y, X, NS, NCOL)


# =================================================================== bass

LINK_KEYS = (
    [('c', d) for d in range(4)]
    + [('f', d, e) for (d, e) in
       [(0, 1), (0, 2), (0, 3), (2, 1), (3, 1), (3, 2), (1, 2), (1, 3), (2, 3)]]
    + [('b', d) for d in (1, 2, 3)]
)
PSI_KEYS = [('c',)] + [('s', e, sgn) for e in (1, 2, 3) for sgn in (1, -1)]


def _lname(key):
    return "u_" + "_".join(str(x) for x in key).replace('-', 'm')


def _pname(key):
    return "psi_" + "_".join(str(x) for x in key).replace('-', 'm')


def _bbuild_table():
    """Per chirality block: list of (plane, A, B(<=A), tgt_im, f9comp, coef)."""
    offd = {(0, 1): 0, (0, 2): 1, (1, 2): 2}
    tables = [[], []]
    for blk, sigs in enumerate((SIG_UP, SIG_DN)):
        for p in range(6):
            sig = sigs[p]
            for a in range(2):
                for b in range(2):
                    s = sig[a, b]
                    if abs(s) < 1e-12:
                        continue
                    cf = -1j * CCLOV * s
                    for i in range(3):
                        for j in range(3):
                            A_, B_ = a * 3 + i, b * 3 + j
                            if A_ < B_:
                                continue
                            if i == j:
                                fre = None
                                fim = (6 + i, 1.0)
                            elif (i, j) in offd:
                                q = offd[(i, j)]
                                fre = (2 * q, 1.0); fim = (2 * q + 1, 1.0)
                            else:
                                q = offd[(j, i)]
                                fre = (2 * q, -1.0); fim = (2 * q + 1, 1.0)
                            cr, ci = cf.real, cf.imag
                            for tgt_im, parts in ((0, [(fre, cr), (fim, -ci)]),
                                                  (1, [(fim, cr), (fre, ci)])):
                                if A_ == B_ and tgt_im:
                                    continue
                                for src, c0 in parts:
                                    if src is None or abs(c0) < 1e-15:
                                        continue
                                    comp, s0 = src
                                    tables[blk].append((p, A_, B_, tgt_im, comp, c0 * s0))
    # sanity: every lower-tri re comp and offdiag im comp gets >=1 write
    for blk in range(2):
        seen = {(A_, B_, t) for (_, A_, B_, t, _, _) in tables[blk]}
        for A_ in range(6):
            for B_ in range(A_ + 1):
                assert (A_, B_, 0) in seen, (blk, A_, B_)
                if A_ != B_:
                    assert (A_, B_, 1) in seen, (blk, A_, B_)
    return tables


BTABLES = _bbuild_table()


def _axis_pieces(d, L):
    """dst[i] = src[(i+d) % L] -> (dst_start, src_start, length) pieces."""
    d %= L
    if d == 0:
        return [(0, 0, L)]
    return [(0, d, L - d), (L - d, 0, d)]


def _build_device_program():
    import concourse.bacc as bacc
    import concourse.mybir as mybir
    from concourse import tile as ctile

    FP16, FP32 = mybir.dt.float16, mybir.dt.float32
    AL = mybir.AluOpType
    nc = bacc.Bacc(None, target_bir_lowering=False)

    u_in = {k: nc.declare_dram_parameter(_lname(k), [NWIN, P, 18, F], FP16, isOutput=False)
            for k in LINK_KEYS}
    p_in = {k: nc.declare_dram_parameter(_pname(k), [NWIN, P, 24, F], FP16, isOutput=False)
            for k in PSI_KEYS}
    out_dram = nc.declare_dram_parameter("out", [TLOC, P, 24, F], FP32, isOutput=True)

    dbg = {}
    if DEBUG_DUMP:
        dbg['g'] = nc.declare_dram_parameter("dbg_g", [6, NWIN, 9, NSITE], FP16, isOutput=True)
        dbg['ft'] = nc.declare_dram_parameter("dbg_ft", [6, P, 9, F], FP16, isOutput=True)
        dbg['bb'] = nc.declare_dram_parameter("dbg_bb", [2, P, 72, F], FP16, isOutput=True)
        dbg['ap'] = nc.declare_dram_parameter("dbg_ap", [P, 24, F], FP16, isOutput=True)
    gps = [[nc.dram_tensor(f"gp{p}_{w}", [9, NSITE], FP16) for w in range(NWIN)]
           for p in range(6)]
    # deduped shifted-G buffers keyed (plane, w_src, spatial shift)
    shuf_map = {}
    for p in range(6):
        for k, (dt, dz, dy, dx) in enumerate(DELTAS[p]):
            for o in range(TLOC):
                wsrc = o + 2 + dt
                key = (p, wsrc, dz, dy, dx)
                if key not in shuf_map:
                    shuf_map[key] = nc.dram_tensor(
                        f"gsh{p}_{wsrc}_{dz}_{dy}_{dx}".replace('-', 'm'),
                        [9, NSITE], FP16)

    def emit_cmatmul(pool, out_t, a_t, b_t, dag_b, eng=None, tp=""):
        """out = A @ B(^+), 3x3 complex (30 ops, per output column)."""
        eng = eng if eng is not None else nc.vector
        P4 = {}
        for ra in (0, 1):
            for rb in (0, 1):
                P4[(ra, rb)] = pool.tile([P, 9, F], FP16, tag=f"mmP{ra}{rb}{tp}",
                                         name=f"mmP{ra}{rb}{tp}", bufs=1)
        Dre = pool.tile([P, 9, F], FP16, tag="mmDre" + tp, name="mmDre" + tp, bufs=1)
        Dim = pool.tile([P, 9, F], FP16, tag="mmDim" + tp, name="mmDim" + tp, bufs=1)
        av_all = a_t[:].rearrange("p (i j r) f -> p i j r f", i=3, j=3)
        bv_all = b_t[:].rearrange("p (j k r) f -> p j k r f", j=3, k=3)
        bv_dag = b_t[:].rearrange("p (k j r) f -> p k j r f", k=3, j=3)
        ov_all = out_t[:].rearrange("p (i k r) f -> p i k r f", i=3, k=3)
        for k in range(3):
            for (ra, rb), pt in P4.items():
                if dag_b:
                    bsel = bv_dag[:, k, :, rb, :]  # B[k,j]: [P, j(3), F]
                else:
                    bsel = bv_all[:, :, k, rb, :]  # B[j,k]: [P, j(3), F]
                bb = bsel.unsqueeze(1).broadcast_to([P, 3, 3, F])
                eng.tensor_mul(
                    pt[:].rearrange("p (i j) f -> p i j f", i=3),
                    av_all[:, :, :, ra, :], bb)
            if dag_b:
                eng.tensor_add(Dre[:], P4[(0, 0)][:], P4[(1, 1)][:])
                eng.tensor_sub(Dim[:], P4[(1, 0)][:], P4[(0, 1)][:])
            else:
                eng.tensor_sub(Dre[:], P4[(0, 0)][:], P4[(1, 1)][:])
                eng.tensor_add(Dim[:], P4[(0, 1)][:], P4[(1, 0)][:])
            for r, Dt in ((0, Dre), (1, Dim)):
                ov = ov_all[:, :, k, r, :]  # [P, i(3), F]
                Dv = Dt[:].rearrange("p (i j) f -> p i j f", i=3)
                eng.tensor_add(ov, Dv[:, :, 0, :], Dv[:, :, 1, :])
                eng.tensor_add(ov, ov, Dv[:, :, 2, :])

    def emit_cmatvec(pool, uh_t, u_t, h_t, dag):
        """uh[c,i] = sum_j Utilde[i,j] h[c,j]; h/uh: [P,12,F] (c=2 cols)."""
        if dag:
            uv = u_t[:].rearrange("p (j i r) f -> p i j r f", j=3, i=3)
        else:
            uv = u_t[:].rearrange("p (i j r) f -> p i j r f", i=3, j=3)
        hv = h_t[:].rearrange("p (c cl r) f -> p c cl r f", c=2, cl=3)
        ov = uh_t[:].rearrange("p (c i r) f -> p c i r f", c=2, i=3)
        P4 = {}
        for ra in (0, 1):
            for rb in (0, 1):
                P4[(ra, rb)] = pool.tile([P, 9, F], FP16, tag=f"mvP{ra}{rb}",
                                         name=f"mvP{ra}{rb}", bufs=1)
        Dre = pool.tile([P, 9, F], FP16, tag="mvDre", name="mvDre", bufs=1)
        Dim = pool.tile([P, 9, F], FP16, tag="mvDim", name="mvDim", bufs=1)
        for c in range(2):
            for (ra, rb), pt in P4.items():
                hb = hv[:, c, :, rb, :].unsqueeze(1).broadcast_to([P, 3, 3, F])
                nc.vector.tensor_mul(
                    pt[:].rearrange("p (i j) f -> p i j f", i=3),
                    uv[:, :, :, ra, :], hb)
            if dag:
                # conj is on U (first factor): im = Ur*hi - Ui*hr
                nc.vector.tensor_add(Dre[:], P4[(0, 0)][:], P4[(1, 1)][:])
                nc.vector.tensor_sub(Dim[:], P4[(0, 1)][:], P4[(1, 0)][:])
            else:
                nc.vector.tensor_sub(Dre[:], P4[(0, 0)][:], P4[(1, 1)][:])
                nc.vector.tensor_add(Dim[:], P4[(0, 1)][:], P4[(1, 0)][:])
            for r, Dt in ((0, Dre), (1, Dim)):
                o1 = ov[:, c, :, r, :]  # [P, i(3), F]
                Dv = Dt[:].rearrange("p (i j) f -> p i j f", i=3)
                nc.vector.tensor_add(o1, Dv[:, :, 0, :], Dv[:, :, 1, :])
                nc.vector.tensor_add(o1, o1, Dv[:, :, 2, :])

    POOL_CMM = True
    _shuf_engs = [nc.sync, nc.scalar]
    _shuf_idx = [0]

    def _next_shuf_eng():
        _shuf_idx[0] += 1
        return _shuf_engs[_shuf_idx[0] % len(_shuf_engs)]

    with ctile.TileContext(nc) as tc:
        # ---------------- phase 1: G build ----------------
        with tc.tile_pool(name="lnk", bufs=2) as lnk, \
             tc.tile_pool(name="gtmp", bufs=2) as gtmp, \
             tc.tile_pool(name="gout", bufs=2) as goutp:
            for w in range(6):
                cache = {}

                def load_link(key, wi, tag):
                    ck = (key, wi)
                    if ck not in cache:
                        t = lnk.tile([P, 18, F], FP16, tag=tag, name=tag)
                        nc.sync.dma_start(t[:], u_in[key][wi])
                        cache[ck] = t
                    return cache[ck]

                for p, (mu, nu) in enumerate(PAIRS):
                    if mu != 0 and w < 2:
                        continue
                    if mu == 0:
                        M1 = load_link(('c', 0), w, "m1_" + str(p))
                        M2 = load_link(('c', nu), w + 1, "m2_" + str(p))
                        M3 = load_link(('c', nu), w, "m3_" + str(p))
                        M4 = load_link(('f', 0, nu), w, "m4_" + str(p))
                    else:
                        M1 = load_link(('c', mu), w, "m1_" + str(p))
                        M2 = load_link(('f', nu, mu), w, "m2_" + str(p))
                        M3 = load_link(('c', nu), w, "m3_" + str(p))
                        M4 = load_link(('f', mu, nu), w, "m4_" + str(p))
                    # offload some units' independent A/B products to Pool
                    on_pool = ((2 * p + w) % 3 == 0) and POOL_CMM
                    At = gtmp.tile([P, 18, F], FP16, tag="A", name="A")
                    Bt = gtmp.tile([P, 18, F], FP16, tag="B", name="B")
                    Wt = gtmp.tile([P, 18, F], FP16, tag="W", name="W")
                    peng = nc.gpsimd if on_pool else nc.vector
                    ptp = "g" if on_pool else ""
                    emit_cmatmul(gtmp, At, M1, M2, dag_b=False, eng=peng, tp=ptp)
                    emit_cmatmul(gtmp, Bt, M3, M4, dag_b=False, eng=peng, tp=ptp)
                    emit_cmatmul(gtmp, Wt, At, Bt, dag_b=True)
                    Gt = goutp.tile([P, 9, F], FP16, tag="G", name="G")
                    offd = [(0, 1), (0, 2), (1, 2)]
                    for q, (i, j) in enumerate(offd):
                        a_, b_ = (i * 3 + j) * 2, (j * 3 + i) * 2
                        nc.vector.tensor_sub(Gt[:, 2 * q:2 * q + 1, :],
                                             Wt[:, a_:a_ + 1, :], Wt[:, b_:b_ + 1, :])
                        nc.vector.tensor_add(Gt[:, 2 * q + 1:2 * q + 2, :],
                                             Wt[:, a_ + 1:a_ + 2, :], Wt[:, b_ + 1:b_ + 2, :])
                    for d in range(3):
                        c_ = (d * 3 + d) * 2 + 1
                        nc.vector.tensor_scalar_mul(Gt[:, 6 + d:7 + d, :],
                                                    Wt[:, c_:c_ + 1, :], 2.0)
                    nc.scalar.dma_start(
                        gps[p][w].rearrange("c (p2 f) -> p2 c f", p2=P), Gt[:])
                    if DEBUG_DUMP:
                        nc.sync.dma_start(
                            dbg['g'][p, w].rearrange("c (p2 f) -> p2 c f", p2=P), Gt[:])

                # deduped G shuffles whose source slice just became ready
                for (p, wsrc, dz, dy, dx), buf in shuf_map.items():
                    if wsrc != w:
                        continue
                    src = gps[p][w].rearrange("c (z y x) -> c z y x", z=Z, y=Y)
                    dst = buf.rearrange("c (z y x) -> c z y x", z=Z, y=Y)
                    qeng = _next_shuf_eng()
                    for (zd, zs, zl) in _axis_pieces(dz, Z):
                        for (yd, ys, yl) in _axis_pieces(dy, Y):
                            for (xd, xs, xl) in _axis_pieces(dx, X):
                                with nc.allow_non_contiguous_dma(reason="wrap"):
                                    qeng.dma_start(
                                        dst[:, zd:zd + zl, yd:yd + yl, xd:xd + xl],
                                        src[:, zs:zs + zl, ys:ys + yl, xs:xs + xl])

        # ---------------- phase 2: apply + hop ----------------
        with tc.tile_pool(name="gld", bufs=2) as gld, \
             tc.tile_pool(name="ftl", bufs=2) as ftl, \
             tc.tile_pool(name="bbl", bufs=2) as bbl, \
             tc.tile_pool(name="psl", bufs=2) as psl, \
             tc.tile_pool(name="uhp", bufs=2) as uhp, \
             tc.tile_pool(name="htm", bufs=2) as htm, \
             tc.tile_pool(name="oot", bufs=2) as oot:
            for o in range(TLOC):
                w = o + 2
                # F_tilde per plane
                ftil = []
                for p in range(6):
                    g0 = gld.tile([P, 9, F], FP16, tag="g0", name="g0")
                    nc.sync.dma_start(g0[:], gps[p][w].rearrange("c (p2 f) -> p2 c f", p2=P))
                    ft = ftl.tile([P, 9, F], FP16, tag=f"ft{p}", name=f"ft{p}")
                    first = True
                    for k in range(3):
                        dt, dz, dy, dx = DELTAS[p][k]
                        gbuf = shuf_map[(p, o + 2 + dt, dz, dy, dx)]
                        gk = gld.tile([P, 9, F], FP16, tag=f"g{k + 1}", name=f"g{k + 1}")
                        nc.sync.dma_start(gk[:], gbuf.rearrange("c (p2 f) -> p2 c f", p2=P))
                        if first:
                            nc.vector.tensor_sub(ft[:], g0[:], gk[:])
                            first = False
                        else:
                            nc.vector.tensor_sub(ft[:], ft[:], gk[:])
                    if DEBUG_DUMP and o == 0:
                        nc.sync.dma_start(dbg['ft'][p], ft[:])
                    ftil.append(ft)

                # B blocks (lower-tri build + conj fill)
                bts = [bbl.tile([P, 72, F], FP16, tag=f"B{blk}", name=f"B{blk}") for blk in range(2)]
                for blk in range(2):
                    bt = bts[blk]
                    written = set()
                    for (p, A_, B_, tgt_im, comp, coef) in (BTABLES[blk] if ENABLE_CLOVER else [(p_, A_, A_, 0, 0, 0.0) for p_ in [0] for A_ in range(6)]):
                        e = (A_ * 6 + B_) * 2 + tgt_im
                        dst = bt[:, e:e + 1, :]
                        src = ftil[p][:, comp:comp + 1, :]
                        if e not in written:
                            nc.vector.tensor_scalar_mul(dst, src, float(coef))
                            written.add(e)
                        else:
                            nc.vector.scalar_tensor_tensor(
                                dst, src, float(coef), dst, AL.mult, AL.add)
                    for A_ in range(6):
                        e = (A_ * 6 + A_) * 2
                        nc.vector.tensor_scalar_add(bt[:, e:e + 1, :], bt[:, e:e + 1, :], DIAG)
                        nc.vector.memzero(bt[:, e + 1:e + 2, :])
                    for A_ in range(6):
                        for B_ in range(A_ + 1, 6):
                            esrc = (B_ * 6 + A_) * 2
                            edst = (A_ * 6 + B_) * 2
                            nc.scalar.copy(bt[:, edst:edst + 1, :], bt[:, esrc:esrc + 1, :])
                            nc.scalar.mul(bt[:, edst + 1:edst + 2, :],
                                          bt[:, esrc + 1:esrc + 2, :], -1.0)

                # apply B to psi -> out tile
                psi_c = psl.tile([P, 24, F], FP16, tag="psc", name="psc")
                nc.sync.dma_start(psi_c[:], p_in[('c',)][w])
                out_t = oot.tile([P, 24, F], FP16, tag="out", name="out")
                aptmp = htm.tile([P, 6, F], FP16, tag="aptmp", name="aptmp")
                aptm2 = htm.tile([P, 12, F], FP16, tag="aptm2", name="aptm2")
                for blk in range(2):
                    bt = bts[blk]
                    bv = bt[:].rearrange("p (a b r) f -> p a b r f", a=6, b=6)
                    ovv = out_t[:].rearrange("p (s r) f -> p s r f", r=2)
                    pvv = psi_c[:].rearrange("p (s r) f -> p s r f", r=2)
                    out_ri = out_t[:, blk * 12:(blk + 1) * 12, :]  # [P,12,F] (A,r)
                    out_re = ovv[:, blk * 6:(blk + 1) * 6, 0, :]
                    out_im = ovv[:, blk * 6:(blk + 1) * 6, 1, :]
                    for B_ in range(6):
                        sB = blk * 6 + B_
                        pr = pvv[:, sB:sB + 1, 0, :].broadcast_to([P, 6, F])
                        pi = pvv[:, sB:sB + 1, 1, :].broadcast_to([P, 6, F])
                        # psi (re,im) pair broadcast over A: [P, A(6), r(2), F]
                        pri = (psi_c[:, sB * 2:sB * 2 + 2, :]
                               .unsqueeze(1).broadcast_to([P, 6, 2, F]))
                        Brv = bv[:, :, B_, 0, :]
                        # Br broadcast over r: [P, A(6), r(2), F]
                        Brr = Brv.unsqueeze(2).broadcast_to([P, 6, 2, F])
                        Biv = bv[:, :, B_, 1, :]
                        ori = out_ri.rearrange("p (a r) f -> p a r f", a=6)
                        if B_ == 0:
                            nc.vector.tensor_mul(ori, Brr, pri)
                        else:
                            nc.vector.tensor_mul(
                                aptm2[:].rearrange("p (a r) f -> p a r f", a=6),
                                Brr, pri)
                            nc.vector.tensor_add(out_ri, out_ri, aptm2[:])
                        nc.vector.tensor_mul(aptmp[:], Biv, pi)
                        nc.vector.tensor_sub(out_re, out_re, aptmp[:])
                        nc.vector.tensor_mul(aptmp[:], Biv, pr)
                        nc.vector.tensor_add(out_im, out_im, aptmp[:])

                if DEBUG_DUMP and o == 0:
                    for blk in range(2):
                        nc.sync.dma_start(dbg['bb'][blk], bts[blk][:])
                    nc.sync.dma_start(dbg['ap'][:], out_t[:])

                # hop terms
                for mu in (range(4) if ENABLE_HOP else ()):
                    tbl = HOP[mu]
                    for sgn in (1, -1):
                        # psi source tile
                        psv = psl.tile([P, 24, F], FP16, tag="psv", name="psv")
                        if mu == 0:
                            nc.sync.dma_start(psv[:], p_in[('c',)][w + (1 if sgn > 0 else -1)])
                        else:
                            nc.sync.dma_start(psv[:], p_in[('s', mu, 1 if sgn > 0 else -1)][w])
                        # U tile
                        ut = uhp.tile([P, 18, F], FP16, tag="ut", name="ut")
                        if sgn > 0:
                            nc.sync.dma_start(ut[:], u_in[('c', mu)][w])
                        elif mu == 0:
                            nc.sync.dma_start(ut[:], u_in[('c', 0)][w - 1])
                        else:
                            nc.sync.dma_start(ut[:], u_in[('b', mu)][w])
                        # projection -> h [P,12,F]
                        h = htm.tile([P, 12, F], FP16, tag="h", name="h")
                        pvv = psv[:].rearrange("p (s r) f -> p s r f", r=2)
                        hvv = h[:].rearrange("p (s r) f -> p s r f", r=2)
                        for c in range(2):
                            cf = sgn * tbl['coef'][c]
                            b_ = tbl['b'][c]
                            if cf == 1:
                                nc.vector.tensor_add(h[:, c * 6:(c + 1) * 6, :],
                                                     psv[:, c * 6:(c + 1) * 6, :],
                                                     psv[:, b_ * 6:(b_ + 1) * 6, :])
                            elif cf == -1:
                                nc.vector.tensor_sub(h[:, c * 6:(c + 1) * 6, :],
                                                     psv[:, c * 6:(c + 1) * 6, :],
                                                     psv[:, b_ * 6:(b_ + 1) * 6, :])
                            else:
                                hre = hvv[:, c * 3:(c + 1) * 3, 0, :]
                                him = hvv[:, c * 3:(c + 1) * 3, 1, :]
                                pre = pvv[:, c * 3:(c + 1) * 3, 0, :]
                                pim = pvv[:, c * 3:(c + 1) * 3, 1, :]
                                qre = pvv[:, b_ * 3:(b_ + 1) * 3, 0, :]
                                qim = pvv[:, b_ * 3:(b_ + 1) * 3, 1, :]
                                if cf == 1j:
                                    nc.vector.tensor_sub(hre, pre, qim)
                                    nc.vector.tensor_add(him, pim, qre)
                                else:  # -1j
                                    nc.vector.tensor_add(hre, pre, qim)
                                    nc.vector.tensor_sub(him, pim, qre)
                        # color mult
                        uh = htm.tile([P, 12, F], FP16, tag="uh", name="uh")
                        emit_cmatvec(uhp, uh, ut, h, dag=(sgn < 0))
                        # accumulate into out (rows 0,1 in one op)
                        sl = out_t[:, 0:12, :]
                        nc.vector.scalar_tensor_tensor(
                            sl, uh[:, 0:12, :], -0.5, sl, AL.mult, AL.add)
                        uvv = uh[:].rearrange("p (s r) f -> p s r f", r=2)
                        ovv = out_t[:].rearrange("p (s r) f -> p s r f", r=2)
                        rcs = [sgn * tbl['rc'][cp] for cp in range(2)]
                        if rcs[0] == rcs[1] and tbl['m'] == (0, 1) and rcs[0] in (1, -1):
                            sl = out_t[:, 12:24, :]
                            nc.vector.scalar_tensor_tensor(
                                sl, uh[:, 0:12, :], -0.5 * rcs[0], sl,
                                AL.mult, AL.add)
                            continue
                        for cp in range(2):
                            rc = rcs[cp]
                            mm = tbl['m'][cp]
                            row = 2 + cp
                            if rc in (1, -1):
                                sl = out_t[:, row * 6:(row + 1) * 6, :]
                                nc.vector.scalar_tensor_tensor(
                                    sl, uh[:, mm * 6:(mm + 1) * 6, :], -0.5 * rc, sl,
                                    AL.mult, AL.add)
                            else:
                                s_i = rc.imag
                                o_re = ovv[:, row * 3:(row + 1) * 3, 0, :]
                                o_im = ovv[:, row * 3:(row + 1) * 3, 1, :]
                                u_re = uvv[:, mm * 3:(mm + 1) * 3, 0, :]
                                u_im = uvv[:, mm * 3:(mm + 1) * 3, 1, :]
                                nc.vector.scalar_tensor_tensor(
                                    o_re, u_im, 0.5 * s_i, o_re, AL.mult, AL.add)
                                nc.vector.scalar_tensor_tensor(
                                    o_im, u_re, -0.5 * s_i, o_im, AL.mult, AL.add)

                # store (fp16 -> fp32 cast via SWDGE)
                nc.gpsimd.dma_start(out_dram[o], out_t[:])

    nc.finalize()
    return nc


_PROG_CACHE = {}


def _get_program():
    if 'nc' not in _PROG_CACHE:
        _PROG_CACHE['nc'] = _build_device_program()
    return _PROG_CACHE['nc']


def _sbuf_image(a, C):
    """[T, C, NSITE] -> [T, P, C, F] contiguous."""
    return np.ascontiguousarray(a.reshape(T, C, P, F).transpose(0, 2, 1, 3))


def build_in_maps(psi, U):
    link_vars = _to_planar_links(U)
    psi_vars = _to_planar_psi(psi)
    link_imgs = {k: _sbuf_image(v, 18) for k, v in link_vars.items()}
    psi_imgs = {k: _sbuf_image(v, 24) for k, v in psi_vars.items()}
    in_maps = []
    for core in range(NCORES):
        t0 = core * TLOC
        tw = [(t0 - 2 + w) % T for w in range(NWIN)]
        m = {}
        for k in LINK_KEYS:
            m[_lname(k)] = np.ascontiguousarray(link_imgs[k][tw])
        for k in PSI_KEYS:
            m[_pname(k)] = np.ascontiguousarray(psi_imgs[k][tw])
        in_maps.append(m)
    return in_maps


def assemble_output(results):
    out = np.empty((T, 24, NSITE), np.float32)
    for core in range(NCORES):
        r = results[core]['out']  # [TLOC, P, 24, F] fp32
        out[core * TLOC:(core + 1) * TLOC] = r.transpose(0, 2, 1, 3).reshape(TLOC, 24, NSITE)
    res = (out[:, 0::2, :] + 1j * out[:, 1::2, :]).astype(np.complex64)
    return res.transpose(0, 2, 1).reshape(T, Z, Y, X, NS, NCOL)


def kernel(psi, U):
    psi = np.asarray(psi)
    U = np.asarray(U)
    from concourse.bass_utils import run_bass_kernel_spmd
    nc = _get_program()
    in_maps = build_in_maps(psi, U)
    res = run_bass_kernel_spmd(nc, in_maps, core_ids=list(range(NCORES)))
    return assemble_output(res.results)



# revision 12
# speedup vs baseline: 1.1976x; 1.1976x over previous
"""Clover-Wilson Dirac operator on Trainium2 (8 NeuronCores, T-sharded).

Math summary (derived + numerically verified against the reference):
- The reference's 4-leaf "clover" Q for plane (mu,nu) factorizes as
      Q(x) = W(x) + W(x+d1)^+ + W(x+d2)^+ + W(x+d3)^+
  with W(x) = [U_mu(x) U_nu(x+mu)] [U_nu(x) U_mu(x+nu)]^+,
  d1 = nu-mu, d2 = -2mu-2nu, d3 = -2nu (unit lattice vectors).
- With G = W - W^+ (anti-Hermitian), Ftil := Q - Q^+ = G(x) - G(x+d1) - G(x+d2) - G(x+d3).
- C psi + (4+m) psi = (5+m) psi + (csw/32) * sum_p (sigma_p (x) (-i Ftil_p)) psi,
  where sigma_p is block-diagonal (2x2 chiral blocks) in this basis.
- Wilson hop uses the standard spin-projection trick (2 half-spinors per direction).

Distribution: T=32 sharded 4 slices per core; U needs halo t0-2..t0+4 (7 slices),
psi needs t0-1..t0+4. All jnp.roll shifts are pushed into host-precomputed
pre-rolled planar fp16 arrays; on-device shifted reads of the intermediate G
use DRAM->DRAM affine shuffle DMAs.
"""
import numpy as np

T, Z, Y, X = 32, 24, 24, 24
NCOL, NS = 3, 4
MASS, CSW = 0.1, 1.0
PAIRS = [(0, 1), (0, 2), (0, 3), (1, 2), (1, 3), (2, 3)]
NCORES = 8
TLOC = T // NCORES          # 4 output slices per core
NSITE = Z * Y * X           # 13824
P = 128
F = NSITE // P              # 108
NWIN = 7                    # U window slices: t0-2 .. t0+4
DIAG = 5.0 + MASS           # (4+m) + clover identity
CCLOV = CSW / 32.0          # |coefficient| of sigma (x) Ftil; overall factor -i


# ----------------------------------------------------------------- tables

def _gammas():
    i = 1j
    g0 = np.array([[0, 0, 1, 0], [0, 0, 0, 1], [1, 0, 0, 0], [0, 1, 0, 0]], np.complex128)
    g1 = np.array([[0, 0, 0, i], [0, 0, i, 0], [0, -i, 0, 0], [-i, 0, 0, 0]], np.complex128)
    g2 = np.array([[0, 0, 0, -1], [0, 0, 1, 0], [0, 1, 0, 0], [-1, 0, 0, 0]], np.complex128)
    g3 = np.array([[0, 0, i, 0], [0, 0, 0, -i], [-i, 0, 0, 0], [0, i, 0, 0]], np.complex128)
    return [g0, g1, g2, g3]


def _sigma_blocks():
    """Chiral 2x2 blocks of sigma_{mu nu} = i g_mu g_nu for each plane."""
    G = _gammas()
    ups, dns = [], []
    for mu, nu in PAIRS:
        s = 1j * (G[mu] @ G[nu])
        assert np.abs(s[:2, 2:]).max() < 1e-12 and np.abs(s[2:, :2]).max() < 1e-12
        ups.append(s[:2, :2].copy())
        dns.append(s[2:, 2:].copy())
    return ups, dns


SIG_UP, SIG_DN = _sigma_blocks()

# per-plane shift deltas (t, z, y, x) for the W-factorization
def _deltas():
    out = []
    for mu, nu in PAIRS:
        e_mu = np.zeros(4, np.int64); e_mu[mu] = 1
        e_nu = np.zeros(4, np.int64); e_nu[nu] = 1
        out.append([tuple(e_nu - e_mu), tuple(-2 * e_mu - 2 * e_nu), tuple(-2 * e_nu)])
    return out


DELTAS = _deltas()

# debug toggles (affect both simulate_core and the device program)
ENABLE_CLOVER = True
ENABLE_HOP = True
DEBUG_DUMP = False

# hop projection tables: psi_h[c] = psi[c] + coef * psi[b[c]]; lower rows:
# row_{2+c} = rc[c] * h[m[c]]  (forward, i.e. (1-gamma)); backward negates
# coef and rc. Verified against gammas in _check_hop_tables().
HOP = {
    0: dict(b=(2, 3), coef=(-1, -1), m=(0, 1), rc=(-1, -1)),
    1: dict(b=(3, 2), coef=(-1j, -1j), m=(1, 0), rc=(1j, 1j)),
    2: dict(b=(3, 2), coef=(1, -1), m=(1, 0), rc=(-1, 1)),
    3: dict(b=(2, 3), coef=(-1j, 1j), m=(0, 1), rc=(1j, -1j)),
}


def _check_hop_tables():
    G = _gammas()
    for mu, t in HOP.items():
        for sgn in (+1, -1):  # +1: (1-g) fwd ; -1: (1+g) bwd
            M = np.eye(4) - sgn * G[mu]
            # build from table
            B = np.zeros((4, 4), np.complex128)
            for c in range(2):
                B[c, c] += 1
                B[c, t['b'][c]] += sgn * t['coef'][c]
            for c in range(2):
                rc = sgn * t['rc'][c]
                B[2 + c, t['m'][c]] += rc
                B[2 + c, t['b'][t['m'][c]]] += rc * sgn * t['coef'][t['m'][c]]
            assert np.abs(B - M).max() < 1e-12, (mu, sgn, B, M)


_check_hop_tables()


# ------------------------------------------------- planar layout helpers

def _to_planar_links(U):
    """U: (T,Z,Y,X,4,3,3) complex64 -> dict of fp16 planar arrays.

    Returns variants[key] = array [T, 18, NSITE] fp16 with comp c=(i*3+j)*2+r.
    Keys: ('c', d) centered; ('f', d, e) = U_d(x+e_hat) spatial e;
          ('b', d) = U_d(x - d_hat) spatial d.
    """
    Uf32 = np.ascontiguousarray(U)  # complex64
    planar = np.empty((4, T, 18, NSITE), np.float16)
    Um = Uf32.reshape(T, NSITE, 4, 9)
    for d in range(4):
        re = Um[..., d, :].real.astype(np.float16)  # (T, NSITE, 9)
        im = Um[..., d, :].imag.astype(np.float16)
        planar[d, :, 0::2, :] = re.transpose(0, 2, 1)
        planar[d, :, 1::2, :] = im.transpose(0, 2, 1)

    def roll_sites(arr, delta):  # arr [..., NSITE]; value at x+delta
        dz, dy, dx = delta
        a = arr.reshape(*arr.shape[:-1], Z, Y, X)
        if dz: a = np.roll(a, -dz, axis=-3)
        if dy: a = np.roll(a, -dy, axis=-2)
        if dx: a = np.roll(a, -dx, axis=-1)
        return a.reshape(*arr.shape[:-1], NSITE)

    variants = {}
    for d in range(4):
        variants[('c', d)] = planar[d]
    needed_f = {(0, 1), (0, 2), (0, 3), (2, 1), (3, 1), (3, 2), (1, 2), (1, 3), (2, 3)}
    for (d, e) in needed_f:
        delta = [0, 0, 0]; delta[e - 1] = 1
        variants[('f', d, e)] = roll_sites(planar[d], delta)
    for d in (1, 2, 3):
        delta = [0, 0, 0]; delta[d - 1] = -1
        variants[('b', d)] = roll_sites(planar[d], delta)
    return variants


def _to_planar_psi(psi):
    """psi: (T,Z,Y,X,4,3) complex64 -> dict: ('c',) and ('s', e, sgn) ->
    [T, 24, NSITE] fp16, comp c=(s*3+cl)*2+r."""
    pm = psi.reshape(T, NSITE, 12)
    planar = np.empty((T, 24, NSITE), np.float16)
    planar[:, 0::2, :] = pm.real.astype(np.float16).transpose(0, 2, 1)
    planar[:, 1::2, :] = pm.imag.astype(np.float16).transpose(0, 2, 1)

    def roll_sites(arr, delta):
        dz, dy, dx = delta
        a = arr.reshape(*arr.shape[:-1], Z, Y, X)
        if dz: a = np.roll(a, -dz, axis=-3)
        if dy: a = np.roll(a, -dy, axis=-2)
        if dx: a = np.roll(a, -dx, axis=-1)
        return a.reshape(*arr.shape[:-1], NSITE)

    out = {('c',): planar}
    for e in (1, 2, 3):
        for sgn in (1, -1):
            delta = [0, 0, 0]; delta[e - 1] = sgn
            out[('s', e, sgn)] = roll_sites(planar, delta)
    return out


# ------------------------------------------------------ numpy simulator
# Step-wise fp16 mirror of the device dataflow (for validation).

def _cmm16(A, B, dag_b=False):
    """A,B: [18, N] fp16 planar 3x3 complex; returns C = A @ B(^+) fp16."""
    C = np.zeros_like(A)
    for i in range(3):
        for k in range(3):
            cre = np.zeros(A.shape[-1], np.float16)
            cim = np.zeros(A.shape[-1], np.float16)
            for j in range(3):
                ar = A[(i * 3 + j) * 2]; ai = A[(i * 3 + j) * 2 + 1]
                if dag_b:
                    br = B[(k * 3 + j) * 2]; bi = -B[(k * 3 + j) * 2 + 1].astype(np.float16)
                else:
                    br = B[(j * 3 + k) * 2]; bi = B[(j * 3 + k) * 2 + 1]
                cre = (cre + (ar * br - ai * bi)).astype(np.float16)
                cim = (cim + (ar * bi + ai * br)).astype(np.float16)
            C[(i * 3 + k) * 2] = cre
            C[(i * 3 + k) * 2 + 1] = cim
    return C


def _antiherm9(Wm):
    """W planar 18 -> G = W - W^+ in 9-comp layout:
    q*2 / q*2+1 = re/im of G[i,j] for (i,j) in [(0,1),(0,2),(1,2)]; 6+d = im G[d,d]."""
    G = np.empty((9, Wm.shape[-1]), np.float16)
    offd = [(0, 1), (0, 2), (1, 2)]
    for q, (i, j) in enumerate(offd):
        G[q * 2] = (Wm[(i * 3 + j) * 2] - Wm[(j * 3 + i) * 2]).astype(np.float16)
        G[q * 2 + 1] = (Wm[(i * 3 + j) * 2 + 1] + Wm[(j * 3 + i) * 2 + 1]).astype(np.float16)
    for d in range(3):
        G[6 + d] = (Wm[(d * 3 + d) * 2 + 1] * np.float16(2.0)).astype(np.float16)
    return G


def _f9_entry(F9, i, j):
    """(re, im) pair (arrays or (None, arr)) of Ftil[i,j] from 9-comp planar."""
    offd = {(0, 1): 0, (0, 2): 1, (1, 2): 2}
    if i == j:
        return None, F9[6 + i]
    if (i, j) in offd:
        q = offd[(i, j)]
        return F9[q * 2], F9[q * 2 + 1]
    q = offd[(j, i)]
    return -F9[q * 2], F9[q * 2 + 1]  # G[i>j] = -conj(G[j,i]) -> (-re, +im)


def _roll_sites_np(a, delta):
    dz, dy, dx = delta
    a = a.reshape(*a.shape[:-1], Z, Y, X)
    if dz: a = np.roll(a, -dz, axis=-3)
    if dy: a = np.roll(a, -dy, axis=-2)
    if dx: a = np.roll(a, -dx, axis=-1)
    return a.reshape(*a.shape[:-2], -1) if False else a.reshape(*a.shape[:-4], a.shape[-4] if a.ndim > 3 else -1, NSITE) if False else a.reshape(-1, NSITE) if a.ndim == 4 else a.reshape(NSITE)


def simulate_core(link_vars, psi_vars, t0):
    """Numpy fp16 mirror. link_vars/psi_vars: full-T variant dicts.
    Returns planar out [TLOC, 24, NSITE] float32."""
    tw = [(t0 - 2 + w) % T for w in range(NWIN)]

    def LV(key, w):
        return link_vars[key][tw[w]]

    def PV(key, w):
        return psi_vars[key][tw[w]]

    # ---- phase 1: G per plane per window slice
    Gs = {}
    for p, (mu, nu) in enumerate(PAIRS):
        ws = range(0, 6) if mu == 0 else range(2, 6)
        for w in ws:
            if mu == 0:
                M1, M2 = LV(('c', 0), w), LV(('c', nu), w + 1)
                M3, M4 = LV(('c', nu), w), LV(('f', 0, nu), w)
            else:
                M1, M2 = LV(('c', mu), w), LV(('f', nu, mu), w)
                M3, M4 = LV(('c', nu), w), LV(('f', mu, nu), w)
            A = _cmm16(M1, M2)
            B = _cmm16(M3, M4)
            Wm = _cmm16(A, B, dag_b=True)
            Gs[(p, w)] = _antiherm9(Wm)

    out = np.zeros((TLOC, 24, NSITE), np.float32)
    for o in range(TLOC):
        w = o + 2
        # ---- Ftil per plane
        F9s = []
        for p in range(6):
            acc = Gs[(p, w)].copy()
            for (dt, dz, dy, dx) in DELTAS[p]:
                g = Gs[(p, w + dt)]
                gsh = g.reshape(9, Z, Y, X)
                if dz: gsh = np.roll(gsh, -dz, axis=1)
                if dy: gsh = np.roll(gsh, -dy, axis=2)
                if dx: gsh = np.roll(gsh, -dx, axis=3)
                acc = (acc - gsh.reshape(9, NSITE)).astype(np.float16)
            F9s.append(acc)

        if not ENABLE_CLOVER:
            F9s = [np.zeros((9, NSITE), np.float16) for _ in range(6)]
        # ---- B blocks (full 6x6 complex per chirality block), fp16
        Bblk = [np.zeros((6, 6, 2, NSITE), np.float16) for _ in range(2)]
        for blk, sigs in enumerate((SIG_UP, SIG_DN)):
            for p in range(6):
                sig = sigs[p]
                for a in range(2):
                    for b in range(2):
                        s = sig[a, b]
                        if abs(s) < 1e-12:
                            continue
                        cf = -1j * CCLOV * s  # complex coefficient
                        for i in range(3):
                            for j in range(3):
                                fre, fim = _f9_entry(F9s[p], i, j)
                                A_, B_ = a * 3 + i, b * 3 + j
                                # coeff*(fre + i fim): accumulate re and im
                                cr, ci = cf.real, cf.imag
                                tgt = Bblk[blk][A_, B_]
                                if fre is not None:
                                    if cr: tgt[0] = (tgt[0] + np.float16(cr) * fre).astype(np.float16)
                                    if ci: tgt[1] = (tgt[1] + np.float16(ci) * fre).astype(np.float16)
                                if cr: tgt[1] = (tgt[1] + np.float16(cr) * fim).astype(np.float16)
                                if ci: tgt[0] = (tgt[0] - np.float16(ci) * fim).astype(np.float16)
            for A_ in range(6):
                Bblk[blk][A_, A_, 0] = (Bblk[blk][A_, A_, 0] + np.float16(DIAG)).astype(np.float16)

        # ---- apply B to psi
        psi_c = PV(('c',), w)
        for blk in range(2):
            for A_ in range(6):
                s_out = (blk * 2 + A_ // 3) * 3 + (A_ % 3)  # spinor comp index s*3+cl
                accr = np.zeros(NSITE, np.float16)
                acci = np.zeros(NSITE, np.float16)
                for B_ in range(6):
                    s_in = (blk * 2 + B_ // 3) * 3 + (B_ % 3)
                    pr = psi_c[s_in * 2]; pi = psi_c[s_in * 2 + 1]
                    br = Bblk[blk][A_, B_, 0]; bi = Bblk[blk][A_, B_, 1]
                    accr = (accr + br * pr - bi * pi).astype(np.float16)
                    acci = (acci + br * pi + bi * pr).astype(np.float16)
                out[o, s_out * 2] += accr.astype(np.float32)
                out[o, s_out * 2 + 1] += acci.astype(np.float32)

        # ---- hop terms
        for mu in (range(4) if ENABLE_HOP else ()):
            tbl = HOP[mu]
            for sgn, wpsi_key, woff, ukey, udag in (
                (+1, 'f', +1, ('c', mu), False),
                (-1, 'b', -1, ('b', mu) if mu else ('c', 0), True),
            ):
                if mu == 0:
                    psv = PV(('c',), w + woff)
                else:
                    psv = PV(('s', mu, +1 if sgn > 0 else -1), w)
                uar = LV(ukey, w) if mu else LV(ukey, w + (0 if sgn > 0 else -1))
                # project: h[c] = psi[c] + sgn*coef[c]*psi[b[c]] (2 spins x 3 col)
                h = np.zeros((2, 3, 2, NSITE), np.float16)
                for c in range(2):
                    cf = sgn * tbl['coef'][c]
                    for cl in range(3):
                        pr = psv[(c * 3 + cl) * 2]; pi = psv[(c * 3 + cl) * 2 + 1]
                        qr = psv[(tbl['b'][c] * 3 + cl) * 2]; qi = psv[(tbl['b'][c] * 3 + cl) * 2 + 1]
                        if cf == 1:
                            h[c, cl, 0] = (pr + qr).astype(np.float16); h[c, cl, 1] = (pi + qi).astype(np.float16)
                        elif cf == -1:
                            h[c, cl, 0] = (pr - qr).astype(np.float16); h[c, cl, 1] = (pi - qi).astype(np.float16)
                        elif cf == 1j:
                            h[c, cl, 0] = (pr - qi).astype(np.float16); h[c, cl, 1] = (pi + qr).astype(np.float16)
                        else:  # -1j
                            h[c, cl, 0] = (pr + qi).astype(np.float16); h[c, cl, 1] = (pi - qr).astype(np.float16)
                # color mult: uh[c, i] = sum_j U[i,j] h[c, j] (or U^+ )
                uh = np.zeros((2, 3, 2, NSITE), np.float16)
                for c in range(2):
                    for i in range(3):
                        ar = np.zeros(NSITE, np.float16); ai = np.zeros(NSITE, np.float16)
                        for j in range(3):
                            if udag:
                                ur = uar[(j * 3 + i) * 2]; ui = -uar[(j * 3 + i) * 2 + 1].astype(np.float16)
                            else:
                                ur = uar[(i * 3 + j) * 2]; ui = uar[(i * 3 + j) * 2 + 1]
                            ar = (ar + ur * h[c, j, 0] - ui * h[c, j, 1]).astype(np.float16)
                            ai = (ai + ur * h[c, j, 1] + ui * h[c, j, 0]).astype(np.float16)
                        uh[c, i, 0] = ar; uh[c, i, 1] = ai
                # accumulate: rows 0,1: -1/2*uh[c]; rows 2+c': -1/2*sgn... rc
                for c in range(2):
                    for cl in range(3):
                        out[o, (c * 3 + cl) * 2] -= 0.5 * uh[c, cl, 0].astype(np.float32)
                        out[o, (c * 3 + cl) * 2 + 1] -= 0.5 * uh[c, cl, 1].astype(np.float32)
                for cp in range(2):
                    rc = sgn * tbl['rc'][cp]
                    mm = tbl['m'][cp]
                    for cl in range(3):
                        tr = uh[mm, cl, 0].astype(np.float32); ti = uh[mm, cl, 1].astype(np.float32)
                        if rc == 1:
                            out[o, ((2 + cp) * 3 + cl) * 2] -= 0.5 * tr
                            out[o, ((2 + cp) * 3 + cl) * 2 + 1] -= 0.5 * ti
                        elif rc == -1:
                            out[o, ((2 + cp) * 3 + cl) * 2] += 0.5 * tr
                            out[o, ((2 + cp) * 3 + cl) * 2 + 1] += 0.5 * ti
                        elif rc == 1j:
                            out[o, ((2 + cp) * 3 + cl) * 2] += 0.5 * ti
                            out[o, ((2 + cp) * 3 + cl) * 2 + 1] -= 0.5 * tr
                        else:  # -1j
                            out[o, ((2 + cp) * 3 + cl) * 2] -= 0.5 * ti
                            out[o, ((2 + cp) * 3 + cl) * 2 + 1] += 0.5 * tr
    return out


def simulate(psi, U):
    """Full-lattice numpy fp16 simulation -> complex64 (T,Z,Y,X,4,3)."""
    link_vars = _to_planar_links(U)
    psi_vars = _to_planar_psi(psi)
    out = np.zeros((T, 24, NSITE), np.float32)
    for core in range(NCORES):
        out[core * TLOC:(core + 1) * TLOC] = simulate_core(link_vars, psi_vars, core * TLOC)
    res = (out[:, 0::2, :] + 1j * out[:, 1::2, :]).astype(np.complex64)
    return res.transpose(0, 2, 1).reshape(T, Z, Y, X, NS, NCOL)


# =================================================================== bass

LINK_KEYS = (
    [('c', d) for d in range(4)]
    + [('f', d, e) for (d, e) in
       [(0, 1), (0, 2), (0, 3), (2, 1), (3, 1), (3, 2), (1, 2), (1, 3), (2, 3)]]
    + [('b', d) for d in (1, 2, 3)]
)
PSI_KEYS = [('c',)] + [('s', e, sgn) for e in (1, 2, 3) for sgn in (1, -1)]


def _lname(key):
    return "u_" + "_".join(str(x) for x in key).replace('-', 'm')


def _pname(key):
    return "psi_" + "_".join(str(x) for x in key).replace('-', 'm')


def _bbuild_table():
    """Per chirality block: list of (plane, A, B(<=A), tgt_im, f9comp, coef)."""
    offd = {(0, 1): 0, (0, 2): 1, (1, 2): 2}
    tables = [[], []]
    for blk, sigs in enumerate((SIG_UP, SIG_DN)):
        for p in range(6):
            sig = sigs[p]
            for a in range(2):
                for b in range(2):
                    s = sig[a, b]
                    if abs(s) < 1e-12:
                        continue
                    cf = -1j * CCLOV * s
                    for i in range(3):
                        for j in range(3):
                            A_, B_ = a * 3 + i, b * 3 + j
                            if A_ < B_:
                                continue
                            if i == j:
                                fre = None
                                fim = (6 + i, 1.0)
                            elif (i, j) in offd:
                                q = offd[(i, j)]
                                fre = (2 * q, 1.0); fim = (2 * q + 1, 1.0)
                            else:
                                q = offd[(j, i)]
                                fre = (2 * q, -1.0); fim = (2 * q + 1, 1.0)
                            cr, ci = cf.real, cf.imag
                            for tgt_im, parts in ((0, [(fre, cr), (fim, -ci)]),
                                                  (1, [(fim, cr), (fre, ci)])):
                                if A_ == B_ and tgt_im:
                                    continue
                                for src, c0 in parts:
                                    if src is None or abs(c0) < 1e-15:
                                        continue
                                    comp, s0 = src
                                    tables[blk].append((p, A_, B_, tgt_im, comp, c0 * s0))
    # sanity: every lower-tri re comp and offdiag im comp gets >=1 write
    for blk in range(2):
        seen = {(A_, B_, t) for (_, A_, B_, t, _, _) in tables[blk]}
        for A_ in range(6):
            for B_ in range(A_ + 1):
                assert (A_, B_, 0) in seen, (blk, A_, B_)
                if A_ != B_:
                    assert (A_, B_, 1) in seen, (blk, A_, B_)
    return tables


BTABLES = _bbuild_table()


def _axis_pieces(d, L):
    """dst[i] = src[(i+d) % L] -> (dst_start, src_start, length) pieces."""
    d %= L
    if d == 0:
        return [(0, 0, L)]
    return [(0, d, L - d), (L - d, 0, d)]


def _build_device_program():
    import concourse.bacc as bacc
    import concourse.mybir as mybir
    from concourse import tile as ctile

    FP16, FP32 = mybir.dt.float16, mybir.dt.float32
    AL = mybir.AluOpType
    nc = bacc.Bacc(None, target_bir_lowering=False)

    u_in = {k: nc.declare_dram_parameter(_lname(k), [NWIN, P, 18, F], FP16, isOutput=False)
            for k in LINK_KEYS}
    p_in = {k: nc.declare_dram_parameter(_pname(k), [NWIN, P, 24, F], FP16, isOutput=False)
            for k in PSI_KEYS}
    out_dram = nc.declare_dram_parameter("out", [TLOC, P, 24, F], FP32, isOutput=True)

    dbg = {}
    if DEBUG_DUMP:
        dbg['g'] = nc.declare_dram_parameter("dbg_g", [6, NWIN, 9, NSITE], FP16, isOutput=True)
        dbg['ft'] = nc.declare_dram_parameter("dbg_ft", [6, P, 9, F], FP16, isOutput=True)
        dbg['bb'] = nc.declare_dram_parameter("dbg_bb", [2, P, 72, F], FP16, isOutput=True)
        dbg['ap'] = nc.declare_dram_parameter("dbg_ap", [P, 24, F], FP16, isOutput=True)
    gps = [[nc.dram_tensor(f"gp{p}_{w}", [9, NSITE], FP16) for w in range(NWIN)]
           for p in range(6)]
    # deduped shifted-G buffers keyed (plane, w_src, spatial shift)
    shuf_map = {}
    for p in range(6):
        for k, (dt, dz, dy, dx) in enumerate(DELTAS[p]):
            for o in range(TLOC):
                wsrc = o + 2 + dt
                key = (p, wsrc, dz, dy, dx)
                if key not in shuf_map:
                    shuf_map[key] = nc.dram_tensor(
                        f"gsh{p}_{wsrc}_{dz}_{dy}_{dx}".replace('-', 'm'),
                        [9, NSITE], FP16)

    def emit_cmatmul(pool, out_t, a_t, b_t, dag_b, eng=None, tp=""):
        """out = A @ B(^+), 3x3 complex (30 ops, per output column)."""
        eng = eng if eng is not None else nc.vector
        P4 = {}
        for ra in (0, 1):
            for rb in (0, 1):
                P4[(ra, rb)] = pool.tile([P, 9, F], FP16, tag=f"mmP{ra}{rb}{tp}",
                                         name=f"mmP{ra}{rb}{tp}", bufs=1)
        Dre = pool.tile([P, 9, F], FP16, tag="mmDre" + tp, name="mmDre" + tp, bufs=1)
        Dim = pool.tile([P, 9, F], FP16, tag="mmDim" + tp, name="mmDim" + tp, bufs=1)
        av_all = a_t[:].rearrange("p (i j r) f -> p i j r f", i=3, j=3)
        bv_all = b_t[:].rearrange("p (j k r) f -> p j k r f", j=3, k=3)
        bv_dag = b_t[:].rearrange("p (k j r) f -> p k j r f", k=3, j=3)
        ov_all = out_t[:].rearrange("p (i k r) f -> p i k r f", i=3, k=3)
        for k in range(3):
            for (ra, rb), pt in P4.items():
                if dag_b:
                    bsel = bv_dag[:, k, :, rb, :]  # B[k,j]: [P, j(3), F]
                else:
                    bsel = bv_all[:, :, k, rb, :]  # B[j,k]: [P, j(3), F]
                bb = bsel.unsqueeze(1).broadcast_to([P, 3, 3, F])
                eng.tensor_mul(
                    pt[:].rearrange("p (i j) f -> p i j f", i=3),
                    av_all[:, :, :, ra, :], bb)
            if dag_b:
                eng.tensor_add(Dre[:], P4[(0, 0)][:], P4[(1, 1)][:])
                eng.tensor_sub(Dim[:], P4[(1, 0)][:], P4[(0, 1)][:])
            else:
                eng.tensor_sub(Dre[:], P4[(0, 0)][:], P4[(1, 1)][:])
                eng.tensor_add(Dim[:], P4[(0, 1)][:], P4[(1, 0)][:])
            for r, Dt in ((0, Dre), (1, Dim)):
                ov = ov_all[:, :, k, r, :]  # [P, i(3), F]
                Dv = Dt[:].rearrange("p (i j) f -> p i j f", i=3)
                eng.tensor_add(ov, Dv[:, :, 0, :], Dv[:, :, 1, :])
                eng.tensor_add(ov, ov, Dv[:, :, 2, :])

    def emit_cmatvec(pool, uh_t, u_t, h_t, dag):
        """uh[c,i] = sum_j Utilde[i,j] h[c,j]; h/uh: [P,12,F]; fused over c."""
        if dag:
            uv = u_t[:].rearrange("p (j i r) f -> p i j r f", j=3, i=3)
        else:
            uv = u_t[:].rearrange("p (i j r) f -> p i j r f", i=3, j=3)
        hv = h_t[:].rearrange("p (c cl r) f -> p c cl r f", c=2, cl=3)
        ov = uh_t[:].rearrange("p (c i r) f -> p c i r f", c=2, i=3)
        P4 = {}
        for ra in (0, 1):
            for rb in (0, 1):
                P4[(ra, rb)] = pool.tile([P, 18, F], FP16, tag=f"mvP{ra}{rb}",
                                         name=f"mvP{ra}{rb}", bufs=1)
        Dre = pool.tile([P, 18, F], FP16, tag="mvDre", name="mvDre", bufs=1)
        Dim = pool.tile([P, 18, F], FP16, tag="mvDim", name="mvDim", bufs=1)
        for c in range(2):
            for (ra, rb), pt in P4.items():
                hb = hv[:, c, :, rb, :].unsqueeze(1).broadcast_to([P, 3, 3, F])
                nc.vector.tensor_mul(
                    pt[:].rearrange("p (c2 i j) f -> p c2 i j f", c2=2, i=3)[:, c],
                    uv[:, :, :, ra, :], hb)
        if dag:
            # conj is on U (first factor): im = Ur*hi - Ui*hr
            nc.vector.tensor_add(Dre[:], P4[(0, 0)][:], P4[(1, 1)][:])
            nc.vector.tensor_sub(Dim[:], P4[(0, 1)][:], P4[(1, 0)][:])
        else:
            nc.vector.tensor_sub(Dre[:], P4[(0, 0)][:], P4[(1, 1)][:])
            nc.vector.tensor_add(Dim[:], P4[(0, 1)][:], P4[(1, 0)][:])
        ov2 = uh_t[:].rearrange("p (ci r) f -> p ci r f", ci=6)
        for r, Dt in ((0, Dre), (1, Dim)):
            o1 = ov2[:, :, r, :]  # [P, (c i)(6), F]
            Dv = Dt[:].rearrange("p (ci j) f -> p ci j f", ci=6)
            nc.vector.tensor_add(o1, Dv[:, :, 0, :], Dv[:, :, 1, :])
            nc.vector.tensor_add(o1, o1, Dv[:, :, 2, :])

    POOL_CMM = False
    _shuf_engs = [nc.scalar]
    _shuf_idx = [0]

    def _next_shuf_eng():
        _shuf_idx[0] += 1
        return _shuf_engs[_shuf_idx[0] % len(_shuf_engs)]

    with ctile.TileContext(nc) as tc:
        # ---------------- phase 1: G build ----------------
        with tc.tile_pool(name="lnk", bufs=2) as lnk, \
             tc.tile_pool(name="gtmp", bufs=2) as gtmp, \
             tc.tile_pool(name="gout", bufs=2) as goutp:
            for w in range(6):
                cache = {}

                def load_link(key, wi, tag):
                    ck = (key, wi)
                    if ck not in cache:
                        t = lnk.tile([P, 18, F], FP16, tag=tag, name=tag)
                        nc.sync.dma_start(t[:], u_in[key][wi])
                        cache[ck] = t
                    return cache[ck]

                for p, (mu, nu) in enumerate(PAIRS):
                    if mu != 0 and w < 2:
                        continue
                    if mu == 0:
                        M1 = load_link(('c', 0), w, "m1_" + str(p))
                        M2 = load_link(('c', nu), w + 1, "m2_" + str(p))
                        M3 = load_link(('c', nu), w, "m3_" + str(p))
                        M4 = load_link(('f', 0, nu), w, "m4_" + str(p))
                    else:
                        M1 = load_link(('c', mu), w, "m1_" + str(p))
                        M2 = load_link(('f', nu, mu), w, "m2_" + str(p))
                        M3 = load_link(('c', nu), w, "m3_" + str(p))
                        M4 = load_link(('f', mu, nu), w, "m4_" + str(p))
                    # offload some units' independent A/B products to Pool
                    on_pool = ((2 * p + w) % 3 == 0) and POOL_CMM
                    At = gtmp.tile([P, 18, F], FP16, tag="A", name="A")
                    Bt = gtmp.tile([P, 18, F], FP16, tag="B", name="B")
                    Wt = gtmp.tile([P, 18, F], FP16, tag="W", name="W")
                    peng = nc.gpsimd if on_pool else nc.vector
                    ptp = "g" if on_pool else ""
                    emit_cmatmul(gtmp, At, M1, M2, dag_b=False, eng=peng, tp=ptp)
                    emit_cmatmul(gtmp, Bt, M3, M4, dag_b=False, eng=peng, tp=ptp)
                    emit_cmatmul(gtmp, Wt, At, Bt, dag_b=True)
                    Gt = goutp.tile([P, 9, F], FP16, tag="G", name="G")
                    offd = [(0, 1), (0, 2), (1, 2)]
                    for q, (i, j) in enumerate(offd):
                        a_, b_ = (i * 3 + j) * 2, (j * 3 + i) * 2
                        nc.vector.tensor_sub(Gt[:, 2 * q:2 * q + 1, :],
                                             Wt[:, a_:a_ + 1, :], Wt[:, b_:b_ + 1, :])
                        nc.vector.tensor_add(Gt[:, 2 * q + 1:2 * q + 2, :],
                                             Wt[:, a_ + 1:a_ + 2, :], Wt[:, b_ + 1:b_ + 2, :])
                    for d in range(3):
                        c_ = (d * 3 + d) * 2 + 1
                        nc.vector.tensor_scalar_mul(Gt[:, 6 + d:7 + d, :],
                                                    Wt[:, c_:c_ + 1, :], 2.0)
                    nc.scalar.dma_start(
                        gps[p][w].rearrange("c (p2 f) -> p2 c f", p2=P), Gt[:])
                    if DEBUG_DUMP:
                        nc.sync.dma_start(
                            dbg['g'][p, w].rearrange("c (p2 f) -> p2 c f", p2=P), Gt[:])

                # deduped G shuffles whose source slice just became ready
                for (p, wsrc, dz, dy, dx), buf in shuf_map.items():
                    if wsrc != w:
                        continue
                    src = gps[p][w].rearrange("c (z y x) -> c z y x", z=Z, y=Y)
                    dst = buf.rearrange("c (z y x) -> c z y x", z=Z, y=Y)
                    qeng = _next_shuf_eng()
                    for (zd, zs, zl) in _axis_pieces(dz, Z):
                        for (yd, ys, yl) in _axis_pieces(dy, Y):
                            for (xd, xs, xl) in _axis_pieces(dx, X):
                                with nc.allow_non_contiguous_dma(reason="wrap"):
                                    qeng.dma_start(
                                        dst[:, zd:zd + zl, yd:yd + yl, xd:xd + xl],
                                        src[:, zs:zs + zl, ys:ys + yl, xs:xs + xl])

        # ---------------- phase 2: apply + hop ----------------
        with tc.tile_pool(name="gld", bufs=2) as gld, \
             tc.tile_pool(name="ftl", bufs=2) as ftl, \
             tc.tile_pool(name="bbl", bufs=2) as bbl, \
             tc.tile_pool(name="psl", bufs=2) as psl, \
             tc.tile_pool(name="uhp", bufs=2) as uhp, \
             tc.tile_pool(name="htm", bufs=2) as htm, \
             tc.tile_pool(name="oot", bufs=2) as oot:
            for o in range(TLOC):
                w = o + 2
                # F_tilde per plane
                ftil = []
                for p in range(6):
                    g0 = gld.tile([P, 9, F], FP16, tag="g0", name="g0")
                    nc.sync.dma_start(g0[:], gps[p][w].rearrange("c (p2 f) -> p2 c f", p2=P))
                    ft = ftl.tile([P, 9, F], FP16, tag=f"ft{p}", name=f"ft{p}")
                    first = True
                    for k in range(3):
                        dt, dz, dy, dx = DELTAS[p][k]
                        gbuf = shuf_map[(p, o + 2 + dt, dz, dy, dx)]
                        gk = gld.tile([P, 9, F], FP16, tag=f"g{k + 1}", name=f"g{k + 1}")
                        nc.sync.dma_start(gk[:], gbuf.rearrange("c (p2 f) -> p2 c f", p2=P))
                        if first:
                            nc.vector.tensor_sub(ft[:], g0[:], gk[:])
                            first = False
                        else:
                            nc.vector.tensor_sub(ft[:], ft[:], gk[:])
                    if DEBUG_DUMP and o == 0:
                        nc.sync.dma_start(dbg['ft'][p], ft[:])
                    ftil.append(ft)

                # B blocks: block-structured build.
                # B/c = [[M~, L~+],[L~, -M~]] (hermitian), from raw F-combos:
                #   M9 = F3 + s*F2 ; S9 = F4 - s*F1 ; T9 = s*F0 + F5  (s=+1 blk0, -1 blk1)
                # CCLOV scale applied via pre-scaled psi; DIAG handled post-apply.
                bts = [bbl.tile([P, 72, F], FP16, tag=f"B{blk}", name=f"B{blk}") for blk in range(2)]
                stt_t = [bbl.tile([P, 18, F], FP16, tag=f"ST{blk}", name=f"ST{blk}") for blk in range(2)]
                for blk in range(2):
                    bt = bts[blk]
                    bv = bt[:].rearrange("p (A B r) f -> p A B r f", A=6, B=6)
                    sv = bt[:].rearrange("p (A B r) f -> p B A r f", A=6, B=6)
                    St = stt_t[blk][:, 0:9, :]
                    Tt = stt_t[blk][:, 9:18, :]
                    Ft = [ftil[p] for p in range(6)]
                    if blk == 0:
                        nc.vector.tensor_sub(St, Ft[4][:], Ft[1][:])
                        nc.vector.tensor_add(Tt, Ft[0][:], Ft[5][:])
                    else:
                        nc.vector.tensor_add(St, Ft[4][:], Ft[1][:])
                        nc.vector.tensor_sub(Tt, Ft[5][:], Ft[0][:])

                    def madd(dst, ca, cb):  # dst = F3[ca] + s*F2[cb-slice]
                        if blk == 0:
                            nc.vector.tensor_add(dst, Ft[3][:, ca, :], Ft[2][:, cb, :])
                        else:
                            nc.vector.tensor_sub(dst, Ft[3][:, ca, :], Ft[2][:, cb, :])

                    def mneg(dst, ca, cb):  # dst = -(F3[ca] + s*F2[cb])
                        if blk == 0:
                            nc.vector.scalar_tensor_tensor(
                                dst, Ft[3][:, ca, :], -1.0, Ft[2][:, cb, :],
                                AL.mult, AL.subtract)
                        else:
                            nc.vector.tensor_sub(dst, Ft[2][:, cb, :], Ft[3][:, ca, :])

                    odd2, odd1 = slice(1, 5, 2), slice(5, 6)
                    ev2, ev1 = slice(0, 4, 2), slice(4, 5)
                    # UL quadrant: up.re / up.im
                    madd(bv[:, 0, 1:3, 0, :], odd2, odd2)
                    madd(bv[:, 1, 2:3, 0, :], odd1, odd1)
                    mneg(bv[:, 0, 1:3, 1, :], ev2, ev2)
                    mneg(bv[:, 1, 2:3, 1, :], ev1, ev1)
                    # UL lo.re / lo.im
                    madd(bv[:, 1, 0:1, 0, :], slice(1, 2), slice(1, 2))
                    madd(bv[:, 2, 0:2, 0, :], slice(3, 7, 2), slice(3, 7, 2))
                    madd(bv[:, 1, 0:1, 1, :], slice(0, 1), slice(0, 1))
                    madd(bv[:, 2, 0:2, 1, :], slice(2, 6, 2), slice(2, 6, 2))
                    # UL diag: re = M9[6+d]; im = 0
                    madd(bt[:, 0:29:14, :], slice(6, 9), slice(6, 9))
                    nc.vector.memzero(bt[:, 1:30:14, :])
                    # LL: up.re = S[2q]+T[2q+1] ; up.im = S[2q+1]-T[2q]
                    nc.vector.tensor_add(bv[:, 3, 1:3, 0, :], St[:, 0:4:2, :], Tt[:, 1:5:2, :])
                    nc.vector.tensor_add(bv[:, 4, 2:3, 0, :], St[:, 4:5, :], Tt[:, 5:6, :])
                    nc.vector.tensor_sub(bv[:, 3, 1:3, 1, :], St[:, 1:5:2, :], Tt[:, 0:4:2, :])
                    nc.vector.tensor_sub(bv[:, 4, 2:3, 1, :], St[:, 5:6, :], Tt[:, 4:5, :])
                    # LL lo.re = -S[2q]+T[2q+1] ; lo.im = S[2q+1]+T[2q]
                    nc.vector.tensor_sub(bv[:, 4, 0:1, 0, :], Tt[:, 1:2, :], St[:, 0:1, :])
                    nc.vector.tensor_sub(bv[:, 5, 0:2, 0, :], Tt[:, 3:7:2, :], St[:, 2:6:2, :])
                    nc.vector.tensor_add(bv[:, 4, 0:1, 1, :], St[:, 1:2, :], Tt[:, 0:1, :])
                    nc.vector.tensor_add(bv[:, 5, 0:2, 1, :], St[:, 3:7:2, :], Tt[:, 2:6:2, :])
                    # LL diag: re = T[6+d], im = S[6+d]  (comps 36/50/64, 37/51/65)
                    nc.scalar.copy(bt[:, 36:65:14, :], Tt[:, 6:9, :])
                    nc.scalar.copy(bt[:, 37:66:14, :], St[:, 6:9, :])
                    # LR = -UL  (flattened (B,r) view keeps the AP 3-D)
                    bv2 = bt[:].rearrange("p (A BR) f -> p A BR f", A=6)
                    nc.scalar.mul(bv2[:, 3:6, 6:12, :], bv2[:, 0:3, 0:6, :], -1.0)
                    # UR = conj-transpose(LL)
                    nc.scalar.copy(bv[:, 0:3, 3:6, 0, :], sv[:, 0:3, 3:6, 0, :])
                    nc.scalar.mul(bv[:, 0:3, 3:6, 1, :], sv[:, 0:3, 3:6, 1, :], -1.0)

                # apply B to psi -> out tile (psi pre-scaled by CCLOV for the
                # F-part; the (4+m)+identity diagonal added afterwards via STT)
                psi_cr = psl.tile([P, 24, F], FP16, tag="pscr", name="pscr")
                nc.sync.dma_start(psi_cr[:], p_in[('c',)][w])
                psi_c = psl.tile([P, 24, F], FP16, tag="psc", name="psc")
                nc.vector.tensor_scalar_mul(psi_c[:], psi_cr[:], CCLOV)
                out_t = oot.tile([P, 24, F], FP16, tag="out", name="out")
                aptmp = htm.tile([P, 6, F], FP16, tag="aptmp", name="aptmp")
                aptm2 = htm.tile([P, 12, F], FP16, tag="aptm2", name="aptm2")
                for blk in range(2):
                    bt = bts[blk]
                    bv = bt[:].rearrange("p (a b r) f -> p a b r f", a=6, b=6)
                    ovv = out_t[:].rearrange("p (s r) f -> p s r f", r=2)
                    pvv = psi_c[:].rearrange("p (s r) f -> p s r f", r=2)
                    out_ri = out_t[:, blk * 12:(blk + 1) * 12, :]  # [P,12,F] (A,r)
                    out_re = ovv[:, blk * 6:(blk + 1) * 6, 0, :]
                    out_im = ovv[:, blk * 6:(blk + 1) * 6, 1, :]
                    for B_ in range(6):
                        sB = blk * 6 + B_
                        pr = pvv[:, sB:sB + 1, 0, :].broadcast_to([P, 6, F])
                        pi = pvv[:, sB:sB + 1, 1, :].broadcast_to([P, 6, F])
                        # psi (re,im) pair broadcast over A: [P, A(6), r(2), F]
                        pri = (psi_c[:, sB * 2:sB * 2 + 2, :]
                               .unsqueeze(1).broadcast_to([P, 6, 2, F]))
                        Brv = bv[:, :, B_, 0, :]
                        # Br broadcast over r: [P, A(6), r(2), F]
                        Brr = Brv.unsqueeze(2).broadcast_to([P, 6, 2, F])
                        Biv = bv[:, :, B_, 1, :]
                        ori = out_ri.rearrange("p (a r) f -> p a r f", a=6)
                        if B_ == 0:
                            nc.vector.tensor_mul(ori, Brr, pri)
                        else:
                            nc.vector.tensor_mul(
                                aptm2[:].rearrange("p (a r) f -> p a r f", a=6),
                                Brr, pri)
                            nc.vector.tensor_add(out_ri, out_ri, aptm2[:])
                        nc.vector.tensor_mul(aptmp[:], Biv, pi)
                        nc.vector.tensor_sub(out_re, out_re, aptmp[:])
                        nc.vector.tensor_mul(aptmp[:], Biv, pr)
                        nc.vector.tensor_add(out_im, out_im, aptmp[:])

                # diagonal (4+m)+identity term, on the unscaled psi
                nc.vector.scalar_tensor_tensor(
                    out_t[:], psi_cr[:], DIAG, out_t[:], AL.mult, AL.add)

                if DEBUG_DUMP and o == 0:
                    for blk in range(2):
                        nc.sync.dma_start(dbg['bb'][blk], bts[blk][:])
                    nc.sync.dma_start(dbg['ap'][:], out_t[:])

                # hop terms
                for mu in (range(4) if ENABLE_HOP else ()):
                    tbl = HOP[mu]
                    for sgn in (1, -1):
                        # psi source tile
                        psv = psl.tile([P, 24, F], FP16, tag="psv", name="psv")
                        if mu == 0:
                            nc.sync.dma_start(psv[:], p_in[('c',)][w + (1 if sgn > 0 else -1)])
                        else:
                            nc.sync.dma_start(psv[:], p_in[('s', mu, 1 if sgn > 0 else -1)][w])
                        # U tile
                        ut = uhp.tile([P, 18, F], FP16, tag="ut", name="ut")
                        if sgn > 0:
                            nc.sync.dma_start(ut[:], u_in[('c', mu)][w])
                        elif mu == 0:
                            nc.sync.dma_start(ut[:], u_in[('c', 0)][w - 1])
                        else:
                            nc.sync.dma_start(ut[:], u_in[('b', mu)][w])
                        # projection -> h [P,12,F]
                        h = htm.tile([P, 12, F], FP16, tag="h", name="h")
                        pvv = psv[:].rearrange("p (s r) f -> p s r f", r=2)
                        hvv = h[:].rearrange("p (s r) f -> p s r f", r=2)
                        for c in range(2):
                            cf = sgn * tbl['coef'][c]
                            b_ = tbl['b'][c]
                            if cf == 1:
                                nc.vector.tensor_add(h[:, c * 6:(c + 1) * 6, :],
                                                     psv[:, c * 6:(c + 1) * 6, :],
                                                     psv[:, b_ * 6:(b_ + 1) * 6, :])
                            elif cf == -1:
                                nc.vector.tensor_sub(h[:, c * 6:(c + 1) * 6, :],
                                                     psv[:, c * 6:(c + 1) * 6, :],
                                                     psv[:, b_ * 6:(b_ + 1) * 6, :])
                            else:
                                hre = hvv[:, c * 3:(c + 1) * 3, 0, :]
                                him = hvv[:, c * 3:(c + 1) * 3, 1, :]
                                pre = pvv[:, c * 3:(c + 1) * 3, 0, :]
                                pim = pvv[:, c * 3:(c + 1) * 3, 1, :]
                                qre = pvv[:, b_ * 3:(b_ + 1) * 3, 0, :]
                                qim = pvv[:, b_ * 3:(b_ + 1) * 3, 1, :]
                                if cf == 1j:
                                    nc.vector.tensor_sub(hre, pre, qim)
                                    nc.vector.tensor_add(him, pim, qre)
                                else:  # -1j
                                    nc.vector.tensor_add(hre, pre, qim)
                                    nc.vector.tensor_sub(him, pim, qre)
                        # color mult
                        uh = htm.tile([P, 12, F], FP16, tag="uh", name="uh")
                        emit_cmatvec(uhp, uh, ut, h, dag=(sgn < 0))
                        # accumulate into out (rows 0,1 in one op)
                        sl = out_t[:, 0:12, :]
                        nc.vector.scalar_tensor_tensor(
                            sl, uh[:, 0:12, :], -0.5, sl, AL.mult, AL.add)
                        uvv = uh[:].rearrange("p (s r) f -> p s r f", r=2)
                        ovv = out_t[:].rearrange("p (s r) f -> p s r f", r=2)
                        rcs = [sgn * tbl['rc'][cp] for cp in range(2)]
                        if rcs[0] == rcs[1] and tbl['m'] == (0, 1) and rcs[0] in (1, -1):
                            sl = out_t[:, 12:24, :]
                            nc.vector.scalar_tensor_tensor(
                                sl, uh[:, 0:12, :], -0.5 * rcs[0], sl,
                                AL.mult, AL.add)
                            continue
                        for cp in range(2):
                            rc = rcs[cp]
                            mm = tbl['m'][cp]
                            row = 2 + cp
                            if rc in (1, -1):
                                sl = out_t[:, row * 6:(row + 1) * 6, :]
                                nc.vector.scalar_tensor_tensor(
                                    sl, uh[:, mm * 6:(mm + 1) * 6, :], -0.5 * rc, sl,
                                    AL.mult, AL.add)
                            else:
                                s_i = rc.imag
                                o_re = ovv[:, row * 3:(row + 1) * 3, 0, :]
                                o_im = ovv[:, row * 3:(row + 1) * 3, 1, :]
                                u_re = uvv[:, mm * 3:(mm + 1) * 3, 0, :]
                                u_im = uvv[:, mm * 3:(mm + 1) * 3, 1, :]
                                nc.vector.scalar_tensor_tensor(
                                    o_re, u_im, 0.5 * s_i, o_re, AL.mult, AL.add)
                                nc.vector.scalar_tensor_tensor(
                                    o_im, u_re, -0.5 * s_i, o_im, AL.mult, AL.add)

                # store (fp16 -> fp32 cast via SWDGE)
                nc.gpsimd.dma_start(out_dram[o], out_t[:])

    nc.finalize()
    return nc


_PROG_CACHE = {}


def _get_program():
    if 'nc' not in _PROG_CACHE:
        _PROG_CACHE['nc'] = _build_device_program()
    return _PROG_CACHE['nc']


def _sbuf_image(a, C):
    """[T, C, NSITE] -> [T, P, C, F] contiguous."""
    return np.ascontiguousarray(a.reshape(T, C, P, F).transpose(0, 2, 1, 3))


def build_in_maps(psi, U):
    link_vars = _to_planar_links(U)
    psi_vars = _to_planar_psi(psi)
    link_imgs = {k: _sbuf_image(v, 18) for k, v in link_vars.items()}
    psi_imgs = {k: _sbuf_image(v, 24) for k, v in psi_vars.items()}
    in_maps = []
    for core in range(NCORES):
        t0 = core * TLOC
        tw = [(t0 - 2 + w) % T for w in range(NWIN)]
        m = {}
        for k in LINK_KEYS:
            m[_lname(k)] = np.ascontiguousarray(link_imgs[k][tw])
        for k in PSI_KEYS:
            m[_pname(k)] = np.ascontiguousarray(psi_imgs[k][tw])
        in_maps.append(m)
    return in_maps


def assemble_output(results):
    out = np.empty((T, 24, NSITE), np.float32)
    for core in range(NCORES):
        r = results[core]['out']  # [TLOC, P, 24, F] fp32
        out[core * TLOC:(core + 1) * TLOC] = r.transpose(0, 2, 1, 3).reshape(TLOC, 24, NSITE)
    res = (out[:, 0::2, :] + 1j * out[:, 1::2, :]).astype(np.complex64)
    return res.transpose(0, 2, 1).reshape(T, Z, Y, X, NS, NCOL)


def kernel(psi, U):
    psi = np.asarray(psi)
    U = np.asarray(U)
    from concourse.bass_utils import run_bass_kernel_spmd
    nc = _get_program()
    in_maps = build_in_maps(psi, U)
    res = run_bass_kernel_spmd(nc, in_maps, core_ids=list(range(NCORES)))
    return assemble_output(res.results)



# revision 17
# speedup vs baseline: 1.3186x; 1.1010x over previous
"""Clover-Wilson Dirac operator on Trainium2 (8 NeuronCores, T-sharded).

Math summary (derived + numerically verified against the reference):
- The reference's 4-leaf "clover" Q for plane (mu,nu) factorizes as
      Q(x) = W(x) + W(x+d1)^+ + W(x+d2)^+ + W(x+d3)^+
  with W(x) = [U_mu(x) U_nu(x+mu)] [U_nu(x) U_mu(x+nu)]^+,
  d1 = nu-mu, d2 = -2mu-2nu, d3 = -2nu (unit lattice vectors).
- With G = W - W^+ (anti-Hermitian), Ftil := Q - Q^+ = G(x) - G(x+d1) - G(x+d2) - G(x+d3).
- C psi + (4+m) psi = (5+m) psi + (csw/32) * sum_p (sigma_p (x) (-i Ftil_p)) psi,
  where sigma_p is block-diagonal (2x2 chiral blocks) in this basis.
- Wilson hop uses the standard spin-projection trick (2 half-spinors per direction).

Distribution: T=32 sharded 4 slices per core; U needs halo t0-2..t0+4 (7 slices),
psi needs t0-1..t0+4. All jnp.roll shifts are pushed into host-precomputed
pre-rolled planar fp16 arrays; on-device shifted reads of the intermediate G
use DRAM->DRAM affine shuffle DMAs.
"""
import numpy as np

T, Z, Y, X = 32, 24, 24, 24
NCOL, NS = 3, 4
MASS, CSW = 0.1, 1.0
PAIRS = [(0, 1), (0, 2), (0, 3), (1, 2), (1, 3), (2, 3)]
NCORES = 8
TLOC = T // NCORES          # 4 output slices per core
NSITE = Z * Y * X           # 13824
P = 128
F = NSITE // P              # 108
NWIN = 7                    # U window slices: t0-2 .. t0+4
DIAG = 5.0 + MASS           # (4+m) + clover identity
CCLOV = CSW / 32.0          # |coefficient| of sigma (x) Ftil; overall factor -i


# ----------------------------------------------------------------- tables

def _gammas():
    i = 1j
    g0 = np.array([[0, 0, 1, 0], [0, 0, 0, 1], [1, 0, 0, 0], [0, 1, 0, 0]], np.complex128)
    g1 = np.array([[0, 0, 0, i], [0, 0, i, 0], [0, -i, 0, 0], [-i, 0, 0, 0]], np.complex128)
    g2 = np.array([[0, 0, 0, -1], [0, 0, 1, 0], [0, 1, 0, 0], [-1, 0, 0, 0]], np.complex128)
    g3 = np.array([[0, 0, i, 0], [0, 0, 0, -i], [-i, 0, 0, 0], [0, i, 0, 0]], np.complex128)
    return [g0, g1, g2, g3]


def _sigma_blocks():
    """Chiral 2x2 blocks of sigma_{mu nu} = i g_mu g_nu for each plane."""
    G = _gammas()
    ups, dns = [], []
    for mu, nu in PAIRS:
        s = 1j * (G[mu] @ G[nu])
        assert np.abs(s[:2, 2:]).max() < 1e-12 and np.abs(s[2:, :2]).max() < 1e-12
        ups.append(s[:2, :2].copy())
        dns.append(s[2:, 2:].copy())
    return ups, dns


SIG_UP, SIG_DN = _sigma_blocks()

# per-plane shift deltas (t, z, y, x) for the W-factorization
def _deltas():
    out = []
    for mu, nu in PAIRS:
        e_mu = np.zeros(4, np.int64); e_mu[mu] = 1
        e_nu = np.zeros(4, np.int64); e_nu[nu] = 1
        out.append([tuple(e_nu - e_mu), tuple(-2 * e_mu - 2 * e_nu), tuple(-2 * e_nu)])
    return out


DELTAS = _deltas()

# debug toggles (affect both simulate_core and the device program)
ENABLE_CLOVER = True
ENABLE_HOP = True
DEBUG_DUMP = False

# hop projection tables: psi_h[c] = psi[c] + coef * psi[b[c]]; lower rows:
# row_{2+c} = rc[c] * h[m[c]]  (forward, i.e. (1-gamma)); backward negates
# coef and rc. Verified against gammas in _check_hop_tables().
HOP = {
    0: dict(b=(2, 3), coef=(-1, -1), m=(0, 1), rc=(-1, -1)),
    1: dict(b=(3, 2), coef=(-1j, -1j), m=(1, 0), rc=(1j, 1j)),
    2: dict(b=(3, 2), coef=(1, -1), m=(1, 0), rc=(-1, 1)),
    3: dict(b=(2, 3), coef=(-1j, 1j), m=(0, 1), rc=(1j, -1j)),
}


def _check_hop_tables():
    G = _gammas()
    for mu, t in HOP.items():
        for sgn in (+1, -1):  # +1: (1-g) fwd ; -1: (1+g) bwd
            M = np.eye(4) - sgn * G[mu]
            # build from table
            B = np.zeros((4, 4), np.complex128)
            for c in range(2):
                B[c, c] += 1
                B[c, t['b'][c]] += sgn * t['coef'][c]
            for c in range(2):
                rc = sgn * t['rc'][c]
                B[2 + c, t['m'][c]] += rc
                B[2 + c, t['b'][t['m'][c]]] += rc * sgn * t['coef'][t['m'][c]]
            assert np.abs(B - M).max() < 1e-12, (mu, sgn, B, M)


_check_hop_tables()


# ------------------------------------------------- planar layout helpers

def _to_planar_links(U):
    """U: (T,Z,Y,X,4,3,3) complex64 -> dict of fp16 planar arrays.

    Returns variants[key] = array [T, 18, NSITE] fp16 with comp c=(i*3+j)*2+r.
    Keys: ('c', d) centered; ('f', d, e) = U_d(x+e_hat) spatial e;
          ('b', d) = U_d(x - d_hat) spatial d.
    """
    Uf32 = np.ascontiguousarray(U)  # complex64
    planar = np.empty((4, T, 18, NSITE), np.float16)
    Um = Uf32.reshape(T, NSITE, 4, 9)
    for d in range(4):
        re = Um[..., d, :].real.astype(np.float16)  # (T, NSITE, 9)
        im = Um[..., d, :].imag.astype(np.float16)
        planar[d, :, 0::2, :] = re.transpose(0, 2, 1)
        planar[d, :, 1::2, :] = im.transpose(0, 2, 1)

    def roll_sites(arr, delta):  # arr [..., NSITE]; value at x+delta
        dz, dy, dx = delta
        a = arr.reshape(*arr.shape[:-1], Z, Y, X)
        if dz: a = np.roll(a, -dz, axis=-3)
        if dy: a = np.roll(a, -dy, axis=-2)
        if dx: a = np.roll(a, -dx, axis=-1)
        return a.reshape(*arr.shape[:-1], NSITE)

    variants = {}
    for d in range(4):
        variants[('c', d)] = planar[d]
    needed_f = {(0, 1), (0, 2), (0, 3), (2, 1), (3, 1), (3, 2), (1, 2), (1, 3), (2, 3)}
    for (d, e) in needed_f:
        delta = [0, 0, 0]; delta[e - 1] = 1
        variants[('f', d, e)] = roll_sites(planar[d], delta)
    for d in (1, 2, 3):
        delta = [0, 0, 0]; delta[d - 1] = -1
        variants[('b', d)] = roll_sites(planar[d], delta)
    return variants


def _to_planar_psi(psi):
    """psi: (T,Z,Y,X,4,3) complex64 -> dict: ('c',) and ('s', e, sgn) ->
    [T, 24, NSITE] fp16, comp c=(s*3+cl)*2+r."""
    pm = psi.reshape(T, NSITE, 12)
    planar = np.empty((T, 24, NSITE), np.float16)
    planar[:, 0::2, :] = pm.real.astype(np.float16).transpose(0, 2, 1)
    planar[:, 1::2, :] = pm.imag.astype(np.float16).transpose(0, 2, 1)

    def roll_sites(arr, delta):
        dz, dy, dx = delta
        a = arr.reshape(*arr.shape[:-1], Z, Y, X)
        if dz: a = np.roll(a, -dz, axis=-3)
        if dy: a = np.roll(a, -dy, axis=-2)
        if dx: a = np.roll(a, -dx, axis=-1)
        return a.reshape(*arr.shape[:-1], NSITE)

    out = {('c',): planar}
    for e in (1, 2, 3):
        for sgn in (1, -1):
            delta = [0, 0, 0]; delta[e - 1] = sgn
            out[('s', e, sgn)] = roll_sites(planar, delta)
    return out


# ------------------------------------------------------ numpy simulator
# Step-wise fp16 mirror of the device dataflow (for validation).

def _cmm16(A, B, dag_b=False):
    """A,B: [18, N] fp16 planar 3x3 complex; returns C = A @ B(^+) fp16."""
    C = np.zeros_like(A)
    for i in range(3):
        for k in range(3):
            cre = np.zeros(A.shape[-1], np.float16)
            cim = np.zeros(A.shape[-1], np.float16)
            for j in range(3):
                ar = A[(i * 3 + j) * 2]; ai = A[(i * 3 + j) * 2 + 1]
                if dag_b:
                    br = B[(k * 3 + j) * 2]; bi = -B[(k * 3 + j) * 2 + 1].astype(np.float16)
                else:
                    br = B[(j * 3 + k) * 2]; bi = B[(j * 3 + k) * 2 + 1]
                cre = (cre + (ar * br - ai * bi)).astype(np.float16)
                cim = (cim + (ar * bi + ai * br)).astype(np.float16)
            C[(i * 3 + k) * 2] = cre
            C[(i * 3 + k) * 2 + 1] = cim
    return C


def _antiherm9(Wm):
    """W planar 18 -> G = W - W^+ in 9-comp layout:
    q*2 / q*2+1 = re/im of G[i,j] for (i,j) in [(0,1),(0,2),(1,2)]; 6+d = im G[d,d]."""
    G = np.empty((9, Wm.shape[-1]), np.float16)
    offd = [(0, 1), (0, 2), (1, 2)]
    for q, (i, j) in enumerate(offd):
        G[q * 2] = (Wm[(i * 3 + j) * 2] - Wm[(j * 3 + i) * 2]).astype(np.float16)
        G[q * 2 + 1] = (Wm[(i * 3 + j) * 2 + 1] + Wm[(j * 3 + i) * 2 + 1]).astype(np.float16)
    for d in range(3):
        G[6 + d] = (Wm[(d * 3 + d) * 2 + 1] * np.float16(2.0)).astype(np.float16)
    return G


def _f9_entry(F9, i, j):
    """(re, im) pair (arrays or (None, arr)) of Ftil[i,j] from 9-comp planar."""
    offd = {(0, 1): 0, (0, 2): 1, (1, 2): 2}
    if i == j:
        return None, F9[6 + i]
    if (i, j) in offd:
        q = offd[(i, j)]
        return F9[q * 2], F9[q * 2 + 1]
    q = offd[(j, i)]
    return -F9[q * 2], F9[q * 2 + 1]  # G[i>j] = -conj(G[j,i]) -> (-re, +im)


def _roll_sites_np(a, delta):
    dz, dy, dx = delta
    a = a.reshape(*a.shape[:-1], Z, Y, X)
    if dz: a = np.roll(a, -dz, axis=-3)
    if dy: a = np.roll(a, -dy, axis=-2)
    if dx: a = np.roll(a, -dx, axis=-1)
    return a.reshape(*a.shape[:-2], -1) if False else a.reshape(*a.shape[:-4], a.shape[-4] if a.ndim > 3 else -1, NSITE) if False else a.reshape(-1, NSITE) if a.ndim == 4 else a.reshape(NSITE)


def simulate_core(link_vars, psi_vars, t0):
    """Numpy fp16 mirror. link_vars/psi_vars: full-T variant dicts.
    Returns planar out [TLOC, 24, NSITE] float32."""
    tw = [(t0 - 2 + w) % T for w in range(NWIN)]

    def LV(key, w):
        return link_vars[key][tw[w]]

    def PV(key, w):
        return psi_vars[key][tw[w]]

    # ---- phase 1: G per plane per window slice
    Gs = {}
    for p, (mu, nu) in enumerate(PAIRS):
        ws = range(0, 6) if mu == 0 else range(2, 6)
        for w in ws:
            if mu == 0:
                M1, M2 = LV(('c', 0), w), LV(('c', nu), w + 1)
                M3, M4 = LV(('c', nu), w), LV(('f', 0, nu), w)
            else:
                M1, M2 = LV(('c', mu), w), LV(('f', nu, mu), w)
                M3, M4 = LV(('c', nu), w), LV(('f', mu, nu), w)
            A = _cmm16(M1, M2)
            B = _cmm16(M3, M4)
            Wm = _cmm16(A, B, dag_b=True)
            Gs[(p, w)] = _antiherm9(Wm)

    out = np.zeros((TLOC, 24, NSITE), np.float32)
    for o in range(TLOC):
        w = o + 2
        # ---- Ftil per plane
        F9s = []
        for p in range(6):
            acc = Gs[(p, w)].copy()
            for (dt, dz, dy, dx) in DELTAS[p]:
                g = Gs[(p, w + dt)]
                gsh = g.reshape(9, Z, Y, X)
                if dz: gsh = np.roll(gsh, -dz, axis=1)
                if dy: gsh = np.roll(gsh, -dy, axis=2)
                if dx: gsh = np.roll(gsh, -dx, axis=3)
                acc = (acc - gsh.reshape(9, NSITE)).astype(np.float16)
            F9s.append(acc)

        if not ENABLE_CLOVER:
            F9s = [np.zeros((9, NSITE), np.float16) for _ in range(6)]
        # ---- B blocks (full 6x6 complex per chirality block), fp16
        Bblk = [np.zeros((6, 6, 2, NSITE), np.float16) for _ in range(2)]
        for blk, sigs in enumerate((SIG_UP, SIG_DN)):
            for p in range(6):
                sig = sigs[p]
                for a in range(2):
                    for b in range(2):
                        s = sig[a, b]
                        if abs(s) < 1e-12:
                            continue
                        cf = -1j * CCLOV * s  # complex coefficient
                        for i in range(3):
                            for j in range(3):
                                fre, fim = _f9_entry(F9s[p], i, j)
                                A_, B_ = a * 3 + i, b * 3 + j
                                # coeff*(fre + i fim): accumulate re and im
                                cr, ci = cf.real, cf.imag
                                tgt = Bblk[blk][A_, B_]
                                if fre is not None:
                                    if cr: tgt[0] = (tgt[0] + np.float16(cr) * fre).astype(np.float16)
                                    if ci: tgt[1] = (tgt[1] + np.float16(ci) * fre).astype(np.float16)
                                if cr: tgt[1] = (tgt[1] + np.float16(cr) * fim).astype(np.float16)
                                if ci: tgt[0] = (tgt[0] - np.float16(ci) * fim).astype(np.float16)
            for A_ in range(6):
                Bblk[blk][A_, A_, 0] = (Bblk[blk][A_, A_, 0] + np.float16(DIAG)).astype(np.float16)

        # ---- apply B to psi
        psi_c = PV(('c',), w)
        for blk in range(2):
            for A_ in range(6):
                s_out = (blk * 2 + A_ // 3) * 3 + (A_ % 3)  # spinor comp index s*3+cl
                accr = np.zeros(NSITE, np.float16)
                acci = np.zeros(NSITE, np.float16)
                for B_ in range(6):
                    s_in = (blk * 2 + B_ // 3) * 3 + (B_ % 3)
                    pr = psi_c[s_in * 2]; pi = psi_c[s_in * 2 + 1]
                    br = Bblk[blk][A_, B_, 0]; bi = Bblk[blk][A_, B_, 1]
                    accr = (accr + br * pr - bi * pi).astype(np.float16)
                    acci = (acci + br * pi + bi * pr).astype(np.float16)
                out[o, s_out * 2] += accr.astype(np.float32)
                out[o, s_out * 2 + 1] += acci.astype(np.float32)

        # ---- hop terms
        for mu in (range(4) if ENABLE_HOP else ()):
            tbl = HOP[mu]
            for sgn, wpsi_key, woff, ukey, udag in (
                (+1, 'f', +1, ('c', mu), False),
                (-1, 'b', -1, ('b', mu) if mu else ('c', 0), True),
            ):
                if mu == 0:
                    psv = PV(('c',), w + woff)
                else:
                    psv = PV(('s', mu, +1 if sgn > 0 else -1), w)
                uar = LV(ukey, w) if mu else LV(ukey, w + (0 if sgn > 0 else -1))
                # project: h[c] = psi[c] + sgn*coef[c]*psi[b[c]] (2 spins x 3 col)
                h = np.zeros((2, 3, 2, NSITE), np.float16)
                for c in range(2):
                    cf = sgn * tbl['coef'][c]
                    for cl in range(3):
                        pr = psv[(c * 3 + cl) * 2]; pi = psv[(c * 3 + cl) * 2 + 1]
                        qr = psv[(tbl['b'][c] * 3 + cl) * 2]; qi = psv[(tbl['b'][c] * 3 + cl) * 2 + 1]
                        if cf == 1:
                            h[c, cl, 0] = (pr + qr).astype(np.float16); h[c, cl, 1] = (pi + qi).astype(np.float16)
                        elif cf == -1:
                            h[c, cl, 0] = (pr - qr).astype(np.float16); h[c, cl, 1] = (pi - qi).astype(np.float16)
                        elif cf == 1j:
                            h[c, cl, 0] = (pr - qi).astype(np.float16); h[c, cl, 1] = (pi + qr).astype(np.float16)
                        else:  # -1j
                            h[c, cl, 0] = (pr + qi).astype(np.float16); h[c, cl, 1] = (pi - qr).astype(np.float16)
                # color mult: uh[c, i] = sum_j U[i,j] h[c, j] (or U^+ )
                uh = np.zeros((2, 3, 2, NSITE), np.float16)
                for c in range(2):
                    for i in range(3):
                        ar = np.zeros(NSITE, np.float16); ai = np.zeros(NSITE, np.float16)
                        for j in range(3):
                            if udag:
                                ur = uar[(j * 3 + i) * 2]; ui = -uar[(j * 3 + i) * 2 + 1].astype(np.float16)
                            else:
                                ur = uar[(i * 3 + j) * 2]; ui = uar[(i * 3 + j) * 2 + 1]
                            ar = (ar + ur * h[c, j, 0] - ui * h[c, j, 1]).astype(np.float16)
                            ai = (ai + ur * h[c, j, 1] + ui * h[c, j, 0]).astype(np.float16)
                        uh[c, i, 0] = ar; uh[c, i, 1] = ai
                # accumulate: rows 0,1: -1/2*uh[c]; rows 2+c': -1/2*sgn... rc
                for c in range(2):
                    for cl in range(3):
                        out[o, (c * 3 + cl) * 2] -= 0.5 * uh[c, cl, 0].astype(np.float32)
                        out[o, (c * 3 + cl) * 2 + 1] -= 0.5 * uh[c, cl, 1].astype(np.float32)
                for cp in range(2):
                    rc = sgn * tbl['rc'][cp]
                    mm = tbl['m'][cp]
                    for cl in range(3):
                        tr = uh[mm, cl, 0].astype(np.float32); ti = uh[mm, cl, 1].astype(np.float32)
                        if rc == 1:
                            out[o, ((2 + cp) * 3 + cl) * 2] -= 0.5 * tr
                            out[o, ((2 + cp) * 3 + cl) * 2 + 1] -= 0.5 * ti
                        elif rc == -1:
                            out[o, ((2 + cp) * 3 + cl) * 2] += 0.5 * tr
                            out[o, ((2 + cp) * 3 + cl) * 2 + 1] += 0.5 * ti
                        elif rc == 1j:
                            out[o, ((2 + cp) * 3 + cl) * 2] += 0.5 * ti
                            out[o, ((2 + cp) * 3 + cl) * 2 + 1] -= 0.5 * tr
                        else:  # -1j
                            out[o, ((2 + cp) * 3 + cl) * 2] -= 0.5 * ti
                            out[o, ((2 + cp) * 3 + cl) * 2 + 1] += 0.5 * tr
    return out


def simulate(psi, U):
    """Full-lattice numpy fp16 simulation -> complex64 (T,Z,Y,X,4,3)."""
    link_vars = _to_planar_links(U)
    psi_vars = _to_planar_psi(psi)
    out = np.zeros((T, 24, NSITE), np.float32)
    for core in range(NCORES):
        out[core * TLOC:(core + 1) * TLOC] = simulate_core(link_vars, psi_vars, core * TLOC)
    res = (out[:, 0::2, :] + 1j * out[:, 1::2, :]).astype(np.complex64)
    return res.transpose(0, 2, 1).reshape(T, Z, Y, X, NS, NCOL)


# =================================================================== bass

LINK_KEYS = (
    [('c', d) for d in range(4)]
    + [('f', d, e) for (d, e) in
       [(0, 1), (0, 2), (0, 3), (2, 1), (3, 1), (3, 2), (1, 2), (1, 3), (2, 3)]]
    + [('b', d) for d in (1, 2, 3)]
)
PSI_KEYS = [('c',)] + [('s', e, sgn) for e in (1, 2, 3) for sgn in (1, -1)]


def _lname(key):
    return "u_" + "_".join(str(x) for x in key).replace('-', 'm')


def _pname(key):
    return "psi_" + "_".join(str(x) for x in key).replace('-', 'm')


def _bbuild_table():
    """Per chirality block: list of (plane, A, B(<=A), tgt_im, f9comp, coef)."""
    offd = {(0, 1): 0, (0, 2): 1, (1, 2): 2}
    tables = [[], []]
    for blk, sigs in enumerate((SIG_UP, SIG_DN)):
        for p in range(6):
            sig = sigs[p]
            for a in range(2):
                for b in range(2):
                    s = sig[a, b]
                    if abs(s) < 1e-12:
                        continue
                    cf = -1j * CCLOV * s
                    for i in range(3):
                        for j in range(3):
                            A_, B_ = a * 3 + i, b * 3 + j
                            if A_ < B_:
                                continue
                            if i == j:
                                fre = None
                                fim = (6 + i, 1.0)
                            elif (i, j) in offd:
                                q = offd[(i, j)]
                                fre = (2 * q, 1.0); fim = (2 * q + 1, 1.0)
                            else:
                                q = offd[(j, i)]
                                fre = (2 * q, -1.0); fim = (2 * q + 1, 1.0)
                            cr, ci = cf.real, cf.imag
                            for tgt_im, parts in ((0, [(fre, cr), (fim, -ci)]),
                                                  (1, [(fim, cr), (fre, ci)])):
                                if A_ == B_ and tgt_im:
                                    continue
                                for src, c0 in parts:
                                    if src is None or abs(c0) < 1e-15:
                                        continue
                                    comp, s0 = src
                                    tables[blk].append((p, A_, B_, tgt_im, comp, c0 * s0))
    # sanity: every lower-tri re comp and offdiag im comp gets >=1 write
    for blk in range(2):
        seen = {(A_, B_, t) for (_, A_, B_, t, _, _) in tables[blk]}
        for A_ in range(6):
            for B_ in range(A_ + 1):
                assert (A_, B_, 0) in seen, (blk, A_, B_)
                if A_ != B_:
                    assert (A_, B_, 1) in seen, (blk, A_, B_)
    return tables


BTABLES = _bbuild_table()


def _axis_pieces(d, L):
    """dst[i] = src[(i+d) % L] -> (dst_start, src_start, length) pieces."""
    d %= L
    if d == 0:
        return [(0, 0, L)]
    return [(0, d, L - d), (L - d, 0, d)]


def _build_device_program():
    import concourse.bacc as bacc
    import concourse.mybir as mybir
    from concourse import tile as ctile

    FP16, FP32 = mybir.dt.float16, mybir.dt.float32
    AL = mybir.AluOpType
    nc = bacc.Bacc(None, target_bir_lowering=False)

    u_in = {k: nc.declare_dram_parameter(_lname(k), [NWIN, P, 18, F], FP16, isOutput=False)
            for k in LINK_KEYS}
    p_in = {k: nc.declare_dram_parameter(_pname(k), [NWIN, P, 24, F], FP16, isOutput=False)
            for k in PSI_KEYS}
    out_dram = nc.declare_dram_parameter("out", [TLOC, P, 24, F], FP32, isOutput=True)

    dbg = {}
    if DEBUG_DUMP:
        dbg['g'] = nc.declare_dram_parameter("dbg_g", [6, NWIN, 9, NSITE], FP16, isOutput=True)
        dbg['ft'] = nc.declare_dram_parameter("dbg_ft", [6, P, 9, F], FP16, isOutput=True)
        dbg['bb'] = nc.declare_dram_parameter("dbg_bb", [2, P, 72, F], FP16, isOutput=True)
        dbg['ap'] = nc.declare_dram_parameter("dbg_ap", [P, 24, F], FP16, isOutput=True)
    gps = [[nc.dram_tensor(f"gp{p}_{w}", [9, NSITE], FP16) for w in range(NWIN)]
           for p in range(6)]
    # deduped shifted-G buffers keyed (plane, w_src, spatial shift)
    shuf_map = {}
    for p in range(6):
        for k, (dt, dz, dy, dx) in enumerate(DELTAS[p]):
            for o in range(TLOC):
                wsrc = o + 2 + dt
                key = (p, wsrc, dz, dy, dx)
                if key not in shuf_map:
                    shuf_map[key] = nc.dram_tensor(
                        f"gsh{p}_{wsrc}_{dz}_{dy}_{dx}".replace('-', 'm'),
                        [9, NSITE], FP16)

    def emit_cmatmul(pool, out_t, a_t, b_t, dag_b, eng=None, tp="", skip_diag_re=False):
        """out = A @ B(^+), 3x3 complex, per output column. With
        skip_diag_re, the real parts of out[k,k] are left unwritten
        (garbage) — valid when only the anti-hermitian part is consumed."""
        eng = eng if eng is not None else nc.vector
        P4 = {}
        for ra in (0, 1):
            for rb in (0, 1):
                P4[(ra, rb)] = pool.tile([P, 9, F], FP16, tag=f"mmP{ra}{rb}{tp}",
                                         name=f"mmP{ra}{rb}{tp}", bufs=1)
        Dre = pool.tile([P, 9, F], FP16, tag="mmDre" + tp, name="mmDre" + tp, bufs=1)
        Dim = pool.tile([P, 9, F], FP16, tag="mmDim" + tp, name="mmDim" + tp, bufs=1)
        av_all = a_t[:].rearrange("p (i j r) f -> p i j r f", i=3, j=3)
        bv_all = b_t[:].rearrange("p (j k r) f -> p j k r f", j=3, k=3)
        bv_dag = b_t[:].rearrange("p (k j r) f -> p k j r f", k=3, j=3)
        ov_all = out_t[:].rearrange("p (i k r) f -> p i k r f", i=3, k=3)
        for k in range(3):
            if skip_diag_re:
                isl = (slice(1, 3), slice(0, 3, 2), slice(0, 2))[k]
                ni = 2
            else:
                isl = slice(0, 3)
                ni = 3
            for (ra, rb), pt in P4.items():
                re_pair = (ra == rb)  # these two feed the real path only
                rows = isl if (re_pair and skip_diag_re) else slice(0, 3)
                nr = ni if (re_pair and skip_diag_re) else 3
                if dag_b:
                    bsel = bv_dag[:, k, :, rb, :]  # B[k,j]: [P, j(3), F]
                else:
                    bsel = bv_all[:, :, k, rb, :]  # B[j,k]: [P, j(3), F]
                bb = bsel.unsqueeze(1).broadcast_to([P, nr, 3, F])
                eng.tensor_mul(
                    pt[:].rearrange("p (i j) f -> p i j f", i=3)[:, 0:nr],
                    av_all[:, rows, :, ra, :], bb)
            nre = ni if skip_diag_re else 3
            if dag_b:
                eng.tensor_add(Dre[:, 0:3 * nre, :], P4[(0, 0)][:, 0:3 * nre, :],
                               P4[(1, 1)][:, 0:3 * nre, :])
                eng.tensor_sub(Dim[:], P4[(1, 0)][:], P4[(0, 1)][:])
            else:
                eng.tensor_sub(Dre[:, 0:3 * nre, :], P4[(0, 0)][:, 0:3 * nre, :],
                               P4[(1, 1)][:, 0:3 * nre, :])
                eng.tensor_add(Dim[:], P4[(0, 1)][:], P4[(1, 0)][:])
            for r, Dt in ((0, Dre), (1, Dim)):
                rows = isl if (r == 0 and skip_diag_re) else slice(0, 3)
                nr = nre if r == 0 else 3
                ov = ov_all[:, rows, k, r, :]  # [P, nr, F]
                Dv = Dt[:].rearrange("p (i j) f -> p i j f", i=3)
                eng.tensor_add(ov, Dv[:, 0:nr, 0, :], Dv[:, 0:nr, 1, :])
                eng.tensor_add(ov, ov, Dv[:, 0:nr, 2, :])

    def emit_cmatvec(pool, uh_t, u_t, h_t, dag):
        """uh[c,i] = sum_j Utilde[i,j] h[c,j]; h/uh: [P,12,F]; fused over c."""
        if dag:
            uv = u_t[:].rearrange("p (j i r) f -> p i j r f", j=3, i=3)
        else:
            uv = u_t[:].rearrange("p (i j r) f -> p i j r f", i=3, j=3)
        hv = h_t[:].rearrange("p (c cl r) f -> p c cl r f", c=2, cl=3)
        ov = uh_t[:].rearrange("p (c i r) f -> p c i r f", c=2, i=3)
        P4 = {}
        for ra in (0, 1):
            for rb in (0, 1):
                P4[(ra, rb)] = pool.tile([P, 18, F], FP16, tag=f"mvP{ra}{rb}",
                                         name=f"mvP{ra}{rb}", bufs=1)
        Dre = pool.tile([P, 18, F], FP16, tag="mvDre", name="mvDre", bufs=1)
        Dim = pool.tile([P, 18, F], FP16, tag="mvDim", name="mvDim", bufs=1)
        for c in range(2):
            for (ra, rb), pt in P4.items():
                hb = hv[:, c, :, rb, :].unsqueeze(1).broadcast_to([P, 3, 3, F])
                nc.vector.tensor_mul(
                    pt[:].rearrange("p (c2 i j) f -> p c2 i j f", c2=2, i=3)[:, c],
                    uv[:, :, :, ra, :], hb)
        if dag:
            # conj is on U (first factor): im = Ur*hi - Ui*hr
            nc.vector.tensor_add(Dre[:], P4[(0, 0)][:], P4[(1, 1)][:])
            nc.vector.tensor_sub(Dim[:], P4[(0, 1)][:], P4[(1, 0)][:])
        else:
            nc.vector.tensor_sub(Dre[:], P4[(0, 0)][:], P4[(1, 1)][:])
            nc.vector.tensor_add(Dim[:], P4[(0, 1)][:], P4[(1, 0)][:])
        ov2 = uh_t[:].rearrange("p (ci r) f -> p ci r f", ci=6)
        for r, Dt in ((0, Dre), (1, Dim)):
            o1 = ov2[:, :, r, :]  # [P, (c i)(6), F]
            Dv = Dt[:].rearrange("p (ci j) f -> p ci j f", ci=6)
            nc.vector.tensor_add(o1, Dv[:, :, 0, :], Dv[:, :, 1, :])
            nc.vector.tensor_add(o1, o1, Dv[:, :, 2, :])

    POOL_CMM = False
    _shuf_engs = [nc.gpsimd, nc.scalar]
    _shuf_idx = [0]

    def _next_shuf_eng():
        _shuf_idx[0] += 1
        return _shuf_engs[_shuf_idx[0] % len(_shuf_engs)]

    with ctile.TileContext(nc) as tc:
        # ---------------- phase 1: G build ----------------
        with tc.tile_pool(name="lnk", bufs=2) as lnk, \
             tc.tile_pool(name="gtmp", bufs=2) as gtmp, \
             tc.tile_pool(name="gout", bufs=2) as goutp:
            for w in range(6):
                cache = {}

                def load_link(key, wi, tag):
                    ck = (key, wi)
                    if ck not in cache:
                        t = lnk.tile([P, 18, F], FP16, tag=tag, name=tag)
                        nc.sync.dma_start(t[:], u_in[key][wi])
                        cache[ck] = t
                    return cache[ck]

                for p, (mu, nu) in enumerate(PAIRS):
                    if mu != 0 and w < 2:
                        continue
                    if mu == 0:
                        M1 = load_link(('c', 0), w, "m1_" + str(p))
                        M2 = load_link(('c', nu), w + 1, "m2_" + str(p))
                        M3 = load_link(('c', nu), w, "m3_" + str(p))
                        M4 = load_link(('f', 0, nu), w, "m4_" + str(p))
                    else:
                        M1 = load_link(('c', mu), w, "m1_" + str(p))
                        M2 = load_link(('f', nu, mu), w, "m2_" + str(p))
                        M3 = load_link(('c', nu), w, "m3_" + str(p))
                        M4 = load_link(('f', mu, nu), w, "m4_" + str(p))
                    # offload some units' independent A/B products to Pool
                    on_pool = ((2 * p + w) % 3 == 0) and POOL_CMM
                    At = gtmp.tile([P, 18, F], FP16, tag="A", name="A")
                    Bt = gtmp.tile([P, 18, F], FP16, tag="B", name="B")
                    Wt = gtmp.tile([P, 18, F], FP16, tag="W", name="W")
                    peng = nc.gpsimd if on_pool else nc.vector
                    ptp = "g" if on_pool else ""
                    emit_cmatmul(gtmp, At, M1, M2, dag_b=False, eng=peng, tp=ptp)
                    emit_cmatmul(gtmp, Bt, M3, M4, dag_b=False, eng=peng, tp=ptp)
                    emit_cmatmul(gtmp, Wt, At, Bt, dag_b=True, skip_diag_re=True)
                    Gt = goutp.tile([P, 9, F], FP16, tag="G", name="G")
                    # batched anti-hermitian assembly (pairs (0,1),(0,2),(1,2)):
                    # offd re: G[2q] = W[ij] - W[ji]; im: G[2q+1] = W[ij]+W[ji]
                    nc.vector.tensor_sub(Gt[:, 0:3:2, :], Wt[:, 2:5:2, :], Wt[:, 6:13:6, :])
                    nc.vector.tensor_sub(Gt[:, 4:5, :], Wt[:, 10:11, :], Wt[:, 14:15, :])
                    nc.vector.tensor_add(Gt[:, 1:4:2, :], Wt[:, 3:6:2, :], Wt[:, 7:14:6, :])
                    nc.vector.tensor_add(Gt[:, 5:6, :], Wt[:, 11:12, :], Wt[:, 15:16, :])
                    nc.vector.tensor_scalar_mul(Gt[:, 6:9, :], Wt[:, 1:18:8, :], 2.0)
                    nc.scalar.dma_start(
                        gps[p][w].rearrange("c (p2 f) -> p2 c f", p2=P), Gt[:])
                    if DEBUG_DUMP:
                        nc.sync.dma_start(
                            dbg['g'][p, w].rearrange("c (p2 f) -> p2 c f", p2=P), Gt[:])

                # deduped G shuffles whose source slice just became ready
                for (p, wsrc, dz, dy, dx), buf in shuf_map.items():
                    if wsrc != w:
                        continue
                    src = gps[p][w].rearrange("c (z y x) -> c z y x", z=Z, y=Y)
                    dst = buf.rearrange("c (z y x) -> c z y x", z=Z, y=Y)
                    qeng = _next_shuf_eng()
                    for (zd, zs, zl) in _axis_pieces(dz, Z):
                        for (yd, ys, yl) in _axis_pieces(dy, Y):
                            for (xd, xs, xl) in _axis_pieces(dx, X):
                                with nc.allow_non_contiguous_dma(reason="wrap"):
                                    qeng.dma_start(
                                        dst[:, zd:zd + zl, yd:yd + yl, xd:xd + xl],
                                        src[:, zs:zs + zl, ys:ys + yl, xs:xs + xl])

        # ---------------- phase 2: apply + hop ----------------
        with tc.tile_pool(name="gld", bufs=2) as gld, \
             tc.tile_pool(name="ftl", bufs=2) as ftl, \
             tc.tile_pool(name="bbl", bufs=2) as bbl, \
             tc.tile_pool(name="psl", bufs=2) as psl, \
             tc.tile_pool(name="uhp", bufs=2) as uhp, \
             tc.tile_pool(name="htm", bufs=2) as htm, \
             tc.tile_pool(name="oot", bufs=2) as oot:
            for o in range(TLOC):
                w = o + 2
                # F_tilde per plane
                ftil = []
                for p in range(6):
                    g0 = gld.tile([P, 9, F], FP16, tag="g0", name="g0")
                    nc.sync.dma_start(g0[:], gps[p][w].rearrange("c (p2 f) -> p2 c f", p2=P))
                    ft = ftl.tile([P, 9, F], FP16, tag=f"ft{p}", name=f"ft{p}")
                    first = True
                    for k in range(3):
                        dt, dz, dy, dx = DELTAS[p][k]
                        gbuf = shuf_map[(p, o + 2 + dt, dz, dy, dx)]
                        gk = gld.tile([P, 9, F], FP16, tag=f"g{k + 1}", name=f"g{k + 1}")
                        nc.sync.dma_start(gk[:], gbuf.rearrange("c (p2 f) -> p2 c f", p2=P))
                        if first:
                            nc.vector.tensor_sub(ft[:], g0[:], gk[:])
                            first = False
                        else:
                            nc.vector.tensor_sub(ft[:], ft[:], gk[:])
                    if DEBUG_DUMP and o == 0:
                        nc.sync.dma_start(dbg['ft'][p], ft[:])
                    ftil.append(ft)

                # B blocks: block-structured build.
                # B/c = [[M~, L~+],[L~, -M~]] (hermitian), from raw F-combos:
                #   M9 = F3 + s*F2 ; S9 = F4 - s*F1 ; T9 = s*F0 + F5  (s=+1 blk0, -1 blk1)
                # CCLOV scale applied via pre-scaled psi; DIAG handled post-apply.
                bts = [bbl.tile([P, 72, F], FP16, tag=f"B{blk}", name=f"B{blk}") for blk in range(2)]
                stt_t = [bbl.tile([P, 18, F], FP16, tag=f"ST{blk}", name=f"ST{blk}") for blk in range(2)]
                for blk in range(2):
                    bt = bts[blk]
                    bv = bt[:].rearrange("p (A B r) f -> p A B r f", A=6, B=6)
                    sv = bt[:].rearrange("p (A B r) f -> p B A r f", A=6, B=6)
                    St = stt_t[blk][:, 0:9, :]
                    Tt = stt_t[blk][:, 9:18, :]
                    Ft = [ftil[p] for p in range(6)]
                    if blk == 0:
                        nc.vector.tensor_sub(St, Ft[4][:], Ft[1][:])
                        nc.vector.tensor_add(Tt, Ft[0][:], Ft[5][:])
                    else:
                        nc.vector.tensor_add(St, Ft[4][:], Ft[1][:])
                        nc.vector.tensor_sub(Tt, Ft[5][:], Ft[0][:])

                    def madd(dst, ca, cb):  # dst = F3[ca] + s*F2[cb-slice]
                        if blk == 0:
                            nc.vector.tensor_add(dst, Ft[3][:, ca, :], Ft[2][:, cb, :])
                        else:
                            nc.vector.tensor_sub(dst, Ft[3][:, ca, :], Ft[2][:, cb, :])

                    def mneg(dst, ca, cb):  # dst = -(F3[ca] + s*F2[cb])
                        if blk == 0:
                            nc.vector.scalar_tensor_tensor(
                                dst, Ft[3][:, ca, :], -1.0, Ft[2][:, cb, :],
                                AL.mult, AL.subtract)
                        else:
                            nc.vector.tensor_sub(dst, Ft[2][:, cb, :], Ft[3][:, ca, :])

                    odd2, odd1 = slice(1, 5, 2), slice(5, 6)
                    ev2, ev1 = slice(0, 4, 2), slice(4, 5)
                    # UL quadrant: up.re / up.im
                    madd(bv[:, 0, 1:3, 0, :], odd2, odd2)
                    madd(bv[:, 1, 2:3, 0, :], odd1, odd1)
                    mneg(bv[:, 0, 1:3, 1, :], ev2, ev2)
                    mneg(bv[:, 1, 2:3, 1, :], ev1, ev1)
                    # UL lo.re / lo.im
                    madd(bv[:, 1, 0:1, 0, :], slice(1, 2), slice(1, 2))
                    madd(bv[:, 2, 0:2, 0, :], slice(3, 7, 2), slice(3, 7, 2))
                    madd(bv[:, 1, 0:1, 1, :], slice(0, 1), slice(0, 1))
                    madd(bv[:, 2, 0:2, 1, :], slice(2, 6, 2), slice(2, 6, 2))
                    # UL diag: re = M9[6+d]; im = 0
                    madd(bt[:, 0:29:14, :], slice(6, 9), slice(6, 9))
                    nc.vector.memzero(bt[:, 1:30:14, :])
                    # LL: up.re = S[2q]+T[2q+1] ; up.im = S[2q+1]-T[2q]
                    nc.vector.tensor_add(bv[:, 3, 1:3, 0, :], St[:, 0:4:2, :], Tt[:, 1:5:2, :])
                    nc.vector.tensor_add(bv[:, 4, 2:3, 0, :], St[:, 4:5, :], Tt[:, 5:6, :])
                    nc.vector.tensor_sub(bv[:, 3, 1:3, 1, :], St[:, 1:5:2, :], Tt[:, 0:4:2, :])
                    nc.vector.tensor_sub(bv[:, 4, 2:3, 1, :], St[:, 5:6, :], Tt[:, 4:5, :])
                    # LL lo.re = -S[2q]+T[2q+1] ; lo.im = S[2q+1]+T[2q]
                    nc.vector.tensor_sub(bv[:, 4, 0:1, 0, :], Tt[:, 1:2, :], St[:, 0:1, :])
                    nc.vector.tensor_sub(bv[:, 5, 0:2, 0, :], Tt[:, 3:7:2, :], St[:, 2:6:2, :])
                    nc.vector.tensor_add(bv[:, 4, 0:1, 1, :], St[:, 1:2, :], Tt[:, 0:1, :])
                    nc.vector.tensor_add(bv[:, 5, 0:2, 1, :], St[:, 3:7:2, :], Tt[:, 2:6:2, :])
                    # LL diag: re = T[6+d], im = S[6+d]  (comps 36/50/64, 37/51/65)
                    nc.vector.tensor_copy(bt[:, 36:65:14, :], Tt[:, 6:9, :])
                    nc.vector.tensor_copy(bt[:, 37:66:14, :], St[:, 6:9, :])
                    # LR = -UL  (flattened (B,r) view keeps the AP 3-D)
                    bv2 = bt[:].rearrange("p (A BR) f -> p A BR f", A=6)
                    nc.vector.tensor_scalar_mul(bv2[:, 3:6, 6:12, :], bv2[:, 0:3, 0:6, :], -1.0)
                    # UR = conj-transpose(LL)
                    nc.vector.tensor_copy(bv[:, 0:3, 3:6, 0, :], sv[:, 0:3, 3:6, 0, :])
                    nc.vector.tensor_scalar_mul(bv[:, 0:3, 3:6, 1, :], sv[:, 0:3, 3:6, 1, :], -1.0)

                # apply B to psi -> out tile (psi pre-scaled by CCLOV for the
                # F-part; the (4+m)+identity diagonal added afterwards via STT)
                psi_cr = psl.tile([P, 24, F], FP16, tag="pscr", name="pscr")
                nc.sync.dma_start(psi_cr[:], p_in[('c',)][w])
                psi_c = psl.tile([P, 24, F], FP16, tag="psc", name="psc")
                nc.vector.tensor_scalar_mul(psi_c[:], psi_cr[:], CCLOV)
                out_t = oot.tile([P, 24, F], FP16, tag="out", name="out")
                aptmp = htm.tile([P, 6, F], FP16, tag="aptmp", name="aptmp")
                aptm2 = htm.tile([P, 12, F], FP16, tag="aptm2", name="aptm2")
                for blk in range(2):
                    bt = bts[blk]
                    bv = bt[:].rearrange("p (a b r) f -> p a b r f", a=6, b=6)
                    ovv = out_t[:].rearrange("p (s r) f -> p s r f", r=2)
                    pvv = psi_c[:].rearrange("p (s r) f -> p s r f", r=2)
                    out_ri = out_t[:, blk * 12:(blk + 1) * 12, :]  # [P,12,F] (A,r)
                    out_re = ovv[:, blk * 6:(blk + 1) * 6, 0, :]
                    out_im = ovv[:, blk * 6:(blk + 1) * 6, 1, :]
                    for B_ in range(6):
                        sB = blk * 6 + B_
                        pr = pvv[:, sB:sB + 1, 0, :].broadcast_to([P, 6, F])
                        pi = pvv[:, sB:sB + 1, 1, :].broadcast_to([P, 6, F])
                        # psi (re,im) pair broadcast over A: [P, A(6), r(2), F]
                        pri = (psi_c[:, sB * 2:sB * 2 + 2, :]
                               .unsqueeze(1).broadcast_to([P, 6, 2, F]))
                        Brv = bv[:, :, B_, 0, :]
                        # Br broadcast over r: [P, A(6), r(2), F]
                        Brr = Brv.unsqueeze(2).broadcast_to([P, 6, 2, F])
                        Biv = bv[:, :, B_, 1, :]
                        ori = out_ri.rearrange("p (a r) f -> p a r f", a=6)
                        if B_ == 0:
                            nc.vector.tensor_mul(ori, Brr, pri)
                        else:
                            nc.vector.tensor_mul(
                                aptm2[:].rearrange("p (a r) f -> p a r f", a=6),
                                Brr, pri)
                            nc.vector.tensor_add(out_ri, out_ri, aptm2[:])
                        nc.vector.tensor_mul(aptmp[:], Biv, pi)
                        nc.vector.tensor_sub(out_re, out_re, aptmp[:])
                        nc.vector.tensor_mul(aptmp[:], Biv, pr)
                        nc.vector.tensor_add(out_im, out_im, aptmp[:])

                # diagonal (4+m)+identity term, on the unscaled psi
                nc.vector.scalar_tensor_tensor(
                    out_t[:], psi_cr[:], DIAG, out_t[:], AL.mult, AL.add)

                if DEBUG_DUMP and o == 0:
                    for blk in range(2):
                        nc.sync.dma_start(dbg['bb'][blk], bts[blk][:])
                    nc.sync.dma_start(dbg['ap'][:], out_t[:])

                # hop terms
                for mu in (range(4) if ENABLE_HOP else ()):
                    tbl = HOP[mu]
                    for sgn in (1, -1):
                        # psi source tile
                        psv = psl.tile([P, 24, F], FP16, tag="psv", name="psv")
                        if mu == 0:
                            nc.sync.dma_start(psv[:], p_in[('c',)][w + (1 if sgn > 0 else -1)])
                        else:
                            nc.sync.dma_start(psv[:], p_in[('s', mu, 1 if sgn > 0 else -1)][w])
                        # U tile
                        ut = uhp.tile([P, 18, F], FP16, tag="ut", name="ut")
                        if sgn > 0:
                            nc.sync.dma_start(ut[:], u_in[('c', mu)][w])
                        elif mu == 0:
                            nc.sync.dma_start(ut[:], u_in[('c', 0)][w - 1])
                        else:
                            nc.sync.dma_start(ut[:], u_in[('b', mu)][w])
                        # projection -> h [P,12,F]
                        h = htm.tile([P, 12, F], FP16, tag="h", name="h")
                        pvv = psv[:].rearrange("p (s r) f -> p s r f", r=2)
                        hvv = h[:].rearrange("p (s r) f -> p s r f", r=2)
                        for c in range(2):
                            cf = sgn * tbl['coef'][c]
                            b_ = tbl['b'][c]
                            if cf == 1:
                                nc.vector.tensor_add(h[:, c * 6:(c + 1) * 6, :],
                                                     psv[:, c * 6:(c + 1) * 6, :],
                                                     psv[:, b_ * 6:(b_ + 1) * 6, :])
                            elif cf == -1:
                                nc.vector.tensor_sub(h[:, c * 6:(c + 1) * 6, :],
                                                     psv[:, c * 6:(c + 1) * 6, :],
                                                     psv[:, b_ * 6:(b_ + 1) * 6, :])
                            else:
                                hre = hvv[:, c * 3:(c + 1) * 3, 0, :]
                                him = hvv[:, c * 3:(c + 1) * 3, 1, :]
                                pre = pvv[:, c * 3:(c + 1) * 3, 0, :]
                                pim = pvv[:, c * 3:(c + 1) * 3, 1, :]
                                qre = pvv[:, b_ * 3:(b_ + 1) * 3, 0, :]
                                qim = pvv[:, b_ * 3:(b_ + 1) * 3, 1, :]
                                if cf == 1j:
                                    nc.vector.tensor_sub(hre, pre, qim)
                                    nc.vector.tensor_add(him, pim, qre)
                                else:  # -1j
                                    nc.vector.tensor_add(hre, pre, qim)
                                    nc.vector.tensor_sub(him, pim, qre)
                        # color mult
                        uh = htm.tile([P, 12, F], FP16, tag="uh", name="uh")
                        emit_cmatvec(uhp, uh, ut, h, dag=(sgn < 0))
                        # accumulate into out (rows 0,1 in one op)
                        sl = out_t[:, 0:12, :]
                        nc.vector.scalar_tensor_tensor(
                            sl, uh[:, 0:12, :], -0.5, sl, AL.mult, AL.add)
                        uvv = uh[:].rearrange("p (s r) f -> p s r f", r=2)
                        ovv = out_t[:].rearrange("p (s r) f -> p s r f", r=2)
                        rcs = [sgn * tbl['rc'][cp] for cp in range(2)]
                        if rcs[0] == rcs[1] and tbl['m'] == (0, 1) and rcs[0] in (1, -1):
                            sl = out_t[:, 12:24, :]
                            nc.vector.scalar_tensor_tensor(
                                sl, uh[:, 0:12, :], -0.5 * rcs[0], sl,
                                AL.mult, AL.add)
                            continue
                        for cp in range(2):
                            rc = rcs[cp]
                            mm = tbl['m'][cp]
                            row = 2 + cp
                            if rc in (1, -1):
                                sl = out_t[:, row * 6:(row + 1) * 6, :]
                                nc.vector.scalar_tensor_tensor(
                                    sl, uh[:, mm * 6:(mm + 1) * 6, :], -0.5 * rc, sl,
                                    AL.mult, AL.add)
                            else:
                                s_i = rc.imag
                                o_re = ovv[:, row * 3:(row + 1) * 3, 0, :]
                                o_im = ovv[:, row * 3:(row + 1) * 3, 1, :]
                                u_re = uvv[:, mm * 3:(mm + 1) * 3, 0, :]
                                u_im = uvv[:, mm * 3:(mm + 1) * 3, 1, :]
                                nc.vector.scalar_tensor_tensor(
                                    o_re, u_im, 0.5 * s_i, o_re, AL.mult, AL.add)
                                nc.vector.scalar_tensor_tensor(
                                    o_im, u_re, -0.5 * s_i, o_im, AL.mult, AL.add)

                # store (fp16 -> fp32 cast via SWDGE)
                nc.gpsimd.dma_start(out_dram[o], out_t[:])

    nc.finalize()
    return nc


_PROG_CACHE = {}


def _get_program():
    if 'nc' not in _PROG_CACHE:
        _PROG_CACHE['nc'] = _build_device_program()
    return _PROG_CACHE['nc']


def _sbuf_image(a, C):
    """[T, C, NSITE] -> [T, P, C, F] contiguous."""
    return np.ascontiguousarray(a.reshape(T, C, P, F).transpose(0, 2, 1, 3))


def build_in_maps(psi, U):
    link_vars = _to_planar_links(U)
    psi_vars = _to_planar_psi(psi)
    link_imgs = {k: _sbuf_image(v, 18) for k, v in link_vars.items()}
    psi_imgs = {k: _sbuf_image(v, 24) for k, v in psi_vars.items()}
    in_maps = []
    for core in range(NCORES):
        t0 = core * TLOC
        tw = [(t0 - 2 + w) % T for w in range(NWIN)]
        m = {}
        for k in LINK_KEYS:
            m[_lname(k)] = np.ascontiguousarray(link_imgs[k][tw])
        for k in PSI_KEYS:
            m[_pname(k)] = np.ascontiguousarray(psi_imgs[k][tw])
        in_maps.append(m)
    return in_maps


def assemble_output(results):
    out = np.empty((T, 24, NSITE), np.float32)
    for core in range(NCORES):
        r = results[core]['out']  # [TLOC, P, 24, F] fp32
        out[core * TLOC:(core + 1) * TLOC] = r.transpose(0, 2, 1, 3).reshape(TLOC, 24, NSITE)
    res = (out[:, 0::2, :] + 1j * out[:, 1::2, :]).astype(np.complex64)
    return res.transpose(0, 2, 1).reshape(T, Z, Y, X, NS, NCOL)


def kernel(psi, U):
    psi = np.asarray(psi)
    U = np.asarray(U)
    from concourse.bass_utils import run_bass_kernel_spmd
    nc = _get_program()
    in_maps = build_in_maps(psi, U)
    res = run_bass_kernel_spmd(nc, in_maps, core_ids=list(range(NCORES)))
    return assemble_output(res.results)



# revision 24
# speedup vs baseline: 1.5547x; 1.1791x over previous
"""Clover-Wilson Dirac operator on Trainium2 (8 NeuronCores, T-sharded).

Math summary (derived + numerically verified against the reference):
- The reference's 4-leaf "clover" Q for plane (mu,nu) factorizes as
      Q(x) = W(x) + W(x+d1)^+ + W(x+d2)^+ + W(x+d3)^+
  with W(x) = [U_mu(x) U_nu(x+mu)] [U_nu(x) U_mu(x+nu)]^+,
  d1 = nu-mu, d2 = -2mu-2nu, d3 = -2nu (unit lattice vectors).
- With G = W - W^+ (anti-Hermitian), Ftil := Q - Q^+ = G(x) - G(x+d1) - G(x+d2) - G(x+d3).
- C psi + (4+m) psi = (5+m) psi + (csw/32) * sum_p (sigma_p (x) (-i Ftil_p)) psi,
  where sigma_p is block-diagonal (2x2 chiral blocks) in this basis.
- Wilson hop uses the standard spin-projection trick (2 half-spinors per direction).

Distribution: T=32 sharded 4 slices per core; U needs halo t0-2..t0+4 (7 slices),
psi needs t0-1..t0+4. All jnp.roll shifts are pushed into host-precomputed
pre-rolled planar fp16 arrays; on-device shifted reads of the intermediate G
use DRAM->DRAM affine shuffle DMAs.
"""
import numpy as np

T, Z, Y, X = 32, 24, 24, 24
NCOL, NS = 3, 4
MASS, CSW = 0.1, 1.0
PAIRS = [(0, 1), (0, 2), (0, 3), (1, 2), (1, 3), (2, 3)]
NCORES = 8
TLOC = T // NCORES          # 4 output slices per core
NSITE = Z * Y * X           # 13824
P = 128
F = NSITE // P              # 108
NWIN = 7                    # U window slices: t0-2 .. t0+4
DIAG = 5.0 + MASS           # (4+m) + clover identity
CCLOV = CSW / 32.0          # |coefficient| of sigma (x) Ftil; overall factor -i


# ----------------------------------------------------------------- tables

def _gammas():
    i = 1j
    g0 = np.array([[0, 0, 1, 0], [0, 0, 0, 1], [1, 0, 0, 0], [0, 1, 0, 0]], np.complex128)
    g1 = np.array([[0, 0, 0, i], [0, 0, i, 0], [0, -i, 0, 0], [-i, 0, 0, 0]], np.complex128)
    g2 = np.array([[0, 0, 0, -1], [0, 0, 1, 0], [0, 1, 0, 0], [-1, 0, 0, 0]], np.complex128)
    g3 = np.array([[0, 0, i, 0], [0, 0, 0, -i], [-i, 0, 0, 0], [0, i, 0, 0]], np.complex128)
    return [g0, g1, g2, g3]


def _sigma_blocks():
    """Chiral 2x2 blocks of sigma_{mu nu} = i g_mu g_nu for each plane."""
    G = _gammas()
    ups, dns = [], []
    for mu, nu in PAIRS:
        s = 1j * (G[mu] @ G[nu])
        assert np.abs(s[:2, 2:]).max() < 1e-12 and np.abs(s[2:, :2]).max() < 1e-12
        ups.append(s[:2, :2].copy())
        dns.append(s[2:, 2:].copy())
    return ups, dns


SIG_UP, SIG_DN = _sigma_blocks()

# per-plane shift deltas (t, z, y, x) for the W-factorization
def _deltas():
    out = []
    for mu, nu in PAIRS:
        e_mu = np.zeros(4, np.int64); e_mu[mu] = 1
        e_nu = np.zeros(4, np.int64); e_nu[nu] = 1
        out.append([tuple(e_nu - e_mu), tuple(-2 * e_mu - 2 * e_nu), tuple(-2 * e_nu)])
    return out


DELTAS = _deltas()

# debug toggles (affect both simulate_core and the device program)
ENABLE_CLOVER = True
ENABLE_HOP = True
DEBUG_DUMP = False

# hop projection tables: psi_h[c] = psi[c] + coef * psi[b[c]]; lower rows:
# row_{2+c} = rc[c] * h[m[c]]  (forward, i.e. (1-gamma)); backward negates
# coef and rc. Verified against gammas in _check_hop_tables().
HOP = {
    0: dict(b=(2, 3), coef=(-1, -1), m=(0, 1), rc=(-1, -1)),
    1: dict(b=(3, 2), coef=(-1j, -1j), m=(1, 0), rc=(1j, 1j)),
    2: dict(b=(3, 2), coef=(1, -1), m=(1, 0), rc=(-1, 1)),
    3: dict(b=(2, 3), coef=(-1j, 1j), m=(0, 1), rc=(1j, -1j)),
}


def _check_hop_tables():
    G = _gammas()
    for mu, t in HOP.items():
        for sgn in (+1, -1):  # +1: (1-g) fwd ; -1: (1+g) bwd
            M = np.eye(4) - sgn * G[mu]
            # build from table
            B = np.zeros((4, 4), np.complex128)
            for c in range(2):
                B[c, c] += 1
                B[c, t['b'][c]] += sgn * t['coef'][c]
            for c in range(2):
                rc = sgn * t['rc'][c]
                B[2 + c, t['m'][c]] += rc
                B[2 + c, t['b'][t['m'][c]]] += rc * sgn * t['coef'][t['m'][c]]
            assert np.abs(B - M).max() < 1e-12, (mu, sgn, B, M)


_check_hop_tables()


# ------------------------------------------------- planar layout helpers

def _to_planar_links(U):
    """U: (T,Z,Y,X,4,3,3) complex64 -> dict of fp16 planar arrays.

    Returns variants[key] = array [T, 18, NSITE] fp16 with comp c=(i*3+j)*2+r.
    Keys: ('c', d) centered; ('f', d, e) = U_d(x+e_hat) spatial e;
          ('b', d) = U_d(x - d_hat) spatial d.
    """
    Uf32 = np.ascontiguousarray(U)  # complex64
    planar = np.empty((4, T, 18, NSITE), np.float16)
    Um = Uf32.reshape(T, NSITE, 4, 9)
    for d in range(4):
        re = Um[..., d, :].real.astype(np.float16)  # (T, NSITE, 9)
        im = Um[..., d, :].imag.astype(np.float16)
        planar[d, :, 0::2, :] = re.transpose(0, 2, 1)
        planar[d, :, 1::2, :] = im.transpose(0, 2, 1)

    def roll_sites(arr, delta):  # arr [..., NSITE]; value at x+delta
        dz, dy, dx = delta
        a = arr.reshape(*arr.shape[:-1], Z, Y, X)
        if dz: a = np.roll(a, -dz, axis=-3)
        if dy: a = np.roll(a, -dy, axis=-2)
        if dx: a = np.roll(a, -dx, axis=-1)
        return a.reshape(*arr.shape[:-1], NSITE)

    variants = {}
    for d in range(4):
        variants[('c', d)] = planar[d]
    needed_f = {(0, 1), (0, 2), (0, 3), (2, 1), (3, 1), (3, 2), (1, 2), (1, 3), (2, 3)}
    for (d, e) in needed_f:
        delta = [0, 0, 0]; delta[e - 1] = 1
        variants[('f', d, e)] = roll_sites(planar[d], delta)
    for d in (1, 2, 3):
        delta = [0, 0, 0]; delta[d - 1] = -1
        variants[('b', d)] = roll_sites(planar[d], delta)
    return variants


def _to_planar_psi(psi):
    """psi: (T,Z,Y,X,4,3) complex64 -> dict: ('c',) -> [T, 24, NSITE] fp16
    (comp c=(s*3+cl)*2+r) and pre-projected half-spinors ('h', mu, sgn) ->
    [T, 12, NSITE] fp16 (spatially pre-rolled for mu != 0)."""
    pm = psi.reshape(T, NSITE, 12)
    planar = np.empty((T, 24, NSITE), np.float16)
    planar[:, 0::2, :] = pm.real.astype(np.float16).transpose(0, 2, 1)
    planar[:, 1::2, :] = pm.imag.astype(np.float16).transpose(0, 2, 1)

    def roll_sites(arr, delta):
        dz, dy, dx = delta
        a = arr.reshape(*arr.shape[:-1], Z, Y, X)
        if dz: a = np.roll(a, -dz, axis=-3)
        if dy: a = np.roll(a, -dy, axis=-2)
        if dx: a = np.roll(a, -dx, axis=-1)
        return a.reshape(*arr.shape[:-1], NSITE)

    out = {('c',): planar}
    for mu, tbl in HOP.items():
        for sgn in (1, -1):
            h = np.empty((T, 12, NSITE), np.float16)
            for c in range(2):
                cf = sgn * tbl['coef'][c]
                b_ = tbl['b'][c]
                for cl in range(3):
                    pr = planar[:, (c * 3 + cl) * 2]
                    pi = planar[:, (c * 3 + cl) * 2 + 1]
                    qr = planar[:, (b_ * 3 + cl) * 2]
                    qi = planar[:, (b_ * 3 + cl) * 2 + 1]
                    if cf == 1:
                        hr, hi = pr + qr, pi + qi
                    elif cf == -1:
                        hr, hi = pr - qr, pi - qi
                    elif cf == 1j:
                        hr, hi = pr - qi, pi + qr
                    else:
                        hr, hi = pr + qi, pi - qr
                    h[:, (c * 3 + cl) * 2] = hr
                    h[:, (c * 3 + cl) * 2 + 1] = hi
            if mu != 0:
                delta = [0, 0, 0]
                delta[mu - 1] = 1 if sgn > 0 else -1
                h = roll_sites(h, delta)
            out[('h', mu, sgn)] = h
    return out


# ------------------------------------------------------ numpy simulator
# Step-wise fp16 mirror of the device dataflow (for validation).

def _cmm16(A, B, dag_b=False):
    """A,B: [18, N] fp16 planar 3x3 complex; returns C = A @ B(^+) fp16."""
    C = np.zeros_like(A)
    for i in range(3):
        for k in range(3):
            cre = np.zeros(A.shape[-1], np.float16)
            cim = np.zeros(A.shape[-1], np.float16)
            for j in range(3):
                ar = A[(i * 3 + j) * 2]; ai = A[(i * 3 + j) * 2 + 1]
                if dag_b:
                    br = B[(k * 3 + j) * 2]; bi = -B[(k * 3 + j) * 2 + 1].astype(np.float16)
                else:
                    br = B[(j * 3 + k) * 2]; bi = B[(j * 3 + k) * 2 + 1]
                cre = (cre + (ar * br - ai * bi)).astype(np.float16)
                cim = (cim + (ar * bi + ai * br)).astype(np.float16)
            C[(i * 3 + k) * 2] = cre
            C[(i * 3 + k) * 2 + 1] = cim
    return C


def _antiherm9(Wm):
    """W planar 18 -> G = W - W^+ in 9-comp layout:
    q*2 / q*2+1 = re/im of G[i,j] for (i,j) in [(0,1),(0,2),(1,2)]; 6+d = im G[d,d]."""
    G = np.empty((9, Wm.shape[-1]), np.float16)
    offd = [(0, 1), (0, 2), (1, 2)]
    for q, (i, j) in enumerate(offd):
        G[q * 2] = (Wm[(i * 3 + j) * 2] - Wm[(j * 3 + i) * 2]).astype(np.float16)
        G[q * 2 + 1] = (Wm[(i * 3 + j) * 2 + 1] + Wm[(j * 3 + i) * 2 + 1]).astype(np.float16)
    for d in range(3):
        G[6 + d] = (Wm[(d * 3 + d) * 2 + 1] * np.float16(2.0)).astype(np.float16)
    return G


def _f9_entry(F9, i, j):
    """(re, im) pair (arrays or (None, arr)) of Ftil[i,j] from 9-comp planar."""
    offd = {(0, 1): 0, (0, 2): 1, (1, 2): 2}
    if i == j:
        return None, F9[6 + i]
    if (i, j) in offd:
        q = offd[(i, j)]
        return F9[q * 2], F9[q * 2 + 1]
    q = offd[(j, i)]
    return -F9[q * 2], F9[q * 2 + 1]  # G[i>j] = -conj(G[j,i]) -> (-re, +im)


def _roll_sites_np(a, delta):
    dz, dy, dx = delta
    a = a.reshape(*a.shape[:-1], Z, Y, X)
    if dz: a = np.roll(a, -dz, axis=-3)
    if dy: a = np.roll(a, -dy, axis=-2)
    if dx: a = np.roll(a, -dx, axis=-1)
    return a.reshape(*a.shape[:-2], -1) if False else a.reshape(*a.shape[:-4], a.shape[-4] if a.ndim > 3 else -1, NSITE) if False else a.reshape(-1, NSITE) if a.ndim == 4 else a.reshape(NSITE)


def simulate_core(link_vars, psi_vars, t0):
    """Numpy fp16 mirror. link_vars/psi_vars: full-T variant dicts.
    Returns planar out [TLOC, 24, NSITE] float32."""
    tw = [(t0 - 2 + w) % T for w in range(NWIN)]

    def LV(key, w):
        return link_vars[key][tw[w]]

    def PV(key, w):
        return psi_vars[key][tw[w]]

    # ---- phase 1: G per plane per window slice
    Gs = {}
    for p, (mu, nu) in enumerate(PAIRS):
        ws = range(0, 6) if mu == 0 else range(2, 6)
        for w in ws:
            if mu == 0:
                M1, M2 = LV(('c', 0), w), LV(('c', nu), w + 1)
                M3, M4 = LV(('c', nu), w), LV(('f', 0, nu), w)
            else:
                M1, M2 = LV(('c', mu), w), LV(('f', nu, mu), w)
                M3, M4 = LV(('c', nu), w), LV(('f', mu, nu), w)
            A = _cmm16(M1, M2)
            B = _cmm16(M3, M4)
            Wm = _cmm16(A, B, dag_b=True)
            Gs[(p, w)] = _antiherm9(Wm)

    out = np.zeros((TLOC, 24, NSITE), np.float32)
    for o in range(TLOC):
        w = o + 2
        # ---- Ftil per plane
        F9s = []
        for p in range(6):
            acc = Gs[(p, w)].copy()
            for (dt, dz, dy, dx) in DELTAS[p]:
                g = Gs[(p, w + dt)]
                gsh = g.reshape(9, Z, Y, X)
                if dz: gsh = np.roll(gsh, -dz, axis=1)
                if dy: gsh = np.roll(gsh, -dy, axis=2)
                if dx: gsh = np.roll(gsh, -dx, axis=3)
                acc = (acc - gsh.reshape(9, NSITE)).astype(np.float16)
            F9s.append(acc)

        if not ENABLE_CLOVER:
            F9s = [np.zeros((9, NSITE), np.float16) for _ in range(6)]
        # ---- B blocks (full 6x6 complex per chirality block), fp16
        Bblk = [np.zeros((6, 6, 2, NSITE), np.float16) for _ in range(2)]
        for blk, sigs in enumerate((SIG_UP, SIG_DN)):
            for p in range(6):
                sig = sigs[p]
                for a in range(2):
                    for b in range(2):
                        s = sig[a, b]
                        if abs(s) < 1e-12:
                            continue
                        cf = -1j * CCLOV * s  # complex coefficient
                        for i in range(3):
                            for j in range(3):
                                fre, fim = _f9_entry(F9s[p], i, j)
                                A_, B_ = a * 3 + i, b * 3 + j
                                # coeff*(fre + i fim): accumulate re and im
                                cr, ci = cf.real, cf.imag
                                tgt = Bblk[blk][A_, B_]
                                if fre is not None:
                                    if cr: tgt[0] = (tgt[0] + np.float16(cr) * fre).astype(np.float16)
                                    if ci: tgt[1] = (tgt[1] + np.float16(ci) * fre).astype(np.float16)
                                if cr: tgt[1] = (tgt[1] + np.float16(cr) * fim).astype(np.float16)
                                if ci: tgt[0] = (tgt[0] - np.float16(ci) * fim).astype(np.float16)
            for A_ in range(6):
                Bblk[blk][A_, A_, 0] = (Bblk[blk][A_, A_, 0] + np.float16(DIAG)).astype(np.float16)

        # ---- apply B to psi
        psi_c = PV(('c',), w)
        for blk in range(2):
            for A_ in range(6):
                s_out = (blk * 2 + A_ // 3) * 3 + (A_ % 3)  # spinor comp index s*3+cl
                accr = np.zeros(NSITE, np.float16)
                acci = np.zeros(NSITE, np.float16)
                for B_ in range(6):
                    s_in = (blk * 2 + B_ // 3) * 3 + (B_ % 3)
                    pr = psi_c[s_in * 2]; pi = psi_c[s_in * 2 + 1]
                    br = Bblk[blk][A_, B_, 0]; bi = Bblk[blk][A_, B_, 1]
                    accr = (accr + br * pr - bi * pi).astype(np.float16)
                    acci = (acci + br * pi + bi * pr).astype(np.float16)
                out[o, s_out * 2] += accr.astype(np.float32)
                out[o, s_out * 2 + 1] += acci.astype(np.float32)

        # ---- hop terms
        for mu in (range(4) if ENABLE_HOP else ()):
            tbl = HOP[mu]
            for sgn, wpsi_key, woff, ukey, udag in (
                (+1, 'f', +1, ('c', mu), False),
                (-1, 'b', -1, ('b', mu) if mu else ('c', 0), True),
            ):
                if mu == 0:
                    psv = PV(('c',), w + woff)
                else:
                    psv = PV(('s', mu, +1 if sgn > 0 else -1), w)
                uar = LV(ukey, w) if mu else LV(ukey, w + (0 if sgn > 0 else -1))
                # project: h[c] = psi[c] + sgn*coef[c]*psi[b[c]] (2 spins x 3 col)
                h = np.zeros((2, 3, 2, NSITE), np.float16)
                for c in range(2):
                    cf = sgn * tbl['coef'][c]
                    for cl in range(3):
                        pr = psv[(c * 3 + cl) * 2]; pi = psv[(c * 3 + cl) * 2 + 1]
                        qr = psv[(tbl['b'][c] * 3 + cl) * 2]; qi = psv[(tbl['b'][c] * 3 + cl) * 2 + 1]
                        if cf == 1:
                            h[c, cl, 0] = (pr + qr).astype(np.float16); h[c, cl, 1] = (pi + qi).astype(np.float16)
                        elif cf == -1:
                            h[c, cl, 0] = (pr - qr).astype(np.float16); h[c, cl, 1] = (pi - qi).astype(np.float16)
                        elif cf == 1j:
                            h[c, cl, 0] = (pr - qi).astype(np.float16); h[c, cl, 1] = (pi + qr).astype(np.float16)
                        else:  # -1j
                            h[c, cl, 0] = (pr + qi).astype(np.float16); h[c, cl, 1] = (pi - qr).astype(np.float16)
                # color mult: uh[c, i] = sum_j U[i,j] h[c, j] (or U^+ )
                uh = np.zeros((2, 3, 2, NSITE), np.float16)
                for c in range(2):
                    for i in range(3):
                        ar = np.zeros(NSITE, np.float16); ai = np.zeros(NSITE, np.float16)
                        for j in range(3):
                            if udag:
                                ur = uar[(j * 3 + i) * 2]; ui = -uar[(j * 3 + i) * 2 + 1].astype(np.float16)
                            else:
                                ur = uar[(i * 3 + j) * 2]; ui = uar[(i * 3 + j) * 2 + 1]
                            ar = (ar + ur * h[c, j, 0] - ui * h[c, j, 1]).astype(np.float16)
                            ai = (ai + ur * h[c, j, 1] + ui * h[c, j, 0]).astype(np.float16)
                        uh[c, i, 0] = ar; uh[c, i, 1] = ai
                # accumulate: rows 0,1: -1/2*uh[c]; rows 2+c': -1/2*sgn... rc
                for c in range(2):
                    for cl in range(3):
                        out[o, (c * 3 + cl) * 2] -= 0.5 * uh[c, cl, 0].astype(np.float32)
                        out[o, (c * 3 + cl) * 2 + 1] -= 0.5 * uh[c, cl, 1].astype(np.float32)
                for cp in range(2):
                    rc = sgn * tbl['rc'][cp]
                    mm = tbl['m'][cp]
                    for cl in range(3):
                        tr = uh[mm, cl, 0].astype(np.float32); ti = uh[mm, cl, 1].astype(np.float32)
                        if rc == 1:
                            out[o, ((2 + cp) * 3 + cl) * 2] -= 0.5 * tr
                            out[o, ((2 + cp) * 3 + cl) * 2 + 1] -= 0.5 * ti
                        elif rc == -1:
                            out[o, ((2 + cp) * 3 + cl) * 2] += 0.5 * tr
                            out[o, ((2 + cp) * 3 + cl) * 2 + 1] += 0.5 * ti
                        elif rc == 1j:
                            out[o, ((2 + cp) * 3 + cl) * 2] += 0.5 * ti
                            out[o, ((2 + cp) * 3 + cl) * 2 + 1] -= 0.5 * tr
                        else:  # -1j
                            out[o, ((2 + cp) * 3 + cl) * 2] -= 0.5 * ti
                            out[o, ((2 + cp) * 3 + cl) * 2 + 1] += 0.5 * tr
    return out


def simulate(psi, U):
    """Full-lattice numpy fp16 simulation -> complex64 (T,Z,Y,X,4,3)."""
    link_vars = _to_planar_links(U)
    psi_vars = _to_planar_psi(psi)
    out = np.zeros((T, 24, NSITE), np.float32)
    for core in range(NCORES):
        out[core * TLOC:(core + 1) * TLOC] = simulate_core(link_vars, psi_vars, core * TLOC)
    res = (out[:, 0::2, :] + 1j * out[:, 1::2, :]).astype(np.complex64)
    return res.transpose(0, 2, 1).reshape(T, Z, Y, X, NS, NCOL)


# =================================================================== bass

LINK_KEYS = (
    [('c', d) for d in range(4)]
    + [('f', d, e) for (d, e) in
       [(0, 1), (0, 2), (0, 3), (2, 1), (3, 1), (3, 2), (1, 2), (1, 3), (2, 3)]]
    + [('b', d) for d in (1, 2, 3)]
)
PSI_KEYS = [('c',)] + [('h', mu, sgn) for mu in range(4) for sgn in (1, -1)]


def _lname(key):
    return "u_" + "_".join(str(x) for x in key).replace('-', 'm')


def _pname(key):
    return "psi_" + "_".join(str(x) for x in key).replace('-', 'm')


def _bbuild_table():
    """Per chirality block: list of (plane, A, B(<=A), tgt_im, f9comp, coef)."""
    offd = {(0, 1): 0, (0, 2): 1, (1, 2): 2}
    tables = [[], []]
    for blk, sigs in enumerate((SIG_UP, SIG_DN)):
        for p in range(6):
            sig = sigs[p]
            for a in range(2):
                for b in range(2):
                    s = sig[a, b]
                    if abs(s) < 1e-12:
                        continue
                    cf = -1j * CCLOV * s
                    for i in range(3):
                        for j in range(3):
                            A_, B_ = a * 3 + i, b * 3 + j
                            if A_ < B_:
                                continue
                            if i == j:
                                fre = None
                                fim = (6 + i, 1.0)
                            elif (i, j) in offd:
                                q = offd[(i, j)]
                                fre = (2 * q, 1.0); fim = (2 * q + 1, 1.0)
                            else:
                                q = offd[(j, i)]
                                fre = (2 * q, -1.0); fim = (2 * q + 1, 1.0)
                            cr, ci = cf.real, cf.imag
                            for tgt_im, parts in ((0, [(fre, cr), (fim, -ci)]),
                                                  (1, [(fim, cr), (fre, ci)])):
                                if A_ == B_ and tgt_im:
                                    continue
                                for src, c0 in parts:
                                    if src is None or abs(c0) < 1e-15:
                                        continue
                                    comp, s0 = src
                                    tables[blk].append((p, A_, B_, tgt_im, comp, c0 * s0))
    # sanity: every lower-tri re comp and offdiag im comp gets >=1 write
    for blk in range(2):
        seen = {(A_, B_, t) for (_, A_, B_, t, _, _) in tables[blk]}
        for A_ in range(6):
            for B_ in range(A_ + 1):
                assert (A_, B_, 0) in seen, (blk, A_, B_)
                if A_ != B_:
                    assert (A_, B_, 1) in seen, (blk, A_, B_)
    return tables


BTABLES = _bbuild_table()


def _axis_pieces(d, L):
    """dst[i] = src[(i+d) % L] -> (dst_start, src_start, length) pieces."""
    d %= L
    if d == 0:
        return [(0, 0, L)]
    return [(0, d, L - d), (L - d, 0, d)]


def _build_device_program():
    import concourse.bacc as bacc
    import concourse.mybir as mybir
    from concourse import tile as ctile

    FP16, FP32 = mybir.dt.float16, mybir.dt.float32
    AL = mybir.AluOpType
    nc = bacc.Bacc(None, target_bir_lowering=False)

    u_in = {k: nc.declare_dram_parameter(_lname(k), [NWIN, P, 18, F], FP16, isOutput=False)
            for k in LINK_KEYS}
    p_in = {k: nc.declare_dram_parameter(
                _pname(k), [NWIN, P, 24 if k == ('c',) else 12, F], FP16,
                isOutput=False)
            for k in PSI_KEYS}
    out_dram = nc.declare_dram_parameter("out", [TLOC, P, 24, F], FP32, isOutput=True)

    dbg = {}
    if DEBUG_DUMP:
        dbg['g'] = nc.declare_dram_parameter("dbg_g", [6, NWIN, 9, NSITE], FP16, isOutput=True)
        dbg['ft'] = nc.declare_dram_parameter("dbg_ft", [6, P, 9, F], FP16, isOutput=True)
        dbg['bb'] = nc.declare_dram_parameter("dbg_bb", [2, P, 72, F], FP16, isOutput=True)
        dbg['ap'] = nc.declare_dram_parameter("dbg_ap", [P, 24, F], FP16, isOutput=True)
    gps = [[nc.dram_tensor(f"gp{p}_{w}", [9, NSITE], FP16) for w in range(NWIN)]
           for p in range(6)]
    # deduped shifted-G buffers keyed (plane, w_src, spatial shift)
    shuf_map = {}
    for p in range(6):
        for k, (dt, dz, dy, dx) in enumerate(DELTAS[p]):
            for o in range(TLOC):
                wsrc = o + 2 + dt
                key = (p, wsrc, dz, dy, dx)
                if key not in shuf_map:
                    shuf_map[key] = nc.dram_tensor(
                        f"gsh{p}_{wsrc}_{dz}_{dy}_{dx}".replace('-', 'm'),
                        [9, NSITE], FP16)

    def emit_cmatmul(pool, out_t, a_t, b_t, dag_b, eng=None, tp="", skip_diag_re=False):
        """out = A @ B(^+), 3x3 complex, per output column. With
        skip_diag_re, the real parts of out[k,k] are left unwritten
        (garbage) — valid when only the anti-hermitian part is consumed."""
        eng = eng if eng is not None else nc.vector
        P4 = {}
        for ra in (0, 1):
            for rb in (0, 1):
                P4[(ra, rb)] = pool.tile([P, 9, F], FP16, tag=f"mmP{ra}{rb}{tp}",
                                         name=f"mmP{ra}{rb}{tp}", bufs=1)
        av_all = a_t[:].rearrange("p (i j r) f -> p i j r f", i=3, j=3)
        bv_all = b_t[:].rearrange("p (j k r) f -> p j k r f", j=3, k=3)
        bv_dag = b_t[:].rearrange("p (k j r) f -> p k j r f", k=3, j=3)
        ov_all = out_t[:].rearrange("p (i k r) f -> p i k r f", i=3, k=3)
        if skip_diag_re:
            Dre = pool.tile([P, 9, F], FP16, tag="mmDre" + tp, name="mmDre" + tp, bufs=1)
            Dim = pool.tile([P, 9, F], FP16, tag="mmDim" + tp, name="mmDim" + tp, bufs=1)
        else:
            D2 = pool.tile([P, 18, F], FP16, tag="mmD2" + tp, name="mmD2" + tp, bufs=1)
            D2v = D2[:].rearrange("p (i j r) f -> p i j r f", i=3, j=3)
        for k in range(3):
            if skip_diag_re:
                isl = (slice(1, 3), slice(0, 3, 2), slice(0, 2))[k]
                ni = 2
            else:
                isl = slice(0, 3)
                ni = 3
            for (ra, rb), pt in P4.items():
                re_pair = (ra == rb)  # these two feed the real path only
                rows = isl if (re_pair and skip_diag_re) else slice(0, 3)
                nr = ni if (re_pair and skip_diag_re) else 3
                if dag_b:
                    bsel = bv_dag[:, k, :, rb, :]  # B[k,j]: [P, j(3), F]
                else:
                    bsel = bv_all[:, :, k, rb, :]  # B[j,k]: [P, j(3), F]
                bb = bsel.unsqueeze(1).broadcast_to([P, nr, 3, F])
                eng.tensor_mul(
                    pt[:].rearrange("p (i j) f -> p i j f", i=3)[:, 0:nr],
                    av_all[:, rows, :, ra, :], bb)
            p4v = {rr: P4[rr][:].rearrange("p (i j) f -> p i j f", i=3) for rr in P4}
            if skip_diag_re:
                nre = ni
                if dag_b:
                    eng.tensor_add(Dre[:, 0:3 * nre, :], P4[(0, 0)][:, 0:3 * nre, :],
                                   P4[(1, 1)][:, 0:3 * nre, :])
                    eng.tensor_sub(Dim[:], P4[(1, 0)][:], P4[(0, 1)][:])
                else:
                    eng.tensor_sub(Dre[:, 0:3 * nre, :], P4[(0, 0)][:, 0:3 * nre, :],
                                   P4[(1, 1)][:, 0:3 * nre, :])
                    eng.tensor_add(Dim[:], P4[(0, 1)][:], P4[(1, 0)][:])
                for r, Dt in ((0, Dre), (1, Dim)):
                    rows = isl if r == 0 else slice(0, 3)
                    nr = nre if r == 0 else 3
                    ov = ov_all[:, rows, k, r, :]  # [P, nr, F]
                    Dv = Dt[:].rearrange("p (i j) f -> p i j f", i=3)
                    eng.tensor_add(ov, Dv[:, 0:nr, 0, :], Dv[:, 0:nr, 1, :])
                    eng.tensor_add(ov, ov, Dv[:, 0:nr, 2, :])
            else:
                # interleaved D2 (i,j,r) -> fused (i,r) reduction over j
                if dag_b:
                    eng.tensor_add(D2v[:, :, :, 0, :], p4v[(0, 0)], p4v[(1, 1)])
                    eng.tensor_sub(D2v[:, :, :, 1, :], p4v[(1, 0)], p4v[(0, 1)])
                else:
                    eng.tensor_sub(D2v[:, :, :, 0, :], p4v[(0, 0)], p4v[(1, 1)])
                    eng.tensor_add(D2v[:, :, :, 1, :], p4v[(0, 1)], p4v[(1, 0)])
                ov = ov_all[:, :, k, :, :]  # [P, i(3), r(2), F]
                eng.tensor_add(ov, D2v[:, :, 0, :, :], D2v[:, :, 1, :, :])
                eng.tensor_add(ov, ov, D2v[:, :, 2, :, :])

    def emit_cmatvec(pool, uh_t, u_t, h_t, dag):
        """uh[c,i] = sum_j Utilde[i,j] h[c,j]; h/uh: [P,12,F]; fused over c."""
        if dag:
            uv = u_t[:].rearrange("p (j i r) f -> p i j r f", j=3, i=3)
        else:
            uv = u_t[:].rearrange("p (i j r) f -> p i j r f", i=3, j=3)
        hv = h_t[:].rearrange("p (c cl r) f -> p c cl r f", c=2, cl=3)
        ov = uh_t[:].rearrange("p (c i r) f -> p c i r f", c=2, i=3)
        P4 = {}
        for ra in (0, 1):
            for rb in (0, 1):
                P4[(ra, rb)] = pool.tile([P, 18, F], FP16, tag=f"mvP{ra}{rb}",
                                         name=f"mvP{ra}{rb}", bufs=1)
        Dre = pool.tile([P, 18, F], FP16, tag="mvDre", name="mvDre", bufs=1)
        Dim = pool.tile([P, 18, F], FP16, tag="mvDim", name="mvDim", bufs=1)
        for c in range(2):
            for (ra, rb), pt in P4.items():
                hb = hv[:, c, :, rb, :].unsqueeze(1).broadcast_to([P, 3, 3, F])
                nc.vector.tensor_mul(
                    pt[:].rearrange("p (c2 i j) f -> p c2 i j f", c2=2, i=3)[:, c],
                    uv[:, :, :, ra, :], hb)
        if dag:
            # conj is on U (first factor): im = Ur*hi - Ui*hr
            nc.vector.tensor_add(Dre[:], P4[(0, 0)][:], P4[(1, 1)][:])
            nc.vector.tensor_sub(Dim[:], P4[(0, 1)][:], P4[(1, 0)][:])
        else:
            nc.vector.tensor_sub(Dre[:], P4[(0, 0)][:], P4[(1, 1)][:])
            nc.vector.tensor_add(Dim[:], P4[(0, 1)][:], P4[(1, 0)][:])
        ov2 = uh_t[:].rearrange("p (ci r) f -> p ci r f", ci=6)
        for r, Dt in ((0, Dre), (1, Dim)):
            o1 = ov2[:, :, r, :]  # [P, (c i)(6), F]
            Dv = Dt[:].rearrange("p (ci j) f -> p ci j f", ci=6)
            nc.vector.tensor_add(o1, Dv[:, :, 0, :], Dv[:, :, 1, :])
            nc.vector.tensor_add(o1, o1, Dv[:, :, 2, :])

    POOL_CMM = False
    _shuf_engs = [nc.gpsimd]
    _shuf_idx = [0]

    def _next_shuf_eng():
        _shuf_idx[0] += 1
        return _shuf_engs[_shuf_idx[0] % len(_shuf_engs)]

    with ctile.TileContext(nc) as tc:
        # ---------------- phase 1: G build ----------------
        with tc.tile_pool(name="lnk", bufs=2) as lnk, \
             tc.tile_pool(name="gtmp", bufs=2) as gtmp, \
             tc.tile_pool(name="gout", bufs=2) as goutp:
            for w in range(6):
                cache = {}

                def load_link(key, wi, tag):
                    ck = (key, wi)
                    if ck not in cache:
                        t = lnk.tile([P, 18, F], FP16, tag=tag, name=tag)
                        nc.sync.dma_start(t[:], u_in[key][wi])
                        cache[ck] = t
                    return cache[ck]

                for p, (mu, nu) in enumerate(PAIRS):
                    if mu != 0 and w < 2:
                        continue
                    if mu == 0:
                        M1 = load_link(('c', 0), w, "m1_" + str(p))
                        M2 = load_link(('c', nu), w + 1, "m2_" + str(p))
                        M3 = load_link(('c', nu), w, "m3_" + str(p))
                        M4 = load_link(('f', 0, nu), w, "m4_" + str(p))
                    else:
                        M1 = load_link(('c', mu), w, "m1_" + str(p))
                        M2 = load_link(('f', nu, mu), w, "m2_" + str(p))
                        M3 = load_link(('c', nu), w, "m3_" + str(p))
                        M4 = load_link(('f', mu, nu), w, "m4_" + str(p))
                    # offload some units' independent A/B products to Pool
                    on_pool = ((2 * p + w) % 3 == 0) and POOL_CMM
                    At = gtmp.tile([P, 18, F], FP16, tag="A", name="A")
                    Bt = gtmp.tile([P, 18, F], FP16, tag="B", name="B")
                    Wt = gtmp.tile([P, 18, F], FP16, tag="W", name="W")
                    peng = nc.gpsimd if on_pool else nc.vector
                    ptp = "g" if on_pool else ""
                    emit_cmatmul(gtmp, At, M1, M2, dag_b=False, eng=peng, tp=ptp)
                    emit_cmatmul(gtmp, Bt, M3, M4, dag_b=False, eng=peng, tp=ptp)
                    emit_cmatmul(gtmp, Wt, At, Bt, dag_b=True, skip_diag_re=True)
                    Gt = goutp.tile([P, 9, F], FP16, tag="G", name="G")
                    # batched anti-hermitian assembly (pairs (0,1),(0,2),(1,2)):
                    # offd re: G[2q] = W[ij] - W[ji]; im: G[2q+1] = W[ij]+W[ji]
                    nc.vector.tensor_sub(Gt[:, 0:3:2, :], Wt[:, 2:5:2, :], Wt[:, 6:13:6, :])
                    nc.vector.tensor_sub(Gt[:, 4:5, :], Wt[:, 10:11, :], Wt[:, 14:15, :])
                    nc.vector.tensor_add(Gt[:, 1:4:2, :], Wt[:, 3:6:2, :], Wt[:, 7:14:6, :])
                    nc.vector.tensor_add(Gt[:, 5:6, :], Wt[:, 11:12, :], Wt[:, 15:16, :])
                    nc.vector.tensor_scalar_mul(Gt[:, 6:9, :], Wt[:, 1:18:8, :], 2.0)
                    nc.scalar.dma_start(
                        gps[p][w].rearrange("c (p2 f) -> p2 c f", p2=P), Gt[:])
                    if DEBUG_DUMP:
                        nc.sync.dma_start(
                            dbg['g'][p, w].rearrange("c (p2 f) -> p2 c f", p2=P), Gt[:])

                # deduped G shuffles whose source slice just became ready
                for (p, wsrc, dz, dy, dx), buf in shuf_map.items():
                    if wsrc != w:
                        continue
                    src = gps[p][w].rearrange("c (z y x) -> c z y x", z=Z, y=Y)
                    dst = buf.rearrange("c (z y x) -> c z y x", z=Z, y=Y)
                    qeng = _next_shuf_eng()
                    for (zd, zs, zl) in _axis_pieces(dz, Z):
                        for (yd, ys, yl) in _axis_pieces(dy, Y):
                            for (xd, xs, xl) in _axis_pieces(dx, X):
                                with nc.allow_non_contiguous_dma(reason="wrap"):
                                    qeng.dma_start(
                                        dst[:, zd:zd + zl, yd:yd + yl, xd:xd + xl],
                                        src[:, zs:zs + zl, ys:ys + yl, xs:xs + xl])

        # ---------------- phase 2: apply + hop ----------------
        with tc.tile_pool(name="gld", bufs=2) as gld, \
             tc.tile_pool(name="ftl", bufs=2) as ftl, \
             tc.tile_pool(name="bbl", bufs=2) as bbl, \
             tc.tile_pool(name="psl", bufs=2) as psl, \
             tc.tile_pool(name="uhp", bufs=2) as uhp, \
             tc.tile_pool(name="htm", bufs=2) as htm, \
             tc.tile_pool(name="oot", bufs=2) as oot:
            for o in range(TLOC):
                w = o + 2
                # F_tilde per plane
                ftil = []
                for p in range(6):
                    g0 = gld.tile([P, 9, F], FP16, tag="g0", name="g0")
                    nc.sync.dma_start(g0[:], gps[p][w].rearrange("c (p2 f) -> p2 c f", p2=P))
                    ft = ftl.tile([P, 9, F], FP16, tag=f"ft{p}", name=f"ft{p}")
                    first = True
                    for k in range(3):
                        dt, dz, dy, dx = DELTAS[p][k]
                        gbuf = shuf_map[(p, o + 2 + dt, dz, dy, dx)]
                        gk = gld.tile([P, 9, F], FP16, tag=f"g{k + 1}", name=f"g{k + 1}")
                        nc.sync.dma_start(gk[:], gbuf.rearrange("c (p2 f) -> p2 c f", p2=P))
                        if first:
                            nc.vector.tensor_sub(ft[:], g0[:], gk[:])
                            first = False
                        else:
                            nc.vector.tensor_sub(ft[:], ft[:], gk[:])
                    if DEBUG_DUMP and o == 0:
                        nc.sync.dma_start(dbg['ft'][p], ft[:])
                    ftil.append(ft)

                # B blocks: block-structured build.
                # B/c = [[M~, L~+],[L~, -M~]] (hermitian), from raw F-combos:
                #   M9 = F3 + s*F2 ; S9 = F4 - s*F1 ; T9 = s*F0 + F5  (s=+1 blk0, -1 blk1)
                # CCLOV scale applied via pre-scaled psi; DIAG handled post-apply.
                bts = [bbl.tile([P, 72, F], FP16, tag=f"B{blk}", name=f"B{blk}") for blk in range(2)]
                stt_t = [bbl.tile([P, 18, F], FP16, tag=f"ST{blk}", name=f"ST{blk}") for blk in range(2)]
                for blk in range(2):
                    bt = bts[blk]
                    bv = bt[:].rearrange("p (A B r) f -> p A B r f", A=6, B=6)
                    sv = bt[:].rearrange("p (A B r) f -> p B A r f", A=6, B=6)
                    St = stt_t[blk][:, 0:9, :]
                    Tt = stt_t[blk][:, 9:18, :]
                    Ft = [ftil[p] for p in range(6)]
                    if blk == 0:
                        nc.vector.tensor_sub(St, Ft[4][:], Ft[1][:])
                        nc.vector.tensor_add(Tt, Ft[0][:], Ft[5][:])
                    else:
                        nc.vector.tensor_add(St, Ft[4][:], Ft[1][:])
                        nc.vector.tensor_sub(Tt, Ft[5][:], Ft[0][:])

                    def madd(dst, ca, cb):  # dst = F3[ca] + s*F2[cb-slice]
                        if blk == 0:
                            nc.vector.tensor_add(dst, Ft[3][:, ca, :], Ft[2][:, cb, :])
                        else:
                            nc.vector.tensor_sub(dst, Ft[3][:, ca, :], Ft[2][:, cb, :])

                    def mneg(dst, ca, cb):  # dst = -(F3[ca] + s*F2[cb])
                        if blk == 0:
                            nc.vector.scalar_tensor_tensor(
                                dst, Ft[3][:, ca, :], -1.0, Ft[2][:, cb, :],
                                AL.mult, AL.subtract)
                        else:
                            nc.vector.tensor_sub(dst, Ft[2][:, cb, :], Ft[3][:, ca, :])

                    odd2, odd1 = slice(1, 5, 2), slice(5, 6)
                    ev2, ev1 = slice(0, 4, 2), slice(4, 5)
                    # UL quadrant: up.re / up.im
                    madd(bv[:, 0, 1:3, 0, :], odd2, odd2)
                    madd(bv[:, 1, 2:3, 0, :], odd1, odd1)
                    mneg(bv[:, 0, 1:3, 1, :], ev2, ev2)
                    mneg(bv[:, 1, 2:3, 1, :], ev1, ev1)
                    # UL lo.re / lo.im
                    madd(bv[:, 1, 0:1, 0, :], slice(1, 2), slice(1, 2))
                    madd(bv[:, 2, 0:2, 0, :], slice(3, 7, 2), slice(3, 7, 2))
                    madd(bv[:, 1, 0:1, 1, :], slice(0, 1), slice(0, 1))
                    madd(bv[:, 2, 0:2, 1, :], slice(2, 6, 2), slice(2, 6, 2))
                    # UL diag: re = M9[6+d]; im = 0
                    madd(bt[:, 0:29:14, :], slice(6, 9), slice(6, 9))
                    nc.vector.memzero(bt[:, 1:30:14, :])
                    # LL: up.re = S[2q]+T[2q+1] ; up.im = S[2q+1]-T[2q]
                    nc.vector.tensor_add(bv[:, 3, 1:3, 0, :], St[:, 0:4:2, :], Tt[:, 1:5:2, :])
                    nc.vector.tensor_add(bv[:, 4, 2:3, 0, :], St[:, 4:5, :], Tt[:, 5:6, :])
                    nc.vector.tensor_sub(bv[:, 3, 1:3, 1, :], St[:, 1:5:2, :], Tt[:, 0:4:2, :])
                    nc.vector.tensor_sub(bv[:, 4, 2:3, 1, :], St[:, 5:6, :], Tt[:, 4:5, :])
                    # LL lo.re = -S[2q]+T[2q+1] ; lo.im = S[2q+1]+T[2q]
                    nc.vector.tensor_sub(bv[:, 4, 0:1, 0, :], Tt[:, 1:2, :], St[:, 0:1, :])
                    nc.vector.tensor_sub(bv[:, 5, 0:2, 0, :], Tt[:, 3:7:2, :], St[:, 2:6:2, :])
                    nc.vector.tensor_add(bv[:, 4, 0:1, 1, :], St[:, 1:2, :], Tt[:, 0:1, :])
                    nc.vector.tensor_add(bv[:, 5, 0:2, 1, :], St[:, 3:7:2, :], Tt[:, 2:6:2, :])
                    # LL diag: re = T[6+d], im = S[6+d]  (comps 36/50/64, 37/51/65)
                    nc.vector.tensor_copy(bt[:, 36:65:14, :], Tt[:, 6:9, :])
                    nc.vector.tensor_copy(bt[:, 37:66:14, :], St[:, 6:9, :])
                    # LR = -UL  (flattened (B,r) view keeps the AP 3-D)
                    bv2 = bt[:].rearrange("p (A BR) f -> p A BR f", A=6)
                    nc.vector.tensor_scalar_mul(bv2[:, 3:6, 6:12, :], bv2[:, 0:3, 0:6, :], -1.0)
                    # UR = conj-transpose(LL)
                    nc.vector.tensor_copy(bv[:, 0:3, 3:6, 0, :], sv[:, 0:3, 3:6, 0, :])
                    nc.vector.tensor_scalar_mul(bv[:, 0:3, 3:6, 1, :], sv[:, 0:3, 3:6, 1, :], -1.0)

                # apply B to psi -> out tile (psi pre-scaled by CCLOV for the
                # F-part; the (4+m)+identity diagonal added afterwards via STT)
                psi_cr = psl.tile([P, 24, F], FP16, tag="pscr", name="pscr")
                nc.sync.dma_start(psi_cr[:], p_in[('c',)][w])
                psi_c = psl.tile([P, 24, F], FP16, tag="psc", name="psc")
                nc.vector.tensor_scalar_mul(psi_c[:], psi_cr[:], CCLOV)
                out_t = oot.tile([P, 24, F], FP16, tag="out", name="out")
                aptmp = htm.tile([P, 6, F], FP16, tag="aptmp", name="aptmp")
                aptm2 = htm.tile([P, 12, F], FP16, tag="aptm2", name="aptm2")
                for blk in range(2):
                    bt = bts[blk]
                    bv = bt[:].rearrange("p (a b r) f -> p a b r f", a=6, b=6)
                    ovv = out_t[:].rearrange("p (s r) f -> p s r f", r=2)
                    pvv = psi_c[:].rearrange("p (s r) f -> p s r f", r=2)
                    out_ri = out_t[:, blk * 12:(blk + 1) * 12, :]  # [P,12,F] (A,r)
                    out_re = ovv[:, blk * 6:(blk + 1) * 6, 0, :]
                    out_im = ovv[:, blk * 6:(blk + 1) * 6, 1, :]
                    for B_ in range(6):
                        sB = blk * 6 + B_
                        pr = pvv[:, sB:sB + 1, 0, :].broadcast_to([P, 6, F])
                        pi = pvv[:, sB:sB + 1, 1, :].broadcast_to([P, 6, F])
                        # psi (re,im) pair broadcast over A: [P, A(6), r(2), F]
                        pri = (psi_c[:, sB * 2:sB * 2 + 2, :]
                               .unsqueeze(1).broadcast_to([P, 6, 2, F]))
                        Brv = bv[:, :, B_, 0, :]
                        # Br broadcast over r: [P, A(6), r(2), F]
                        Brr = Brv.unsqueeze(2).broadcast_to([P, 6, 2, F])
                        Biv = bv[:, :, B_, 1, :]
                        ori = out_ri.rearrange("p (a r) f -> p a r f", a=6)
                        if B_ == 0:
                            nc.vector.tensor_mul(ori, Brr, pri)
                        else:
                            nc.vector.tensor_mul(
                                aptm2[:].rearrange("p (a r) f -> p a r f", a=6),
                                Brr, pri)
                            nc.vector.tensor_add(out_ri, out_ri, aptm2[:])
                        nc.vector.tensor_mul(aptmp[:], Biv, pi)
                        nc.vector.tensor_sub(out_re, out_re, aptmp[:])
                        nc.vector.tensor_mul(aptmp[:], Biv, pr)
                        nc.vector.tensor_add(out_im, out_im, aptmp[:])

                # diagonal (4+m)+identity term, on the unscaled psi
                nc.vector.scalar_tensor_tensor(
                    out_t[:], psi_cr[:], DIAG, out_t[:], AL.mult, AL.add)

                if DEBUG_DUMP and o == 0:
                    for blk in range(2):
                        nc.sync.dma_start(dbg['bb'][blk], bts[blk][:])
                    nc.sync.dma_start(dbg['ap'][:], out_t[:])

                # hop terms
                for mu in (range(4) if ENABLE_HOP else ()):
                    tbl = HOP[mu]
                    for sgn in (1, -1):
                        # pre-projected half-spinor tile (host-built)
                        h = psl.tile([P, 12, F], FP16, tag="psv", name="psv")
                        if mu == 0:
                            nc.sync.dma_start(h[:], p_in[('h', 0, sgn)][w + (1 if sgn > 0 else -1)])
                        else:
                            nc.sync.dma_start(h[:], p_in[('h', mu, sgn)][w])
                        # U tile
                        ut = uhp.tile([P, 18, F], FP16, tag="ut", name="ut")
                        if sgn > 0:
                            nc.sync.dma_start(ut[:], u_in[('c', mu)][w])
                        elif mu == 0:
                            nc.sync.dma_start(ut[:], u_in[('c', 0)][w - 1])
                        else:
                            nc.sync.dma_start(ut[:], u_in[('b', mu)][w])
                        # color mult
                        uh = htm.tile([P, 12, F], FP16, tag="uh", name="uh")
                        emit_cmatvec(uhp, uh, ut, h, dag=(sgn < 0))
                        # accumulate into out (rows 0,1 in one op)
                        sl = out_t[:, 0:12, :]
                        nc.vector.scalar_tensor_tensor(
                            sl, uh[:, 0:12, :], -0.5, sl, AL.mult, AL.add)
                        uvv = uh[:].rearrange("p (s r) f -> p s r f", r=2)
                        ovv = out_t[:].rearrange("p (s r) f -> p s r f", r=2)
                        rcs = [sgn * tbl['rc'][cp] for cp in range(2)]
                        if rcs[0] == rcs[1] and tbl['m'] == (0, 1) and rcs[0] in (1, -1):
                            sl = out_t[:, 12:24, :]
                            nc.vector.scalar_tensor_tensor(
                                sl, uh[:, 0:12, :], -0.5 * rcs[0], sl,
                                AL.mult, AL.add)
                            continue
                        for cp in range(2):
                            rc = rcs[cp]
                            mm = tbl['m'][cp]
                            row = 2 + cp
                            if rc in (1, -1):
                                sl = out_t[:, row * 6:(row + 1) * 6, :]
                                nc.vector.scalar_tensor_tensor(
                                    sl, uh[:, mm * 6:(mm + 1) * 6, :], -0.5 * rc, sl,
                                    AL.mult, AL.add)
                            else:
                                s_i = rc.imag
                                o_re = ovv[:, row * 3:(row + 1) * 3, 0, :]
                                o_im = ovv[:, row * 3:(row + 1) * 3, 1, :]
                                u_re = uvv[:, mm * 3:(mm + 1) * 3, 0, :]
                                u_im = uvv[:, mm * 3:(mm + 1) * 3, 1, :]
                                nc.vector.scalar_tensor_tensor(
                                    o_re, u_im, 0.5 * s_i, o_re, AL.mult, AL.add)
                                nc.vector.scalar_tensor_tensor(
                                    o_im, u_re, -0.5 * s_i, o_im, AL.mult, AL.add)

                # store (fp16 -> fp32 cast via SWDGE)
                nc.gpsimd.dma_start(out_dram[o], out_t[:])

    nc.finalize()
    return nc


_PROG_CACHE = {}


def _get_program():
    if 'nc' not in _PROG_CACHE:
        _PROG_CACHE['nc'] = _build_device_program()
    return _PROG_CACHE['nc']


def _sbuf_image(a, C):
    """[T, C, NSITE] -> [T, P, C, F] contiguous."""
    return np.ascontiguousarray(a.reshape(T, C, P, F).transpose(0, 2, 1, 3))


def build_in_maps(psi, U):
    link_vars = _to_planar_links(U)
    psi_vars = _to_planar_psi(psi)
    link_imgs = {k: _sbuf_image(v, 18) for k, v in link_vars.items()}
    psi_imgs = {k: _sbuf_image(v, 24 if k == ('c',) else 12)
                for k, v in psi_vars.items()}
    in_maps = []
    for core in range(NCORES):
        t0 = core * TLOC
        tw = [(t0 - 2 + w) % T for w in range(NWIN)]
        m = {}
        for k in LINK_KEYS:
            m[_lname(k)] = np.ascontiguousarray(link_imgs[k][tw])
        for k in PSI_KEYS:
            m[_pname(k)] = np.ascontiguousarray(psi_imgs[k][tw])
        in_maps.append(m)
    return in_maps


def assemble_output(results):
    out = np.empty((T, 24, NSITE), np.float32)
    for core in range(NCORES):
        r = results[core]['out']  # [TLOC, P, 24, F] fp32
        out[core * TLOC:(core + 1) * TLOC] = r.transpose(0, 2, 1, 3).reshape(TLOC, 24, NSITE)
    res = (out[:, 0::2, :] + 1j * out[:, 1::2, :]).astype(np.complex64)
    return res.transpose(0, 2, 1).reshape(T, Z, Y, X, NS, NCOL)


def kernel(psi, U):
    psi = np.asarray(psi)
    U = np.asarray(U)
    from concourse.bass_utils import run_bass_kernel_spmd
    nc = _get_program()
    in_maps = build_in_maps(psi, U)
    res = run_bass_kernel_spmd(nc, in_maps, core_ids=list(range(NCORES)))
    return assemble_output(res.results)



# revision 26
# speedup vs baseline: 1.5700x; 1.0098x over previous
"""Clover-Wilson Dirac operator on Trainium2 (8 NeuronCores, T-sharded).

Math summary (derived + numerically verified against the reference):
- The reference's 4-leaf "clover" Q for plane (mu,nu) factorizes as
      Q(x) = W(x) + W(x+d1)^+ + W(x+d2)^+ + W(x+d3)^+
  with W(x) = [U_mu(x) U_nu(x+mu)] [U_nu(x) U_mu(x+nu)]^+,
  d1 = nu-mu, d2 = -2mu-2nu, d3 = -2nu (unit lattice vectors).
- With G = W - W^+ (anti-Hermitian), Ftil := Q - Q^+ = G(x) - G(x+d1) - G(x+d2) - G(x+d3).
- C psi + (4+m) psi = (5+m) psi + (csw/32) * sum_p (sigma_p (x) (-i Ftil_p)) psi,
  where sigma_p is block-diagonal (2x2 chiral blocks) in this basis.
- Wilson hop uses the standard spin-projection trick (2 half-spinors per direction).

Distribution: T=32 sharded 4 slices per core; U needs halo t0-2..t0+4 (7 slices),
psi needs t0-1..t0+4. All jnp.roll shifts are pushed into host-precomputed
pre-rolled planar fp16 arrays; on-device shifted reads of the intermediate G
use DRAM->DRAM affine shuffle DMAs.
"""
import numpy as np

T, Z, Y, X = 32, 24, 24, 24
NCOL, NS = 3, 4
MASS, CSW = 0.1, 1.0
PAIRS = [(0, 1), (0, 2), (0, 3), (1, 2), (1, 3), (2, 3)]
NCORES = 8
TLOC = T // NCORES          # 4 output slices per core
NSITE = Z * Y * X           # 13824
P = 128
F = NSITE // P              # 108
NWIN = 7                    # U window slices: t0-2 .. t0+4
DIAG = 5.0 + MASS           # (4+m) + clover identity
CCLOV = CSW / 32.0          # |coefficient| of sigma (x) Ftil; overall factor -i


# ----------------------------------------------------------------- tables

def _gammas():
    i = 1j
    g0 = np.array([[0, 0, 1, 0], [0, 0, 0, 1], [1, 0, 0, 0], [0, 1, 0, 0]], np.complex128)
    g1 = np.array([[0, 0, 0, i], [0, 0, i, 0], [0, -i, 0, 0], [-i, 0, 0, 0]], np.complex128)
    g2 = np.array([[0, 0, 0, -1], [0, 0, 1, 0], [0, 1, 0, 0], [-1, 0, 0, 0]], np.complex128)
    g3 = np.array([[0, 0, i, 0], [0, 0, 0, -i], [-i, 0, 0, 0], [0, i, 0, 0]], np.complex128)
    return [g0, g1, g2, g3]


def _sigma_blocks():
    """Chiral 2x2 blocks of sigma_{mu nu} = i g_mu g_nu for each plane."""
    G = _gammas()
    ups, dns = [], []
    for mu, nu in PAIRS:
        s = 1j * (G[mu] @ G[nu])
        assert np.abs(s[:2, 2:]).max() < 1e-12 and np.abs(s[2:, :2]).max() < 1e-12
        ups.append(s[:2, :2].copy())
        dns.append(s[2:, 2:].copy())
    return ups, dns


SIG_UP, SIG_DN = _sigma_blocks()

# per-plane shift deltas (t, z, y, x) for the W-factorization
def _deltas():
    out = []
    for mu, nu in PAIRS:
        e_mu = np.zeros(4, np.int64); e_mu[mu] = 1
        e_nu = np.zeros(4, np.int64); e_nu[nu] = 1
        out.append([tuple(e_nu - e_mu), tuple(-2 * e_mu - 2 * e_nu), tuple(-2 * e_nu)])
    return out


DELTAS = _deltas()

# debug toggles (affect both simulate_core and the device program)
ENABLE_CLOVER = True
ENABLE_HOP = True
DEBUG_DUMP = False

# hop projection tables: psi_h[c] = psi[c] + coef * psi[b[c]]; lower rows:
# row_{2+c} = rc[c] * h[m[c]]  (forward, i.e. (1-gamma)); backward negates
# coef and rc. Verified against gammas in _check_hop_tables().
HOP = {
    0: dict(b=(2, 3), coef=(-1, -1), m=(0, 1), rc=(-1, -1)),
    1: dict(b=(3, 2), coef=(-1j, -1j), m=(1, 0), rc=(1j, 1j)),
    2: dict(b=(3, 2), coef=(1, -1), m=(1, 0), rc=(-1, 1)),
    3: dict(b=(2, 3), coef=(-1j, 1j), m=(0, 1), rc=(1j, -1j)),
}


def _check_hop_tables():
    G = _gammas()
    for mu, t in HOP.items():
        for sgn in (+1, -1):  # +1: (1-g) fwd ; -1: (1+g) bwd
            M = np.eye(4) - sgn * G[mu]
            # build from table
            B = np.zeros((4, 4), np.complex128)
            for c in range(2):
                B[c, c] += 1
                B[c, t['b'][c]] += sgn * t['coef'][c]
            for c in range(2):
                rc = sgn * t['rc'][c]
                B[2 + c, t['m'][c]] += rc
                B[2 + c, t['b'][t['m'][c]]] += rc * sgn * t['coef'][t['m'][c]]
            assert np.abs(B - M).max() < 1e-12, (mu, sgn, B, M)


_check_hop_tables()


# ------------------------------------------------- planar layout helpers

def _to_planar_links(U):
    """U: (T,Z,Y,X,4,3,3) complex64 -> dict of fp16 planar arrays.

    Returns variants[key] = array [T, 18, NSITE] fp16 with comp c=(i*3+j)*2+r.
    Keys: ('c', d) centered; ('f', d, e) = U_d(x+e_hat) spatial e;
          ('b', d) = U_d(x - d_hat) spatial d.
    """
    Uf32 = np.ascontiguousarray(U)  # complex64
    planar = np.empty((4, T, 18, NSITE), np.float16)
    Um = Uf32.reshape(T, NSITE, 4, 9)
    for d in range(4):
        re = Um[..., d, :].real.astype(np.float16)  # (T, NSITE, 9)
        im = Um[..., d, :].imag.astype(np.float16)
        planar[d, :, 0::2, :] = re.transpose(0, 2, 1)
        planar[d, :, 1::2, :] = im.transpose(0, 2, 1)

    def roll_sites(arr, delta):  # arr [..., NSITE]; value at x+delta
        dz, dy, dx = delta
        a = arr.reshape(*arr.shape[:-1], Z, Y, X)
        if dz: a = np.roll(a, -dz, axis=-3)
        if dy: a = np.roll(a, -dy, axis=-2)
        if dx: a = np.roll(a, -dx, axis=-1)
        return a.reshape(*arr.shape[:-1], NSITE)

    variants = {}
    for d in range(4):
        variants[('c', d)] = planar[d]
    needed_f = {(0, 1), (0, 2), (0, 3), (2, 1), (3, 1), (3, 2), (1, 2), (1, 3), (2, 3)}
    for (d, e) in needed_f:
        delta = [0, 0, 0]; delta[e - 1] = 1
        variants[('f', d, e)] = roll_sites(planar[d], delta)
    for d in (1, 2, 3):
        delta = [0, 0, 0]; delta[d - 1] = -1
        variants[('b', d)] = roll_sites(planar[d], delta)
    return variants


def _to_planar_psi(psi):
    """psi: (T,Z,Y,X,4,3) complex64 -> dict: ('c',) -> [T, 24, NSITE] fp16
    (comp c=(s*3+cl)*2+r) and pre-projected half-spinors ('h', mu, sgn) ->
    [T, 12, NSITE] fp16 (spatially pre-rolled for mu != 0)."""
    pm = psi.reshape(T, NSITE, 12)
    planar = np.empty((T, 24, NSITE), np.float16)
    planar[:, 0::2, :] = pm.real.astype(np.float16).transpose(0, 2, 1)
    planar[:, 1::2, :] = pm.imag.astype(np.float16).transpose(0, 2, 1)

    def roll_sites(arr, delta):
        dz, dy, dx = delta
        a = arr.reshape(*arr.shape[:-1], Z, Y, X)
        if dz: a = np.roll(a, -dz, axis=-3)
        if dy: a = np.roll(a, -dy, axis=-2)
        if dx: a = np.roll(a, -dx, axis=-1)
        return a.reshape(*arr.shape[:-1], NSITE)

    out = {('c',): planar}
    for mu, tbl in HOP.items():
        for sgn in (1, -1):
            h = np.empty((T, 12, NSITE), np.float16)
            for c in range(2):
                cf = sgn * tbl['coef'][c]
                b_ = tbl['b'][c]
                for cl in range(3):
                    pr = planar[:, (c * 3 + cl) * 2]
                    pi = planar[:, (c * 3 + cl) * 2 + 1]
                    qr = planar[:, (b_ * 3 + cl) * 2]
                    qi = planar[:, (b_ * 3 + cl) * 2 + 1]
                    if cf == 1:
                        hr, hi = pr + qr, pi + qi
                    elif cf == -1:
                        hr, hi = pr - qr, pi - qi
                    elif cf == 1j:
                        hr, hi = pr - qi, pi + qr
                    else:
                        hr, hi = pr + qi, pi - qr
                    h[:, (c * 3 + cl) * 2] = hr
                    h[:, (c * 3 + cl) * 2 + 1] = hi
            if mu != 0:
                delta = [0, 0, 0]
                delta[mu - 1] = 1 if sgn > 0 else -1
                h = roll_sites(h, delta)
            out[('h', mu, sgn)] = h
    return out


# ------------------------------------------------------ numpy simulator
# Step-wise fp16 mirror of the device dataflow (for validation).

def _cmm16(A, B, dag_b=False):
    """A,B: [18, N] fp16 planar 3x3 complex; returns C = A @ B(^+) fp16."""
    C = np.zeros_like(A)
    for i in range(3):
        for k in range(3):
            cre = np.zeros(A.shape[-1], np.float16)
            cim = np.zeros(A.shape[-1], np.float16)
            for j in range(3):
                ar = A[(i * 3 + j) * 2]; ai = A[(i * 3 + j) * 2 + 1]
                if dag_b:
                    br = B[(k * 3 + j) * 2]; bi = -B[(k * 3 + j) * 2 + 1].astype(np.float16)
                else:
                    br = B[(j * 3 + k) * 2]; bi = B[(j * 3 + k) * 2 + 1]
                cre = (cre + (ar * br - ai * bi)).astype(np.float16)
                cim = (cim + (ar * bi + ai * br)).astype(np.float16)
            C[(i * 3 + k) * 2] = cre
            C[(i * 3 + k) * 2 + 1] = cim
    return C


def _antiherm9(Wm):
    """W planar 18 -> G = W - W^+ in 9-comp layout:
    q*2 / q*2+1 = re/im of G[i,j] for (i,j) in [(0,1),(0,2),(1,2)]; 6+d = im G[d,d]."""
    G = np.empty((9, Wm.shape[-1]), np.float16)
    offd = [(0, 1), (0, 2), (1, 2)]
    for q, (i, j) in enumerate(offd):
        G[q * 2] = (Wm[(i * 3 + j) * 2] - Wm[(j * 3 + i) * 2]).astype(np.float16)
        G[q * 2 + 1] = (Wm[(i * 3 + j) * 2 + 1] + Wm[(j * 3 + i) * 2 + 1]).astype(np.float16)
    for d in range(3):
        G[6 + d] = (Wm[(d * 3 + d) * 2 + 1] * np.float16(2.0)).astype(np.float16)
    return G


def _f9_entry(F9, i, j):
    """(re, im) pair (arrays or (None, arr)) of Ftil[i,j] from 9-comp planar."""
    offd = {(0, 1): 0, (0, 2): 1, (1, 2): 2}
    if i == j:
        return None, F9[6 + i]
    if (i, j) in offd:
        q = offd[(i, j)]
        return F9[q * 2], F9[q * 2 + 1]
    q = offd[(j, i)]
    return -F9[q * 2], F9[q * 2 + 1]  # G[i>j] = -conj(G[j,i]) -> (-re, +im)


def _roll_sites_np(a, delta):
    dz, dy, dx = delta
    a = a.reshape(*a.shape[:-1], Z, Y, X)
    if dz: a = np.roll(a, -dz, axis=-3)
    if dy: a = np.roll(a, -dy, axis=-2)
    if dx: a = np.roll(a, -dx, axis=-1)
    return a.reshape(*a.shape[:-2], -1) if False else a.reshape(*a.shape[:-4], a.shape[-4] if a.ndim > 3 else -1, NSITE) if False else a.reshape(-1, NSITE) if a.ndim == 4 else a.reshape(NSITE)


def simulate_core(link_vars, psi_vars, t0):
    """Numpy fp16 mirror. link_vars/psi_vars: full-T variant dicts.
    Returns planar out [TLOC, 24, NSITE] float32."""
    tw = [(t0 - 2 + w) % T for w in range(NWIN)]

    def LV(key, w):
        return link_vars[key][tw[w]]

    def PV(key, w):
        return psi_vars[key][tw[w]]

    # ---- phase 1: G per plane per window slice
    Gs = {}
    for p, (mu, nu) in enumerate(PAIRS):
        ws = range(0, 6) if mu == 0 else range(2, 6)
        for w in ws:
            if mu == 0:
                M1, M2 = LV(('c', 0), w), LV(('c', nu), w + 1)
                M3, M4 = LV(('c', nu), w), LV(('f', 0, nu), w)
            else:
                M1, M2 = LV(('c', mu), w), LV(('f', nu, mu), w)
                M3, M4 = LV(('c', nu), w), LV(('f', mu, nu), w)
            A = _cmm16(M1, M2)
            B = _cmm16(M3, M4)
            Wm = _cmm16(A, B, dag_b=True)
            Gs[(p, w)] = _antiherm9(Wm)

    out = np.zeros((TLOC, 24, NSITE), np.float32)
    for o in range(TLOC):
        w = o + 2
        # ---- Ftil per plane
        F9s = []
        for p in range(6):
            acc = Gs[(p, w)].copy()
            for (dt, dz, dy, dx) in DELTAS[p]:
                g = Gs[(p, w + dt)]
                gsh = g.reshape(9, Z, Y, X)
                if dz: gsh = np.roll(gsh, -dz, axis=1)
                if dy: gsh = np.roll(gsh, -dy, axis=2)
                if dx: gsh = np.roll(gsh, -dx, axis=3)
                acc = (acc - gsh.reshape(9, NSITE)).astype(np.float16)
            F9s.append(acc)

        if not ENABLE_CLOVER:
            F9s = [np.zeros((9, NSITE), np.float16) for _ in range(6)]
        # ---- B blocks (full 6x6 complex per chirality block), fp16
        Bblk = [np.zeros((6, 6, 2, NSITE), np.float16) for _ in range(2)]
        for blk, sigs in enumerate((SIG_UP, SIG_DN)):
            for p in range(6):
                sig = sigs[p]
                for a in range(2):
                    for b in range(2):
                        s = sig[a, b]
                        if abs(s) < 1e-12:
                            continue
                        cf = -1j * CCLOV * s  # complex coefficient
                        for i in range(3):
                            for j in range(3):
                                fre, fim = _f9_entry(F9s[p], i, j)
                                A_, B_ = a * 3 + i, b * 3 + j
                                # coeff*(fre + i fim): accumulate re and im
                                cr, ci = cf.real, cf.imag
                                tgt = Bblk[blk][A_, B_]
                                if fre is not None:
                                    if cr: tgt[0] = (tgt[0] + np.float16(cr) * fre).astype(np.float16)
                                    if ci: tgt[1] = (tgt[1] + np.float16(ci) * fre).astype(np.float16)
                                if cr: tgt[1] = (tgt[1] + np.float16(cr) * fim).astype(np.float16)
                                if ci: tgt[0] = (tgt[0] - np.float16(ci) * fim).astype(np.float16)
            for A_ in range(6):
                Bblk[blk][A_, A_, 0] = (Bblk[blk][A_, A_, 0] + np.float16(DIAG)).astype(np.float16)

        # ---- apply B to psi
        psi_c = PV(('c',), w)
        for blk in range(2):
            for A_ in range(6):
                s_out = (blk * 2 + A_ // 3) * 3 + (A_ % 3)  # spinor comp index s*3+cl
                accr = np.zeros(NSITE, np.float16)
                acci = np.zeros(NSITE, np.float16)
                for B_ in range(6):
                    s_in = (blk * 2 + B_ // 3) * 3 + (B_ % 3)
                    pr = psi_c[s_in * 2]; pi = psi_c[s_in * 2 + 1]
                    br = Bblk[blk][A_, B_, 0]; bi = Bblk[blk][A_, B_, 1]
                    accr = (accr + br * pr - bi * pi).astype(np.float16)
                    acci = (acci + br * pi + bi * pr).astype(np.float16)
                out[o, s_out * 2] += accr.astype(np.float32)
                out[o, s_out * 2 + 1] += acci.astype(np.float32)

        # ---- hop terms
        for mu in (range(4) if ENABLE_HOP else ()):
            tbl = HOP[mu]
            for sgn, wpsi_key, woff, ukey, udag in (
                (+1, 'f', +1, ('c', mu), False),
                (-1, 'b', -1, ('b', mu) if mu else ('c', 0), True),
            ):
                if mu == 0:
                    psv = PV(('c',), w + woff)
                else:
                    psv = PV(('s', mu, +1 if sgn > 0 else -1), w)
                uar = LV(ukey, w) if mu else LV(ukey, w + (0 if sgn > 0 else -1))
                # project: h[c] = psi[c] + sgn*coef[c]*psi[b[c]] (2 spins x 3 col)
                h = np.zeros((2, 3, 2, NSITE), np.float16)
                for c in range(2):
                    cf = sgn * tbl['coef'][c]
                    for cl in range(3):
                        pr = psv[(c * 3 + cl) * 2]; pi = psv[(c * 3 + cl) * 2 + 1]
                        qr = psv[(tbl['b'][c] * 3 + cl) * 2]; qi = psv[(tbl['b'][c] * 3 + cl) * 2 + 1]
                        if cf == 1:
                            h[c, cl, 0] = (pr + qr).astype(np.float16); h[c, cl, 1] = (pi + qi).astype(np.float16)
                        elif cf == -1:
                            h[c, cl, 0] = (pr - qr).astype(np.float16); h[c, cl, 1] = (pi - qi).astype(np.float16)
                        elif cf == 1j:
                            h[c, cl, 0] = (pr - qi).astype(np.float16); h[c, cl, 1] = (pi + qr).astype(np.float16)
                        else:  # -1j
                            h[c, cl, 0] = (pr + qi).astype(np.float16); h[c, cl, 1] = (pi - qr).astype(np.float16)
                # color mult: uh[c, i] = sum_j U[i,j] h[c, j] (or U^+ )
                uh = np.zeros((2, 3, 2, NSITE), np.float16)
                for c in range(2):
                    for i in range(3):
                        ar = np.zeros(NSITE, np.float16); ai = np.zeros(NSITE, np.float16)
                        for j in range(3):
                            if udag:
                                ur = uar[(j * 3 + i) * 2]; ui = -uar[(j * 3 + i) * 2 + 1].astype(np.float16)
                            else:
                                ur = uar[(i * 3 + j) * 2]; ui = uar[(i * 3 + j) * 2 + 1]
                            ar = (ar + ur * h[c, j, 0] - ui * h[c, j, 1]).astype(np.float16)
                            ai = (ai + ur * h[c, j, 1] + ui * h[c, j, 0]).astype(np.float16)
                        uh[c, i, 0] = ar; uh[c, i, 1] = ai
                # accumulate: rows 0,1: -1/2*uh[c]; rows 2+c': -1/2*sgn... rc
                for c in range(2):
                    for cl in range(3):
                        out[o, (c * 3 + cl) * 2] -= 0.5 * uh[c, cl, 0].astype(np.float32)
                        out[o, (c * 3 + cl) * 2 + 1] -= 0.5 * uh[c, cl, 1].astype(np.float32)
                for cp in range(2):
                    rc = sgn * tbl['rc'][cp]
                    mm = tbl['m'][cp]
                    for cl in range(3):
                        tr = uh[mm, cl, 0].astype(np.float32); ti = uh[mm, cl, 1].astype(np.float32)
                        if rc == 1:
                            out[o, ((2 + cp) * 3 + cl) * 2] -= 0.5 * tr
                            out[o, ((2 + cp) * 3 + cl) * 2 + 1] -= 0.5 * ti
                        elif rc == -1:
                            out[o, ((2 + cp) * 3 + cl) * 2] += 0.5 * tr
                            out[o, ((2 + cp) * 3 + cl) * 2 + 1] += 0.5 * ti
                        elif rc == 1j:
                            out[o, ((2 + cp) * 3 + cl) * 2] += 0.5 * ti
                            out[o, ((2 + cp) * 3 + cl) * 2 + 1] -= 0.5 * tr
                        else:  # -1j
                            out[o, ((2 + cp) * 3 + cl) * 2] -= 0.5 * ti
                            out[o, ((2 + cp) * 3 + cl) * 2 + 1] += 0.5 * tr
    return out


def simulate(psi, U):
    """Full-lattice numpy fp16 simulation -> complex64 (T,Z,Y,X,4,3)."""
    link_vars = _to_planar_links(U)
    psi_vars = _to_planar_psi(psi)
    out = np.zeros((T, 24, NSITE), np.float32)
    for core in range(NCORES):
        out[core * TLOC:(core + 1) * TLOC] = simulate_core(link_vars, psi_vars, core * TLOC)
    res = (out[:, 0::2, :] + 1j * out[:, 1::2, :]).astype(np.complex64)
    return res.transpose(0, 2, 1).reshape(T, Z, Y, X, NS, NCOL)


# =================================================================== bass

LINK_KEYS = (
    [('c', d) for d in range(4)]
    + [('f', d, e) for (d, e) in
       [(0, 1), (0, 2), (0, 3), (2, 1), (3, 1), (3, 2), (1, 2), (1, 3), (2, 3)]]
    + [('b', d) for d in (1, 2, 3)]
)
PSI_KEYS = [('c',)] + [('h', mu, sgn) for mu in range(4) for sgn in (1, -1)]


def _lname(key):
    return "u_" + "_".join(str(x) for x in key).replace('-', 'm')


def _pname(key):
    return "psi_" + "_".join(str(x) for x in key).replace('-', 'm')


def _bbuild_table():
    """Per chirality block: list of (plane, A, B(<=A), tgt_im, f9comp, coef)."""
    offd = {(0, 1): 0, (0, 2): 1, (1, 2): 2}
    tables = [[], []]
    for blk, sigs in enumerate((SIG_UP, SIG_DN)):
        for p in range(6):
            sig = sigs[p]
            for a in range(2):
                for b in range(2):
                    s = sig[a, b]
                    if abs(s) < 1e-12:
                        continue
                    cf = -1j * CCLOV * s
                    for i in range(3):
                        for j in range(3):
                            A_, B_ = a * 3 + i, b * 3 + j
                            if A_ < B_:
                                continue
                            if i == j:
                                fre = None
                                fim = (6 + i, 1.0)
                            elif (i, j) in offd:
                                q = offd[(i, j)]
                                fre = (2 * q, 1.0); fim = (2 * q + 1, 1.0)
                            else:
                                q = offd[(j, i)]
                                fre = (2 * q, -1.0); fim = (2 * q + 1, 1.0)
                            cr, ci = cf.real, cf.imag
                            for tgt_im, parts in ((0, [(fre, cr), (fim, -ci)]),
                                                  (1, [(fim, cr), (fre, ci)])):
                                if A_ == B_ and tgt_im:
                                    continue
                                for src, c0 in parts:
                                    if src is None or abs(c0) < 1e-15:
                                        continue
                                    comp, s0 = src
                                    tables[blk].append((p, A_, B_, tgt_im, comp, c0 * s0))
    # sanity: every lower-tri re comp and offdiag im comp gets >=1 write
    for blk in range(2):
        seen = {(A_, B_, t) for (_, A_, B_, t, _, _) in tables[blk]}
        for A_ in range(6):
            for B_ in range(A_ + 1):
                assert (A_, B_, 0) in seen, (blk, A_, B_)
                if A_ != B_:
                    assert (A_, B_, 1) in seen, (blk, A_, B_)
    return tables


BTABLES = _bbuild_table()


def _axis_pieces(d, L):
    """dst[i] = src[(i+d) % L] -> (dst_start, src_start, length) pieces."""
    d %= L
    if d == 0:
        return [(0, 0, L)]
    return [(0, d, L - d), (L - d, 0, d)]


def _build_device_program():
    import concourse.bacc as bacc
    import concourse.mybir as mybir
    from concourse import tile as ctile

    FP16, FP32 = mybir.dt.float16, mybir.dt.float32
    AL = mybir.AluOpType
    nc = bacc.Bacc(None, target_bir_lowering=False)

    u_in = {k: nc.declare_dram_parameter(_lname(k), [NWIN, P, 18, F], FP16, isOutput=False)
            for k in LINK_KEYS}
    p_in = {k: nc.declare_dram_parameter(
                _pname(k), [NWIN, P, 24 if k == ('c',) else 12, F], FP16,
                isOutput=False)
            for k in PSI_KEYS}
    out_dram = nc.declare_dram_parameter("out", [TLOC, P, 24, F], FP32, isOutput=True)

    dbg = {}
    if DEBUG_DUMP:
        dbg['g'] = nc.declare_dram_parameter("dbg_g", [6, NWIN, 9, NSITE], FP16, isOutput=True)
        dbg['ft'] = nc.declare_dram_parameter("dbg_ft", [6, P, 9, F], FP16, isOutput=True)
        dbg['bb'] = nc.declare_dram_parameter("dbg_bb", [2, P, 72, F], FP16, isOutput=True)
        dbg['ap'] = nc.declare_dram_parameter("dbg_ap", [P, 24, F], FP16, isOutput=True)
    gps = [[nc.dram_tensor(f"gp{p}_{w}", [9, NSITE], FP16) for w in range(NWIN)]
           for p in range(6)]
    # deduped shifted-G buffers keyed (plane, w_src, spatial shift)
    shuf_map = {}
    for p in range(6):
        for k, (dt, dz, dy, dx) in enumerate(DELTAS[p]):
            for o in range(TLOC):
                wsrc = o + 2 + dt
                key = (p, wsrc, dz, dy, dx)
                if key not in shuf_map:
                    shuf_map[key] = nc.dram_tensor(
                        f"gsh{p}_{wsrc}_{dz}_{dy}_{dx}".replace('-', 'm'),
                        [9, NSITE], FP16)

    def emit_cmatmul(pool, out_t, a_t, b_t, dag_b, eng=None, tp="", skip_diag_re=False):
        """out = A @ B(^+), 3x3 complex, per output column. With
        skip_diag_re, the real parts of out[k,k] are left unwritten
        (garbage) — valid when only the anti-hermitian part is consumed."""
        eng = eng if eng is not None else nc.vector
        P4 = {}
        for ra in (0, 1):
            for rb in (0, 1):
                P4[(ra, rb)] = pool.tile([P, 9, F], FP16, tag=f"mmP{ra}{rb}{tp}",
                                         name=f"mmP{ra}{rb}{tp}", bufs=1)
        av_all = a_t[:].rearrange("p (i j r) f -> p i j r f", i=3, j=3)
        bv_all = b_t[:].rearrange("p (j k r) f -> p j k r f", j=3, k=3)
        bv_dag = b_t[:].rearrange("p (k j r) f -> p k j r f", k=3, j=3)
        ov_all = out_t[:].rearrange("p (i k r) f -> p i k r f", i=3, k=3)
        if skip_diag_re:
            Dre = pool.tile([P, 9, F], FP16, tag="mmDre" + tp, name="mmDre" + tp, bufs=1)
            Dim = pool.tile([P, 9, F], FP16, tag="mmDim" + tp, name="mmDim" + tp, bufs=1)
        else:
            D2 = pool.tile([P, 18, F], FP16, tag="mmD2" + tp, name="mmD2" + tp, bufs=1)
            D2v = D2[:].rearrange("p (i j r) f -> p i j r f", i=3, j=3)
        for k in range(3):
            if skip_diag_re:
                isl = (slice(1, 3), slice(0, 3, 2), slice(0, 2))[k]
                ni = 2
            else:
                isl = slice(0, 3)
                ni = 3
            for (ra, rb), pt in P4.items():
                re_pair = (ra == rb)  # these two feed the real path only
                rows = isl if (re_pair and skip_diag_re) else slice(0, 3)
                nr = ni if (re_pair and skip_diag_re) else 3
                if dag_b:
                    bsel = bv_dag[:, k, :, rb, :]  # B[k,j]: [P, j(3), F]
                else:
                    bsel = bv_all[:, :, k, rb, :]  # B[j,k]: [P, j(3), F]
                bb = bsel.unsqueeze(1).broadcast_to([P, nr, 3, F])
                eng.tensor_mul(
                    pt[:].rearrange("p (i j) f -> p i j f", i=3)[:, 0:nr],
                    av_all[:, rows, :, ra, :], bb)
            p4v = {rr: P4[rr][:].rearrange("p (i j) f -> p i j f", i=3) for rr in P4}
            if skip_diag_re:
                nre = ni
                if dag_b:
                    eng.tensor_add(Dre[:, 0:3 * nre, :], P4[(0, 0)][:, 0:3 * nre, :],
                                   P4[(1, 1)][:, 0:3 * nre, :])
                    eng.tensor_sub(Dim[:], P4[(1, 0)][:], P4[(0, 1)][:])
                else:
                    eng.tensor_sub(Dre[:, 0:3 * nre, :], P4[(0, 0)][:, 0:3 * nre, :],
                                   P4[(1, 1)][:, 0:3 * nre, :])
                    eng.tensor_add(Dim[:], P4[(0, 1)][:], P4[(1, 0)][:])
                for r, Dt in ((0, Dre), (1, Dim)):
                    rows = isl if r == 0 else slice(0, 3)
                    nr = nre if r == 0 else 3
                    ov = ov_all[:, rows, k, r, :]  # [P, nr, F]
                    Dv = Dt[:].rearrange("p (i j) f -> p i j f", i=3)
                    eng.tensor_add(ov, Dv[:, 0:nr, 0, :], Dv[:, 0:nr, 1, :])
                    eng.tensor_add(ov, ov, Dv[:, 0:nr, 2, :])
            else:
                # interleaved D2 (i,j,r) -> fused (i,r) reduction over j
                if dag_b:
                    eng.tensor_add(D2v[:, :, :, 0, :], p4v[(0, 0)], p4v[(1, 1)])
                    eng.tensor_sub(D2v[:, :, :, 1, :], p4v[(1, 0)], p4v[(0, 1)])
                else:
                    eng.tensor_sub(D2v[:, :, :, 0, :], p4v[(0, 0)], p4v[(1, 1)])
                    eng.tensor_add(D2v[:, :, :, 1, :], p4v[(0, 1)], p4v[(1, 0)])
                ov = ov_all[:, :, k, :, :]  # [P, i(3), r(2), F]
                eng.tensor_add(ov, D2v[:, :, 0, :, :], D2v[:, :, 1, :, :])
                eng.tensor_add(ov, ov, D2v[:, :, 2, :, :])

    def emit_cmatvec(pool, uh_t, u_t, h_t, dag):
        """uh[c,i] = sum_j Utilde[i,j] h[c,j]; h/uh: [P,12,F]; fused over c."""
        if dag:
            uv = u_t[:].rearrange("p (j i r) f -> p i j r f", j=3, i=3)
        else:
            uv = u_t[:].rearrange("p (i j r) f -> p i j r f", i=3, j=3)
        hv = h_t[:].rearrange("p (c cl r) f -> p c cl r f", c=2, cl=3)
        ov = uh_t[:].rearrange("p (c i r) f -> p c i r f", c=2, i=3)
        P4 = {}
        for ra in (0, 1):
            for rb in (0, 1):
                P4[(ra, rb)] = pool.tile([P, 18, F], FP16, tag=f"mvP{ra}{rb}",
                                         name=f"mvP{ra}{rb}", bufs=1)
        Dre = pool.tile([P, 18, F], FP16, tag="mvDre", name="mvDre", bufs=1)
        Dim = pool.tile([P, 18, F], FP16, tag="mvDim", name="mvDim", bufs=1)
        for c in range(2):
            for (ra, rb), pt in P4.items():
                hb = hv[:, c, :, rb, :].unsqueeze(1).broadcast_to([P, 3, 3, F])
                nc.vector.tensor_mul(
                    pt[:].rearrange("p (c2 i j) f -> p c2 i j f", c2=2, i=3)[:, c],
                    uv[:, :, :, ra, :], hb)
        if dag:
            # conj is on U (first factor): im = Ur*hi - Ui*hr
            nc.vector.tensor_add(Dre[:], P4[(0, 0)][:], P4[(1, 1)][:])
            nc.vector.tensor_sub(Dim[:], P4[(0, 1)][:], P4[(1, 0)][:])
        else:
            nc.vector.tensor_sub(Dre[:], P4[(0, 0)][:], P4[(1, 1)][:])
            nc.vector.tensor_add(Dim[:], P4[(0, 1)][:], P4[(1, 0)][:])
        ov2 = uh_t[:].rearrange("p (ci r) f -> p ci r f", ci=6)
        for r, Dt in ((0, Dre), (1, Dim)):
            o1 = ov2[:, :, r, :]  # [P, (c i)(6), F]
            Dv = Dt[:].rearrange("p (ci j) f -> p ci j f", ci=6)
            nc.vector.tensor_add(o1, Dv[:, :, 0, :], Dv[:, :, 1, :])
            nc.vector.tensor_add(o1, o1, Dv[:, :, 2, :])

    POOL_CMM = False
    _shuf_engs = [nc.gpsimd]
    _shuf_idx = [0]

    def _next_shuf_eng():
        _shuf_idx[0] += 1
        return _shuf_engs[_shuf_idx[0] % len(_shuf_engs)]

    with ctile.TileContext(nc) as tc:
        # ---------------- phase 1: G build ----------------
        with tc.tile_pool(name="lnk", bufs=2) as lnk, \
             tc.tile_pool(name="gtmp", bufs=2) as gtmp, \
             tc.tile_pool(name="gout", bufs=2) as goutp:
            for w in range(6):
                cache = {}

                def load_link(key, wi, tag):
                    ck = (key, wi)
                    if ck not in cache:
                        t = lnk.tile([P, 18, F], FP16, tag=tag, name=tag)
                        nc.sync.dma_start(t[:], u_in[key][wi])
                        cache[ck] = t
                    return cache[ck]

                for p, (mu, nu) in enumerate(PAIRS):
                    if mu != 0 and w < 2:
                        continue
                    if mu == 0:
                        M1 = load_link(('c', 0), w, "m1_" + str(p))
                        M2 = load_link(('c', nu), w + 1, "m2_" + str(p))
                        M3 = load_link(('c', nu), w, "m3_" + str(p))
                        M4 = load_link(('f', 0, nu), w, "m4_" + str(p))
                    else:
                        M1 = load_link(('c', mu), w, "m1_" + str(p))
                        M2 = load_link(('f', nu, mu), w, "m2_" + str(p))
                        M3 = load_link(('c', nu), w, "m3_" + str(p))
                        M4 = load_link(('f', mu, nu), w, "m4_" + str(p))
                    # offload some units' independent A/B products to Pool
                    on_pool = ((2 * p + w) % 3 == 0) and POOL_CMM
                    At = gtmp.tile([P, 18, F], FP16, tag="A", name="A")
                    Bt = gtmp.tile([P, 18, F], FP16, tag="B", name="B")
                    Wt = gtmp.tile([P, 18, F], FP16, tag="W", name="W")
                    peng = nc.gpsimd if on_pool else nc.vector
                    ptp = "g" if on_pool else ""
                    emit_cmatmul(gtmp, At, M1, M2, dag_b=False, eng=peng, tp=ptp)
                    emit_cmatmul(gtmp, Bt, M3, M4, dag_b=False, eng=peng, tp=ptp)
                    emit_cmatmul(gtmp, Wt, At, Bt, dag_b=True, skip_diag_re=True)
                    Gt = goutp.tile([P, 9, F], FP16, tag="G", name="G")
                    # batched anti-hermitian assembly (pairs (0,1),(0,2),(1,2)):
                    # offd re: G[2q] = W[ij] - W[ji]; im: G[2q+1] = W[ij]+W[ji]
                    nc.vector.tensor_sub(Gt[:, 0:3:2, :], Wt[:, 2:5:2, :], Wt[:, 6:13:6, :])
                    nc.vector.tensor_sub(Gt[:, 4:5, :], Wt[:, 10:11, :], Wt[:, 14:15, :])
                    nc.vector.tensor_add(Gt[:, 1:4:2, :], Wt[:, 3:6:2, :], Wt[:, 7:14:6, :])
                    nc.vector.tensor_add(Gt[:, 5:6, :], Wt[:, 11:12, :], Wt[:, 15:16, :])
                    nc.vector.tensor_scalar_mul(Gt[:, 6:9, :], Wt[:, 1:18:8, :], 2.0)
                    nc.scalar.dma_start(
                        gps[p][w].rearrange("c (p2 f) -> p2 c f", p2=P), Gt[:])
                    if DEBUG_DUMP:
                        nc.sync.dma_start(
                            dbg['g'][p, w].rearrange("c (p2 f) -> p2 c f", p2=P), Gt[:])

                # deduped G shuffles whose source slice just became ready
                for (p, wsrc, dz, dy, dx), buf in shuf_map.items():
                    if wsrc != w:
                        continue
                    src = gps[p][w].rearrange("c (z y x) -> c z y x", z=Z, y=Y)
                    dst = buf.rearrange("c (z y x) -> c z y x", z=Z, y=Y)
                    qeng = _next_shuf_eng()
                    for (zd, zs, zl) in _axis_pieces(dz, Z):
                        for (yd, ys, yl) in _axis_pieces(dy, Y):
                            for (xd, xs, xl) in _axis_pieces(dx, X):
                                with nc.allow_non_contiguous_dma(reason="wrap"):
                                    qeng.dma_start(
                                        dst[:, zd:zd + zl, yd:yd + yl, xd:xd + xl],
                                        src[:, zs:zs + zl, ys:ys + yl, xs:xs + xl])

        # ---------------- phase 2: apply + hop ----------------
        with tc.tile_pool(name="gld", bufs=3) as gld, \
             tc.tile_pool(name="ftl", bufs=2) as ftl, \
             tc.tile_pool(name="bbl", bufs=2) as bbl, \
             tc.tile_pool(name="psl", bufs=2) as psl, \
             tc.tile_pool(name="uhp", bufs=2) as uhp, \
             tc.tile_pool(name="htm", bufs=2) as htm, \
             tc.tile_pool(name="oot", bufs=2) as oot:
            for o in range(TLOC):
                w = o + 2
                # F_tilde per plane
                ftil = []
                for p in range(6):
                    g0 = gld.tile([P, 9, F], FP16, tag="g0", name="g0")
                    nc.scalar.dma_start(g0[:], gps[p][w].rearrange("c (p2 f) -> p2 c f", p2=P))
                    ft = ftl.tile([P, 9, F], FP16, tag=f"ft{p}", name=f"ft{p}")
                    first = True
                    for k in range(3):
                        dt, dz, dy, dx = DELTAS[p][k]
                        gbuf = shuf_map[(p, o + 2 + dt, dz, dy, dx)]
                        gk = gld.tile([P, 9, F], FP16, tag=f"g{k + 1}", name=f"g{k + 1}")
                        nc.scalar.dma_start(gk[:], gbuf.rearrange("c (p2 f) -> p2 c f", p2=P))
                        if first:
                            nc.vector.tensor_sub(ft[:], g0[:], gk[:])
                            first = False
                        else:
                            nc.vector.tensor_sub(ft[:], ft[:], gk[:])
                    if DEBUG_DUMP and o == 0:
                        nc.sync.dma_start(dbg['ft'][p], ft[:])
                    ftil.append(ft)

                # B blocks: block-structured build.
                # B/c = [[M~, L~+],[L~, -M~]] (hermitian), from raw F-combos:
                #   M9 = F3 + s*F2 ; S9 = F4 - s*F1 ; T9 = s*F0 + F5  (s=+1 blk0, -1 blk1)
                # CCLOV scale applied via pre-scaled psi; DIAG handled post-apply.
                bts = [bbl.tile([P, 72, F], FP16, tag=f"B{blk}", name=f"B{blk}") for blk in range(2)]
                stt_t = [bbl.tile([P, 18, F], FP16, tag=f"ST{blk}", name=f"ST{blk}") for blk in range(2)]
                for blk in range(2):
                    bt = bts[blk]
                    bv = bt[:].rearrange("p (A B r) f -> p A B r f", A=6, B=6)
                    sv = bt[:].rearrange("p (A B r) f -> p B A r f", A=6, B=6)
                    St = stt_t[blk][:, 0:9, :]
                    Tt = stt_t[blk][:, 9:18, :]
                    Ft = [ftil[p] for p in range(6)]
                    if blk == 0:
                        nc.vector.tensor_sub(St, Ft[4][:], Ft[1][:])
                        nc.vector.tensor_add(Tt, Ft[0][:], Ft[5][:])
                    else:
                        nc.vector.tensor_add(St, Ft[4][:], Ft[1][:])
                        nc.vector.tensor_sub(Tt, Ft[5][:], Ft[0][:])

                    def madd(dst, ca, cb):  # dst = F3[ca] + s*F2[cb-slice]
                        if blk == 0:
                            nc.vector.tensor_add(dst, Ft[3][:, ca, :], Ft[2][:, cb, :])
                        else:
                            nc.vector.tensor_sub(dst, Ft[3][:, ca, :], Ft[2][:, cb, :])

                    def mneg(dst, ca, cb):  # dst = -(F3[ca] + s*F2[cb])
                        if blk == 0:
                            nc.vector.scalar_tensor_tensor(
                                dst, Ft[3][:, ca, :], -1.0, Ft[2][:, cb, :],
                                AL.mult, AL.subtract)
                        else:
                            nc.vector.tensor_sub(dst, Ft[2][:, cb, :], Ft[3][:, ca, :])

                    odd2, odd1 = slice(1, 5, 2), slice(5, 6)
                    ev2, ev1 = slice(0, 4, 2), slice(4, 5)
                    # UL quadrant: up.re / up.im
                    madd(bv[:, 0, 1:3, 0, :], odd2, odd2)
                    madd(bv[:, 1, 2:3, 0, :], odd1, odd1)
                    mneg(bv[:, 0, 1:3, 1, :], ev2, ev2)
                    mneg(bv[:, 1, 2:3, 1, :], ev1, ev1)
                    # UL lo.re / lo.im
                    madd(bv[:, 1, 0:1, 0, :], slice(1, 2), slice(1, 2))
                    madd(bv[:, 2, 0:2, 0, :], slice(3, 7, 2), slice(3, 7, 2))
                    madd(bv[:, 1, 0:1, 1, :], slice(0, 1), slice(0, 1))
                    madd(bv[:, 2, 0:2, 1, :], slice(2, 6, 2), slice(2, 6, 2))
                    # UL diag: re = M9[6+d]; im = 0
                    madd(bt[:, 0:29:14, :], slice(6, 9), slice(6, 9))
                    nc.vector.memzero(bt[:, 1:30:14, :])
                    # LL: up.re = S[2q]+T[2q+1] ; up.im = S[2q+1]-T[2q]
                    nc.vector.tensor_add(bv[:, 3, 1:3, 0, :], St[:, 0:4:2, :], Tt[:, 1:5:2, :])
                    nc.vector.tensor_add(bv[:, 4, 2:3, 0, :], St[:, 4:5, :], Tt[:, 5:6, :])
                    nc.vector.tensor_sub(bv[:, 3, 1:3, 1, :], St[:, 1:5:2, :], Tt[:, 0:4:2, :])
                    nc.vector.tensor_sub(bv[:, 4, 2:3, 1, :], St[:, 5:6, :], Tt[:, 4:5, :])
                    # LL lo.re = -S[2q]+T[2q+1] ; lo.im = S[2q+1]+T[2q]
                    nc.vector.tensor_sub(bv[:, 4, 0:1, 0, :], Tt[:, 1:2, :], St[:, 0:1, :])
                    nc.vector.tensor_sub(bv[:, 5, 0:2, 0, :], Tt[:, 3:7:2, :], St[:, 2:6:2, :])
                    nc.vector.tensor_add(bv[:, 4, 0:1, 1, :], St[:, 1:2, :], Tt[:, 0:1, :])
                    nc.vector.tensor_add(bv[:, 5, 0:2, 1, :], St[:, 3:7:2, :], Tt[:, 2:6:2, :])
                    # LL diag: re = T[6+d], im = S[6+d]  (comps 36/50/64, 37/51/65)
                    nc.vector.tensor_copy(bt[:, 36:65:14, :], Tt[:, 6:9, :])
                    nc.vector.tensor_copy(bt[:, 37:66:14, :], St[:, 6:9, :])
                    # LR = -UL  (flattened (B,r) view keeps the AP 3-D)
                    bv2 = bt[:].rearrange("p (A BR) f -> p A BR f", A=6)
                    nc.vector.tensor_scalar_mul(bv2[:, 3:6, 6:12, :], bv2[:, 0:3, 0:6, :], -1.0)
                    # UR = conj-transpose(LL)
                    nc.vector.tensor_copy(bv[:, 0:3, 3:6, 0, :], sv[:, 0:3, 3:6, 0, :])
                    nc.vector.tensor_scalar_mul(bv[:, 0:3, 3:6, 1, :], sv[:, 0:3, 3:6, 1, :], -1.0)

                # apply B to psi -> out tile (psi pre-scaled by CCLOV for the
                # F-part; the (4+m)+identity diagonal added afterwards via STT)
                psi_cr = psl.tile([P, 24, F], FP16, tag="pscr", name="pscr")
                nc.sync.dma_start(psi_cr[:], p_in[('c',)][w])
                psi_c = psl.tile([P, 24, F], FP16, tag="psc", name="psc")
                nc.vector.tensor_scalar_mul(psi_c[:], psi_cr[:], CCLOV)
                out_t = oot.tile([P, 24, F], FP16, tag="out", name="out")
                aptmp = htm.tile([P, 6, F], FP16, tag="aptmp", name="aptmp")
                aptm2 = htm.tile([P, 12, F], FP16, tag="aptm2", name="aptm2")
                for blk in range(2):
                    bt = bts[blk]
                    bv = bt[:].rearrange("p (a b r) f -> p a b r f", a=6, b=6)
                    ovv = out_t[:].rearrange("p (s r) f -> p s r f", r=2)
                    pvv = psi_c[:].rearrange("p (s r) f -> p s r f", r=2)
                    out_ri = out_t[:, blk * 12:(blk + 1) * 12, :]  # [P,12,F] (A,r)
                    out_re = ovv[:, blk * 6:(blk + 1) * 6, 0, :]
                    out_im = ovv[:, blk * 6:(blk + 1) * 6, 1, :]
                    for B_ in range(6):
                        sB = blk * 6 + B_
                        pr = pvv[:, sB:sB + 1, 0, :].broadcast_to([P, 6, F])
                        pi = pvv[:, sB:sB + 1, 1, :].broadcast_to([P, 6, F])
                        # psi (re,im) pair broadcast over A: [P, A(6), r(2), F]
                        pri = (psi_c[:, sB * 2:sB * 2 + 2, :]
                               .unsqueeze(1).broadcast_to([P, 6, 2, F]))
                        Brv = bv[:, :, B_, 0, :]
                        # Br broadcast over r: [P, A(6), r(2), F]
                        Brr = Brv.unsqueeze(2).broadcast_to([P, 6, 2, F])
                        Biv = bv[:, :, B_, 1, :]
                        ori = out_ri.rearrange("p (a r) f -> p a r f", a=6)
                        if B_ == 0:
                            nc.vector.tensor_mul(ori, Brr, pri)
                        else:
                            nc.vector.tensor_mul(
                                aptm2[:].rearrange("p (a r) f -> p a r f", a=6),
                                Brr, pri)
                            nc.vector.tensor_add(out_ri, out_ri, aptm2[:])
                        nc.vector.tensor_mul(aptmp[:], Biv, pi)
                        nc.vector.tensor_sub(out_re, out_re, aptmp[:])
                        nc.vector.tensor_mul(aptmp[:], Biv, pr)
                        nc.vector.tensor_add(out_im, out_im, aptmp[:])

                # diagonal (4+m)+identity term, on the unscaled psi
                nc.vector.scalar_tensor_tensor(
                    out_t[:], psi_cr[:], DIAG, out_t[:], AL.mult, AL.add)

                if DEBUG_DUMP and o == 0:
                    for blk in range(2):
                        nc.sync.dma_start(dbg['bb'][blk], bts[blk][:])
                    nc.sync.dma_start(dbg['ap'][:], out_t[:])

                # hop terms
                for mu in (range(4) if ENABLE_HOP else ()):
                    tbl = HOP[mu]
                    for sgn in (1, -1):
                        # pre-projected half-spinor tile (host-built)
                        h = psl.tile([P, 12, F], FP16, tag="psv", name="psv")
                        if mu == 0:
                            nc.sync.dma_start(h[:], p_in[('h', 0, sgn)][w + (1 if sgn > 0 else -1)])
                        else:
                            nc.sync.dma_start(h[:], p_in[('h', mu, sgn)][w])
                        # U tile
                        ut = uhp.tile([P, 18, F], FP16, tag="ut", name="ut")
                        if sgn > 0:
                            nc.sync.dma_start(ut[:], u_in[('c', mu)][w])
                        elif mu == 0:
                            nc.sync.dma_start(ut[:], u_in[('c', 0)][w - 1])
                        else:
                            nc.sync.dma_start(ut[:], u_in[('b', mu)][w])
                        # color mult
                        uh = htm.tile([P, 12, F], FP16, tag="uh", name="uh")
                        emit_cmatvec(uhp, uh, ut, h, dag=(sgn < 0))
                        # accumulate into out (rows 0,1 in one op)
                        sl = out_t[:, 0:12, :]
                        nc.vector.scalar_tensor_tensor(
                            sl, uh[:, 0:12, :], -0.5, sl, AL.mult, AL.add)
                        uvv = uh[:].rearrange("p (s r) f -> p s r f", r=2)
                        ovv = out_t[:].rearrange("p (s r) f -> p s r f", r=2)
                        rcs = [sgn * tbl['rc'][cp] for cp in range(2)]
                        if rcs[0] == rcs[1] and tbl['m'] == (0, 1) and rcs[0] in (1, -1):
                            sl = out_t[:, 12:24, :]
                            nc.vector.scalar_tensor_tensor(
                                sl, uh[:, 0:12, :], -0.5 * rcs[0], sl,
                                AL.mult, AL.add)
                            continue
                        for cp in range(2):
                            rc = rcs[cp]
                            mm = tbl['m'][cp]
                            row = 2 + cp
                            if rc in (1, -1):
                                sl = out_t[:, row * 6:(row + 1) * 6, :]
                                nc.vector.scalar_tensor_tensor(
                                    sl, uh[:, mm * 6:(mm + 1) * 6, :], -0.5 * rc, sl,
                                    AL.mult, AL.add)
                            else:
                                s_i = rc.imag
                                o_re = ovv[:, row * 3:(row + 1) * 3, 0, :]
                                o_im = ovv[:, row * 3:(row + 1) * 3, 1, :]
                                u_re = uvv[:, mm * 3:(mm + 1) * 3, 0, :]
                                u_im = uvv[:, mm * 3:(mm + 1) * 3, 1, :]
                                nc.vector.scalar_tensor_tensor(
                                    o_re, u_im, 0.5 * s_i, o_re, AL.mult, AL.add)
                                nc.vector.scalar_tensor_tensor(
                                    o_im, u_re, -0.5 * s_i, o_im, AL.mult, AL.add)

                # store (fp16 -> fp32 cast via SWDGE)
                nc.gpsimd.dma_start(out_dram[o], out_t[:])

    nc.finalize()
    return nc


_PROG_CACHE = {}


def _get_program():
    if 'nc' not in _PROG_CACHE:
        _PROG_CACHE['nc'] = _build_device_program()
    return _PROG_CACHE['nc']


def _sbuf_image(a, C):
    """[T, C, NSITE] -> [T, P, C, F] contiguous."""
    return np.ascontiguousarray(a.reshape(T, C, P, F).transpose(0, 2, 1, 3))


def build_in_maps(psi, U):
    link_vars = _to_planar_links(U)
    psi_vars = _to_planar_psi(psi)
    link_imgs = {k: _sbuf_image(v, 18) for k, v in link_vars.items()}
    psi_imgs = {k: _sbuf_image(v, 24 if k == ('c',) else 12)
                for k, v in psi_vars.items()}
    in_maps = []
    for core in range(NCORES):
        t0 = core * TLOC
        tw = [(t0 - 2 + w) % T for w in range(NWIN)]
        m = {}
        for k in LINK_KEYS:
            m[_lname(k)] = np.ascontiguousarray(link_imgs[k][tw])
        for k in PSI_KEYS:
            m[_pname(k)] = np.ascontiguousarray(psi_imgs[k][tw])
        in_maps.append(m)
    return in_maps


def assemble_output(results):
    out = np.empty((T, 24, NSITE), np.float32)
    for core in range(NCORES):
        r = results[core]['out']  # [TLOC, P, 24, F] fp32
        out[core * TLOC:(core + 1) * TLOC] = r.transpose(0, 2, 1, 3).reshape(TLOC, 24, NSITE)
    res = (out[:, 0::2, :] + 1j * out[:, 1::2, :]).astype(np.complex64)
    return res.transpose(0, 2, 1).reshape(T, Z, Y, X, NS, NCOL)


def kernel(psi, U):
    psi = np.asarray(psi)
    U = np.asarray(U)
    from concourse.bass_utils import run_bass_kernel_spmd
    nc = _get_program()
    in_maps = build_in_maps(psi, U)
    res = run_bass_kernel_spmd(nc, in_maps, core_ids=list(range(NCORES)))
    return assemble_output(res.results)

